# revision 1
# baseline (speedup 1.0000x reference)
"""Distributed 2-layer GCN (PyG GCNConv-style) on 8 Trainium2 NeuronCores.

Strategy (hardcoded for N=100000, E=3.2M, 512->256->128->4):
  - Nodes are degree-balanced into (ncores*W) windows of 128 nodes; window b is
    owned by core (b % ncores). A node's "global position" is its row in the
    AllGathered feature table, so gathers use plain int positions.
  - Per layer: local dense matmul (bf16 on PE, fp32 PSUM), rows pre-scaled by
    dinv, results AllGathered to a replicated bf16 feature table in DRAM.
  - Aggregation: per (window, class) block of dst-sorted edges, a dma_gather
    (custom SWDGE gather, int16 indices; the table is viewed in 4 strided
    classes of row%4 so indices fit int16) pulls source rows into SBUF; a
    one-hot S matrix built on DVE (is_equal vs iota) feeds a PE matmul
    S^T @ msgs that segment-sums into the window's PSUM accumulator.
    Padding slots carry dcol=128 which never matches iota -> contribute 0.
  - Epilogue: z = dinv*acc + b; relu; layer 2 repeats; final logits + log
    softmax (batched Ln to avoid ACT table thrash).
"""
import math
import numpy as np

import concourse.bass as bass
import concourse.mybir as mybir
import concourse.bass_utils as bass_utils
from concourse import bacc, tile
from concourse.bass_interp import get_hw_module

P = 128
F32 = mybir.dt.float32
BF16 = mybir.dt.bfloat16
I16 = mybir.dt.int16


class Cfg:
    def __init__(self, N, F_IN, H1, H2, C, ncores=8, W=None, maxt=8):
        self.N, self.F_IN, self.H1, self.H2, self.C = N, F_IN, H1, H2, C
        self.ncores = ncores
        B = ncores * P
        self.W = W if W is not None else math.ceil(N / B)
        self.NPAD = self.W * B
        assert self.NPAD >= N and self.NPAD % 4 == 0
        self.CLS = self.NPAD // 4
        assert self.CLS <= 32768
        self.KI = F_IN // P
        self.K2 = H1 // P
        self.maxt = maxt
        self.B = self.W * P  # nodes per core


FULL = Cfg(N=100000, F_IN=512, H1=256, H2=128, C=4)


# ---------------------------------------------------------------- host side
def preprocess(cfg, x, edge_index, W1, b1, W2, b2, Wl, bl):
    N, NC, W, NPAD, B = cfg.N, cfg.ncores, cfg.W, cfg.NPAD, cfg.B
    NW = NC * W

    src = np.asarray(edge_index[0], dtype=np.int64)
    dst = np.asarray(edge_index[1], dtype=np.int64)
    deg = np.bincount(dst, minlength=N).astype(np.float64) + 1.0
    dinv = (1.0 / np.sqrt(deg)).astype(np.float32)

    # node -> global position, degree-balanced across windows (snake fill)
    degall = np.zeros(NPAD, np.int64)
    degall[:N] = deg.astype(np.int64)
    order = np.argsort(-degall, kind="stable")
    i = np.arange(NPAD)
    phase = i % (2 * NW)
    binid = np.where(phase < NW, phase, 2 * NW - 1 - phase)
    by_bin = np.argsort(binid, kind="stable")
    slot = np.empty(NPAD, np.int64)
    slot[by_bin] = i % P  # within each bin, slots fill 0..127 in arrival order
    # position: core = bin % NC, window = bin // NC
    core_of_bin = binid % NC
    w_of_bin = binid // NC
    g_of_i = core_of_bin * B + w_of_bin * P + slot
    pos = np.empty(NPAD, np.int64)
    pos[order] = g_of_i

    node_at = np.empty(NPAD, np.int64)
    node_at[pos] = np.arange(NPAD)

    # edge list incl. one self edge per real node
    S_pos = np.concatenate([pos[src], pos[np.arange(N)]])
    D_pos = np.concatenate([pos[dst], pos[np.arange(N)]])
    core_e = D_pos // B
    w_e = (D_pos % B) // P
    dcol_e = (D_pos % P).astype(np.float32)
    # class tables: class = slot//32; row in class table = rank*B/4 + w*32 + slot%32
    B4 = B // 4
    s_slot = S_pos % P
    cls_e = (s_slot // 32).astype(np.int64)
    idx16_e = ((S_pos // B) * B4 + ((S_pos % B) // P) * 32
               + (s_slot % 32)).astype(np.int16)

    key = ((core_e * W + w_e) * 4 + cls_e).astype(np.int64)
    ordE = np.argsort(key, kind="stable")
    counts = np.bincount(key, minlength=NC * W * 4).reshape(NC, W, 4)
    T = np.maximum(1, np.ceil(counts / P).astype(np.int64).max(axis=0))  # [W, 4]
    TT = int(T.sum())
    SLOT = TT * P

    # ops schedule per (w, cl): list of tile counts
    ops = [[[] for _ in range(4)] for _ in range(W)]
    for w in range(W):
        for cl in range(4):
            t = int(T[w][cl])
            while t > 0:
                c = min(t, cfg.maxt)
                ops[w][cl].append(c)
                t -= c

    starts = np.zeros(NC * W * 4 + 1, np.int64)
    np.cumsum(counts.reshape(-1), out=starts[1:])
    blk_off = np.zeros((W, 4), np.int64)  # slot offset of each (w, cl) block
    acc_off = 0
    for w in range(W):
        for cl in range(4):
            blk_off[w, cl] = acc_off
            acc_off += int(T[w][cl]) * P

    idx16 = np.zeros((NC, SLOT), np.int16)
    dcol = np.full((NC, SLOT), float(P), np.float32)
    for c in range(NC):
        for w in range(W):
            for cl in range(4):
                k = (c * W + w) * 4 + cl
                s0, s1 = starts[k], starts[k + 1]
                n = s1 - s0
                off = blk_off[w, cl]
                seg = ordE[s0:s1]
                idx16[c, off:off + n] = idx16_e[seg]
                dcol[c, off:off + n] = dcol_e[seg]

    # wrap idx16 per-op: element i of an op at [i%16, i//16], replicated x8
    idx_w = np.zeros((NC, 16, SLOT // 16), np.int16)
    for w in range(W):
        for cl in range(4):
            off = int(blk_off[w, cl])
            for t_op in ops[w][cl]:
                n = t_op * P
                blk = idx16[:, off:off + n].reshape(NC, n // 16, 16)
                idx_w[:, :, off // 16:(off + n) // 16] = blk.transpose(0, 2, 1)
                off += n
    idx_rep = np.tile(idx_w, (1, 8, 1))  # [NC, 128, SLOT//16]

    dcol_t = dcol.reshape(NC, TT, P).transpose(0, 2, 1).copy()  # [NC, 128, TT]

    # x shard, transposed chunk layout: xt4[p, w, c2, m] = x[node(w*128+m), c2*128+p]
    xpad = np.zeros((NPAD, cfg.F_IN), np.float32)
    xpad[:N] = np.asarray(x, np.float32)
    dinvpad = np.ones(NPAD, np.float32)
    dinvpad[:N] = dinv

    xt4 = np.empty((NC, P, W, cfg.KI, P), np.float32)
    dinvl = np.empty((NC, P, W), np.float32)
    for c in range(NC):
        ids = node_at[c * B:(c + 1) * B]
        xl = xpad[ids]  # [B, F_IN]
        xt4[c] = xl.reshape(W, P, cfg.KI, P).transpose(3, 0, 2, 1)
        dinvl[c] = dinvpad[ids].reshape(W, P).T

    w1d = np.asarray(W1, np.float32).reshape(cfg.KI, P, cfg.H1).transpose(1, 0, 2)
    w2d = np.asarray(W2, np.float32).reshape(cfg.K2, P, cfg.H2).transpose(1, 0, 2)
    wld = np.asarray(Wl, np.float32)  # [H2=128, C]
    b1b = np.broadcast_to(np.asarray(b1, np.float32), (P, cfg.H1)).copy()
    b2b = np.broadcast_to(np.asarray(b2, np.float32), (P, cfg.H2)).copy()
    blb = np.broadcast_to(np.asarray(bl, np.float32), (P, cfg.C)).copy()
    iota = np.broadcast_to(np.arange(P, dtype=np.float32), (P, P)).copy()
    ident = np.eye(P, dtype=np.float32)

    to_bf16 = lambda a: a.astype(np.dtype("bfloat16")) if hasattr(np, "bfloat16") else a
    import ml_dtypes
    bf = lambda a: a.astype(ml_dtypes.bfloat16)

    in_maps = []
    for c in range(NC):
        in_maps.append({
            "xt4": bf(xt4[c]),
            "w1d": bf(w1d), "w2d": bf(w2d), "wld": bf(wld),
            "b1b": b1b, "b2b": b2b, "blb": blb,
            "dinvl": dinvl[c],
            "idx16": idx_rep[c],
            "dcol": dcol_t[c],
            "iota": iota,
            "ident": bf(ident),
        })

    meta = dict(T=T, ops=ops, pos=pos, node_at=node_at, SLOT=SLOT, TT=TT)
    return in_maps, meta


def assemble_output(cfg, meta, results):
    N, NC, W, C, B = cfg.N, cfg.ncores, cfg.W, cfg.C, cfg.B
    rows = []
    for c in range(NC):
        r = results[c]["outst"].reshape(P, W, C).transpose(1, 0, 2).reshape(B, C)
        rows.append(r)
    allrows = np.concatenate(rows, axis=0)  # [NPAD, C] in position order
    return allrows[meta["pos"][:N]].astype(np.float32)


# ---------------------------------------------------------------- device side
def build_kernel(cfg, T, ops, upto="full"):
    NC, W, NPAD, B = cfg.ncores, cfg.W, cfg.NPAD, cfg.B
    H1, H2, C, KI, K2 = cfg.H1, cfg.H2, cfg.C, cfg.KI, cfg.K2
    TT = int(np.asarray(T).sum())
    SLOT = TT * P

    nc = bacc.Bacc("TRN2", target_bir_lowering=False, debug=False, num_devices=NC)

    xt4 = nc.dram_tensor("xt4", [P, W, KI, P], BF16, kind="ExternalInput")
    w1d = nc.dram_tensor("w1d", [P, KI, H1], BF16, kind="ExternalInput")
    w2d = nc.dram_tensor("w2d", [P, K2, H2], BF16, kind="ExternalInput")
    wld = nc.dram_tensor("wld", [P, C], BF16, kind="ExternalInput")
    b1b = nc.dram_tensor("b1b", [P, H1], F32, kind="ExternalInput")
    b2b = nc.dram_tensor("b2b", [P, H2], F32, kind="ExternalInput")
    blb = nc.dram_tensor("blb", [P, C], F32, kind="ExternalInput")
    dinvl = nc.dram_tensor("dinvl", [P, W], F32, kind="ExternalInput")
    idx16 = nc.dram_tensor("idx16", [P, SLOT // 16], I16, kind="ExternalInput")
    dcol = nc.dram_tensor("dcol", [P, TT], F32, kind="ExternalInput")
    iota = nc.dram_tensor("iota", [P, P], F32, kind="ExternalInput")
    ident = nc.dram_tensor("ident", [P, P], BF16, kind="ExternalInput")
    outst = nc.dram_tensor("outst", [P, W * C], F32, kind="ExternalOutput")

    # per-window column ranges in idx16 / dcol
    blk_tiles = np.asarray(T)  # [W, 4]
    w_tile_off = np.zeros(W + 1, np.int64)
    np.cumsum(blk_tiles.sum(axis=1), out=w_tile_off[1:])

    rg = [list(range(NC))]

    B4 = B // 4
    with tile.TileContext(nc) as tc:
        with tc.tile_pool(name="const", bufs=1) as cpool, \
             tc.tile_pool(name="dram", bufs=1, space="DRAM") as dram:
            ag1_in = [dram.tile([B4, H1], BF16, name=f"ag1i{i}") for i in range(4)]
            ag1_out = [dram.tile([NPAD // 4, H1], BF16, addr_space="Shared",
                                 name=f"ag1o{i}") for i in range(4)]
            ag2_in = [dram.tile([B4, H2], BF16, name=f"ag2i{i}") for i in range(4)]
            ag2_out = [dram.tile([NPAD // 4, H2], BF16, addr_space="Shared",
                                 name=f"ag2o{i}") for i in range(4)]

            w1sb = cpool.tile([P, KI, H1], BF16)
            nc.sync.dma_start(w1sb[:], w1d[:])
            w2sb = cpool.tile([P, K2, H2], BF16)
            nc.sync.dma_start(w2sb[:], w2d[:])
            wlsb = cpool.tile([P, C], BF16)
            nc.sync.dma_start(wlsb[:], wld[:])
            b1sb = cpool.tile([P, H1], F32)
            nc.sync.dma_start(b1sb[:], b1b[:])
            b2sb = cpool.tile([P, H2], F32)
            nc.sync.dma_start(b2sb[:], b2b[:])
            blsb = cpool.tile([P, C], F32)
            nc.sync.dma_start(blsb[:], blb[:])
            dinvsb = cpool.tile([P, W], F32)
            nc.sync.dma_start(dinvsb[:], dinvl[:])
            iotasb = cpool.tile([P, P], F32)
            nc.sync.dma_start(iotasb[:], iota[:])
            idsb = cpool.tile([P, P], BF16)
            nc.sync.dma_start(idsb[:], ident[:])
            dcolsb = cpool.tile([P, TT], F32)
            nc.sync.dma_start(dcolsb[:], dcol[:])

            lgst = cpool.tile([P, W * C], F32)
            sst = cpool.tile([P, W], F32)
            outsb = cpool.tile([P, W * C], F32)

            # ---------------- phase A: h1' = dinv * (x @ W1), allgather
            with tc.tile_pool(name="phA", bufs=3) as sbA, \
                 tc.tile_pool(name="phA_ps", bufs=2, space="PSUM") as psA:
                for w in (range(W) if upto != "noop" else []):
                    xt = sbA.tile([P, KI, P], BF16, tag="xt")
                    nc.sync.dma_start(xt[:], xt4[:, w])
                    hp = psA.tile([P, H1], F32, tag="hp")
                    for c2 in range(KI):
                        nc.tensor.matmul(hp[:], xt[:, c2], w1sb[:, c2],
                                         start=(c2 == 0), stop=(c2 == KI - 1))
                    h1p = sbA.tile([P, H1], BF16, tag="h1p")
                    nc.scalar.activation(h1p[:], hp[:],
                                         mybir.ActivationFunctionType.Copy,
                                         scale=dinvsb[:, w:w + 1])
                    for c4 in range(4):
                        nc.sync.dma_start(ag1_in[c4][w * 32:(w + 1) * 32, :],
                                          h1p[c4 * 32:(c4 + 1) * 32, :])

            for c4 in (range(4) if upto != "noop" else []):
                if NC == 1:
                    nc.sync.dma_start(ag1_out[c4][:], ag1_in[c4][:])
                else:
                    nc.gpsimd.collective_compute(
                        "AllGather", mybir.AluOpType.bypass,
                        ins=[ag1_in[c4][:]], outs=[ag1_out[c4][:]],
                        replica_groups=rg)

            def aggregate(w, sb, sbS, ps, cls_tabs, F, tag,
                          skip_gather=False, skip_mm=False):
                """Gather + S-matmul segment-sum for window w at width F.
                Returns the PSUM accumulator tile."""
                t0 = int(w_tile_off[w])
                ntile_w = int(w_tile_off[w + 1] - w_tile_off[w])
                idxw = sb.tile([P, ntile_w * 8], I16, tag=f"idxw{tag}")
                nc.sync.dma_start(idxw[:], idx16[:, t0 * 8:(t0 + ntile_w) * 8])
                acc = ps.tile([P, F], F32, tag=f"acc{tag}")
                if skip_mm:
                    nc.vector.memset(acc[:], 0.0)
                ti = 0
                for cl in range(4):
                    for t_op in ops[w][cl]:
                        g = sb.tile([P, cfg.maxt, F], BF16, tag=f"g{tag}")
                        if skip_gather:
                            nc.vector.memset(g[:, :t_op], 0.0)
                        else:
                            nc.gpsimd.dma_gather(
                                g[:, :t_op], cls_tabs[cl][:],
                                idxw[:, ti * 8:(ti + t_op) * 8],
                                t_op * P, t_op * P, F)
                        if skip_mm:
                            ti += t_op
                            continue
                        for t in range(t_op):
                            S = sbS.tile([P, P], BF16, tag=f"S{tag}")
                            nc.vector.tensor_tensor(
                                S[:],
                                dcolsb[:, t0 + ti + t:t0 + ti + t + 1].to_broadcast([P, P]),
                                iotasb[:], op=mybir.AluOpType.is_equal)
                            nc.tensor.matmul(acc[:], S[:], g[:, t],
                                             start=(ti + t == 0),
                                             stop=(ti + t == ntile_w - 1))
                        ti += t_op
                return acc

            # ---------------- phase C/D: aggregate layer1, h2' = dinv*(a1@W2)
            if upto.startswith("CD") or upto == "full":
                with tc.tile_pool(name="phC", bufs=3) as sbC, \
                     tc.tile_pool(name="phC_s", bufs=4) as sbS, \
                     tc.tile_pool(name="phC_ps", bufs=2, space="PSUM") as psC, \
                     tc.tile_pool(name="phD_ps", bufs=2, space="PSUM") as psD:
                    for w in range(W):
                        if upto == "CD_noagg":
                            acc = psC.tile([P, H1], F32, tag="acc1")
                            nc.vector.memset(acc[:], 0.0)
                        else:
                            acc = aggregate(w, sbC, sbS, psC, ag1_out, H1, "1",
                                            skip_gather=(upto == "CD_nogather"),
                                            skip_mm=(upto == "CD_nomm"))
                        z = sbC.tile([P, H1], F32, tag="z")
                        nc.vector.tensor_scalar_mul(z[:], acc[:], dinvsb[:, w:w + 1])
                        z2 = sbC.tile([P, H1], F32, tag="z2")
                        nc.vector.tensor_tensor(z2[:], z[:], b1sb[:],
                                                op=mybir.AluOpType.add)
                        a1 = sbC.tile([P, H1], BF16, tag="a1")
                        nc.scalar.activation(a1[:], z2[:],
                                             mybir.ActivationFunctionType.Relu)
                        if upto == "CD_nod":
                            for c4 in range(4):
                                nc.sync.dma_start(
                                    ag2_in[c4][w * 32:(w + 1) * 32, :],
                                    a1[c4 * 32:(c4 + 1) * 32, :H2])
                            continue
                        h2p = psD.tile([P, H2], F32, tag="h2p")
                        for c2 in range(K2):
                            a1tp = psD.tile([P, P], BF16, tag="a1tp")
                            nc.tensor.transpose(a1tp[:], a1[:, c2 * P:(c2 + 1) * P],
                                                idsb[:])
                            a1t = sbC.tile([P, P], BF16, tag="a1t")
                            nc.vector.tensor_copy(a1t[:], a1tp[:])
                            nc.tensor.matmul(h2p[:], a1t[:], w2sb[:, c2],
                                             start=(c2 == 0), stop=(c2 == K2 - 1))
                        h2pp = sbC.tile([P, H2], BF16, tag="h2pp")
                        nc.scalar.activation(h2pp[:], h2p[:],
                                             mybir.ActivationFunctionType.Copy,
                                             scale=dinvsb[:, w:w + 1])
                        for c4 in range(4):
                            nc.sync.dma_start(ag2_in[c4][w * 32:(w + 1) * 32, :],
                                              h2pp[c4 * 32:(c4 + 1) * 32, :])

                for c4 in range(4):
                    if NC == 1:
                        nc.sync.dma_start(ag2_out[c4][:], ag2_in[c4][:])
                    else:
                        nc.gpsimd.collective_compute(
                            "AllGather", mybir.AluOpType.bypass,
                            ins=[ag2_in[c4][:]], outs=[ag2_out[c4][:]],
                            replica_groups=rg)

            # ---------------- phase E/F: aggregate layer2, logits, log_softmax
            if upto == "full":
                with tc.tile_pool(name="phE", bufs=3) as sbE, \
                     tc.tile_pool(name="phE_s", bufs=4) as sbS2, \
                     tc.tile_pool(name="phE_ps", bufs=2, space="PSUM") as psE, \
                     tc.tile_pool(name="phL_ps", bufs=2, space="PSUM") as psL:
                    for w in range(W):
                        acc = aggregate(w, sbE, sbS2, psE, ag2_out, H2, "2")
                        z = sbE.tile([P, H2], F32, tag="ze")
                        nc.vector.tensor_scalar_mul(z[:], acc[:], dinvsb[:, w:w + 1])
                        z2 = sbE.tile([P, H2], F32, tag="z2e")
                        nc.vector.tensor_tensor(z2[:], z[:], b2sb[:],
                                                op=mybir.AluOpType.add)
                        a2 = sbE.tile([P, H2], BF16, tag="a2")
                        nc.scalar.activation(a2[:], z2[:],
                                             mybir.ActivationFunctionType.Relu)
                        a2tp = psL.tile([P, P], BF16, tag="a2tp")
                        nc.tensor.transpose(a2tp[:], a2[:], idsb[:])
                        a2t = sbE.tile([P, P], BF16, tag="a2t")
                        nc.vector.tensor_copy(a2t[:], a2tp[:])
                        lg = psL.tile([P, C], F32, tag="lg")
                        nc.tensor.matmul(lg[:], a2t[:], wlsb[:], start=True, stop=True)
                        nc.vector.tensor_tensor(lgst[:, w * C:(w + 1) * C], lg[:],
                                                blsb[:], op=mybir.AluOpType.add)
                        e = sbE.tile([P, C], F32, tag="e")
                        nc.scalar.activation(e[:], lgst[:, w * C:(w + 1) * C],
                                             mybir.ActivationFunctionType.Exp,
                                             accum_out=sst[:, w:w + 1])
                    lns = cpool.tile([P, W], F32)
                    nc.scalar.activation(lns[:], sst[:],
                                         mybir.ActivationFunctionType.Ln)
                    for w in range(W):
                        nc.vector.tensor_scalar(
                            outsb[:, w * C:(w + 1) * C], lgst[:, w * C:(w + 1) * C],
                            lns[:, w:w + 1], None, op0=mybir.AluOpType.subtract)
                    nc.sync.dma_start(outst[:], outsb[:])
            else:
                # debug variants: dummy output proving the kept phases ran
                nc.vector.memset(outsb[:], 0.0)
                if upto != "noop":
                    probe_src = ag1_out[0] if upto == "A" else ag2_out[0]
                    probe = cpool.tile([P, C], BF16)
                    nc.sync.dma_start(probe[:], probe_src[:P, :C])
                    nc.vector.tensor_copy(outsb[:, :C], probe[:])
                nc.sync.dma_start(outst[:], outsb[:])

    nc.compile()
    return nc

# ---------------------------------------------------------------- entry point
_CACHE = {}


def _get_compiled(cfg, key, T, ops):
    if key not in _CACHE:
        nc = build_kernel(cfg, T, ops)
        nc.m = get_hw_module(nc.m)
        _CACHE[key] = nc
    return _CACHE[key]


def run(cfg, inputs):
    in_maps, meta = preprocess(cfg, **inputs)
    key = (cfg.N, cfg.F_IN, meta["TT"])
    nc = _get_compiled(cfg, key, meta["T"], meta["ops"])
    res = bass_utils.run_bass_kernel_spmd(
        nc, in_maps, core_ids=list(range(cfg.ncores)))
    out = assemble_output(cfg, meta, res.results)
    return out, res


class _TimedRunner:
    """PJRT runner mirroring bass2jax.run_bass_via_pjrt's multi-core branch,
    but with a cached jit and device-resident inputs for repeatable timing."""

    def __init__(self, nc, n_cores):
        import jax
        import concourse.mybir as mb
        from concourse import bass2jax
        from jax.sharding import Mesh, PartitionSpec, NamedSharding
        from jax.experimental.shard_map import shard_map

        bass2jax.install_neuronx_cc_hook()
        partition_name = (nc.partition_id_tensor.name
                          if nc.partition_id_tensor else None)
        in_names, out_names, out_avals, zero_shapes = [], [], [], []
        for alloc in nc.m.functions[0].allocations:
            if not isinstance(alloc, mb.MemoryLocationSet):
                continue
            name = alloc.memorylocations[0].name
            if alloc.kind == "ExternalInput":
                if name != partition_name:
                    in_names.append(name)
            elif alloc.kind == "ExternalOutput":
                out_names.append(name)
                shape = tuple(alloc.tensor_shape)
                dtype = mb.dt.np(alloc.dtype)
                out_avals.append(jax.core.ShapedArray(shape, dtype))
                zero_shapes.append((shape, dtype))
        n_params = len(in_names)
        all_in_names = list(in_names) + list(out_names)
        if partition_name is not None:
            all_in_names.append(partition_name)
        donate = tuple(range(n_params, n_params + len(out_names)))

        def _body(*args):
            operands = list(args)
            if partition_name is not None:
                operands.append(bass2jax.partition_id_tensor())
            outs = bass2jax._bass_exec_p.bind(
                *operands,
                out_avals=tuple(out_avals),
                in_names=tuple(all_in_names),
                out_names=tuple(out_names),
                lowering_input_output_aliases=(),
                sim_require_finite=True,
                sim_require_nnan=True,
                nc=nc,
            )
            return tuple(outs)

        devices = jax.devices()[:n_cores]
        mesh = Mesh(np.asarray(devices), ("core",))
        in_specs = (PartitionSpec("core"),) * (n_params + len(out_names))
        out_specs = (PartitionSpec("core"),) * len(out_names)
        self.fn = jax.jit(
            shard_map(_body, mesh=mesh, in_specs=in_specs,
                      out_specs=out_specs, check_rep=False),
            donate_argnums=donate, keep_unused=True)
        self.jax = jax
        self.mesh = mesh
        self.sharding = NamedSharding(mesh, PartitionSpec("core"))
        self.in_names = in_names
        self.out_names = out_names
        self.zero_shapes = zero_shapes
        self.n_cores = n_cores
        self.dev_inputs = None

    def stage_inputs(self, in_maps):
        concat_in = [
            np.concatenate([np.asarray(in_maps[c][n])
                            for c in range(self.n_cores)], axis=0)
            for n in self.in_names
        ]
        self.dev_inputs = [self.jax.device_put(a, self.sharding)
                           for a in concat_in]
        for a in self.dev_inputs:
            a.block_until_ready()

    def exec_once(self):
        import time
        zeros = [np.zeros((self.n_cores * s[0], *s[1:]), d)
                 for s, d in self.zero_shapes]
        dz = [self.jax.device_put(z, self.sharding) for z in zeros]
        for z in dz:
            z.block_until_ready()
        t0 = time.perf_counter()
        outs = self.fn(*self.dev_inputs, *dz)
        for o in outs:
            o.block_until_ready()
        t1 = time.perf_counter()
        return outs, t1 - t0

    def results(self, outs):
        res = []
        for c in range(self.n_cores):
            m = {}
            for i, n in enumerate(self.out_names):
                full = np.asarray(outs[i])
                per = full.reshape(self.n_cores, -1, *full.shape[1:])[c]
                m[n] = per
            res.append(m)
        return res


def run_timed(cfg, inputs, iters=3):
    in_maps, meta = preprocess(cfg, **inputs)
    key = (cfg.N, cfg.F_IN, meta["TT"])
    nc = _get_compiled(cfg, key, meta["T"], meta["ops"])
    rkey = ("runner",) + key
    if rkey not in _CACHE:
        _CACHE[rkey] = _TimedRunner(nc, cfg.ncores)
    runner = _CACHE[rkey]
    runner.stage_inputs(in_maps)
    times = []
    outs = None
    for _ in range(iters):
        outs, dt = runner.exec_once()
        times.append(dt)
    results = runner.results(outs)
    out = assemble_output(cfg, meta, results)
    return out, times


def kernel(x, edge_index, W1, b1, W2, b2, Wl, bl):
    out, _ = run(FULL, dict(x=x, edge_index=edge_index, W1=W1, b1=b1,
                            W2=W2, b2=b2, Wl=Wl, bl=bl))
    return out



# revision 11
# speedup vs baseline: 11.2097x; 11.2097x over previous
"""Distributed 2-layer GCN (PyG GCNConv-style) on 8 Trainium2 NeuronCores.

Strategy (hardcoded for N=100000, E=3.2M, 512->256->128->4):
  - Nodes are degree-balanced into (ncores*W) windows of 128 nodes; window b is
    owned by core (b % ncores). A node's "global position" is its row in the
    AllGathered feature table, so gathers use plain int positions.
  - Per layer: local dense matmul (bf16 on PE, fp32 PSUM), rows pre-scaled by
    dinv, results AllGathered to a replicated bf16 feature table in DRAM.
  - Aggregation: per (window, class) block of dst-sorted edges, a dma_gather
    (custom SWDGE gather, int16 indices; the table is viewed in 4 strided
    classes of row%4 so indices fit int16) pulls source rows into SBUF; a
    one-hot S matrix built on DVE (is_equal vs iota, 4 tiles per op, bf16)
    feeds a PE matmul S^T @ msgs that segment-sums into the window's PSUM
    accumulator. Gathers stripe across 4 SWDGE queues (2.3x faster Q7
    descriptor generation). Padding slots carry dcol=128 which never
    matches iota -> contribute 0.
  - Self loops are folded into the epilogue: agg = dinv*(acc + h'own) + b
    with h' windows retained in SBUF (saves ~100K gather descriptors).
  - Epilogue: relu; layer 2 repeats; final logits + log softmax.
"""
import math
import numpy as np

import concourse.bass as bass
import concourse.mybir as mybir
import concourse.bass_utils as bass_utils
from concourse import bacc, tile
from concourse.bass_interp import get_hw_module

P = 128
F32 = mybir.dt.float32
BF16 = mybir.dt.bfloat16
I16 = mybir.dt.int16


class Cfg:
    def __init__(self, N, F_IN, H1, H2, C, ncores=8, W=None, maxt=8):
        self.N, self.F_IN, self.H1, self.H2, self.C = N, F_IN, H1, H2, C
        self.ncores = ncores
        B = ncores * P
        self.W = W if W is not None else math.ceil(N / B)
        self.NPAD = self.W * B
        assert self.NPAD >= N and self.NPAD % 4 == 0
        self.CLS = self.NPAD // 4
        assert self.CLS <= 32768
        self.KI = F_IN // P
        self.K2 = H1 // P
        self.maxt = maxt
        self.B = self.W * P  # nodes per core


FULL = Cfg(N=100000, F_IN=512, H1=256, H2=128, C=4)


# ---------------------------------------------------------------- host side
def preprocess(cfg, x, edge_index, W1, b1, W2, b2, Wl, bl):
    N, NC, W, NPAD, B = cfg.N, cfg.ncores, cfg.W, cfg.NPAD, cfg.B
    NW = NC * W

    src = np.asarray(edge_index[0], dtype=np.int64)
    dst = np.asarray(edge_index[1], dtype=np.int64)
    deg = np.bincount(dst, minlength=N).astype(np.float64) + 1.0
    dinv = (1.0 / np.sqrt(deg)).astype(np.float32)

    # node -> global position, degree-balanced across windows (snake fill)
    degall = np.zeros(NPAD, np.int64)
    degall[:N] = deg.astype(np.int64)
    order = np.argsort(-degall, kind="stable")
    i = np.arange(NPAD)
    phase = i % (2 * NW)
    binid = np.where(phase < NW, phase, 2 * NW - 1 - phase)
    by_bin = np.argsort(binid, kind="stable")
    slot = np.empty(NPAD, np.int64)
    slot[by_bin] = i % P  # within each bin, slots fill 0..127 in arrival order
    # position: core = bin % NC, window = bin // NC
    core_of_bin = binid % NC
    w_of_bin = binid // NC
    g_of_i = core_of_bin * B + w_of_bin * P + slot
    pos = np.empty(NPAD, np.int64)
    pos[order] = g_of_i

    node_at = np.empty(NPAD, np.int64)
    node_at[pos] = np.arange(NPAD)

    # edge list; self loops are folded into the epilogue on-device
    S_pos = pos[src]
    D_pos = pos[dst]
    core_e = D_pos // B
    w_e = (D_pos % B) // P
    dcol_e = (D_pos % P).astype(np.float32)
    # class tables: class = slot//32; row in class table = rank*B/4 + w*32 + slot%32
    B4 = B // 4
    s_slot = S_pos % P
    cls_e = (s_slot // 32).astype(np.int64)
    idx16_e = ((S_pos // B) * B4 + ((S_pos % B) // P) * 32
               + (s_slot % 32)).astype(np.int16)

    key = ((core_e * W + w_e) * 4 + cls_e).astype(np.int64)
    ordE = np.argsort(key, kind="stable")
    counts = np.bincount(key, minlength=NC * W * 4).reshape(NC, W, 4)
    T = np.maximum(1, np.ceil(counts / P).astype(np.int64).max(axis=0))  # [W, 4]
    TT = int(T.sum())
    SLOT = TT * P

    # ops schedule per (w, cl): list of tile counts
    ops = [[[] for _ in range(4)] for _ in range(W)]
    for w in range(W):
        for cl in range(4):
            t = int(T[w][cl])
            while t > 0:
                c = min(t, cfg.maxt)
                ops[w][cl].append(c)
                t -= c

    starts = np.zeros(NC * W * 4 + 1, np.int64)
    np.cumsum(counts.reshape(-1), out=starts[1:])
    blk_off = np.zeros((W, 4), np.int64)  # slot offset of each (w, cl) block
    acc_off = 0
    for w in range(W):
        for cl in range(4):
            blk_off[w, cl] = acc_off
            acc_off += int(T[w][cl]) * P

    idx16 = np.zeros((NC, SLOT), np.int16)
    dcol = np.full((NC, SLOT), float(P), np.float32)  # cast bf16 at ship time
    for c in range(NC):
        for w in range(W):
            for cl in range(4):
                k = (c * W + w) * 4 + cl
                s0, s1 = starts[k], starts[k + 1]
                n = s1 - s0
                off = blk_off[w, cl]
                seg = ordE[s0:s1]
                idx16[c, off:off + n] = idx16_e[seg]
                dcol[c, off:off + n] = dcol_e[seg]

    # wrap idx16 per-op: element i of an op at [i%16, i//16], replicated x8
    idx_w = np.zeros((NC, 16, SLOT // 16), np.int16)
    for w in range(W):
        for cl in range(4):
            off = int(blk_off[w, cl])
            for t_op in ops[w][cl]:
                n = t_op * P
                blk = idx16[:, off:off + n].reshape(NC, n // 16, 16)
                idx_w[:, :, off // 16:(off + n) // 16] = blk.transpose(0, 2, 1)
                off += n
    idx_rep = np.tile(idx_w, (1, 8, 1))  # [NC, 128, SLOT//16]

    dcol_t = dcol.reshape(NC, TT, P).transpose(0, 2, 1).copy()  # [NC, 128, TT]

    # x shard, transposed chunk layout: xt4[p, w, c2, m] = x[node(w*128+m), c2*128+p]
    xpad = np.zeros((NPAD, cfg.F_IN), np.float32)
    xpad[:N] = np.asarray(x, np.float32)
    dinvpad = np.ones(NPAD, np.float32)
    dinvpad[:N] = dinv

    xt4 = np.empty((NC, P, W, cfg.KI, P), np.float32)
    dinvl = np.empty((NC, P, W), np.float32)
    for c in range(NC):
        ids = node_at[c * B:(c + 1) * B]
        xl = xpad[ids]  # [B, F_IN]
        xt4[c] = xl.reshape(W, P, cfg.KI, P).transpose(3, 0, 2, 1)
        dinvl[c] = dinvpad[ids].reshape(W, P).T

    w1d = np.asarray(W1, np.float32).reshape(cfg.KI, P, cfg.H1).transpose(1, 0, 2)
    w2d = np.asarray(W2, np.float32).reshape(cfg.K2, P, cfg.H2).transpose(1, 0, 2)
    wld = np.asarray(Wl, np.float32)  # [H2=128, C]
    b1b = np.broadcast_to(np.asarray(b1, np.float32), (P, cfg.H1)).copy()
    b2b = np.broadcast_to(np.asarray(b2, np.float32), (P, cfg.H2)).copy()
    blb = np.broadcast_to(np.asarray(bl, np.float32), (P, cfg.C)).copy()
    iota4 = np.broadcast_to(np.arange(P, dtype=np.float32),
                            (P, 4, P)).reshape(P, 4 * P).copy()
    ident = np.eye(P, dtype=np.float32)

    import ml_dtypes
    bf = lambda a: a.astype(ml_dtypes.bfloat16)

    in_maps = []
    for c in range(NC):
        in_maps.append({
            "xt4": bf(xt4[c]),
            "w1d": bf(w1d), "w2d": bf(w2d), "wld": bf(wld),
            "b1b": b1b, "b2b": b2b, "blb": blb,
            "dinvl": dinvl[c],
            "idx16": idx_rep[c],
            "dcol": bf(dcol_t[c]),
            "iota": bf(iota4),
            "ident": bf(ident),
        })

    meta = dict(T=T, ops=ops, pos=pos, node_at=node_at, SLOT=SLOT, TT=TT)
    return in_maps, meta


def assemble_output(cfg, meta, results):
    N, NC, W, C, B = cfg.N, cfg.ncores, cfg.W, cfg.C, cfg.B
    rows = []
    for c in range(NC):
        r = results[c]["outst"].reshape(P, W, C).transpose(1, 0, 2).reshape(B, C)
        rows.append(r)
    allrows = np.concatenate(rows, axis=0)  # [NPAD, C] in position order
    return allrows[meta["pos"][:N]].astype(np.float32)


# ---------------------------------------------------------------- device side
def build_kernel(cfg, T, ops, upto="full"):
    NC, W, NPAD, B = cfg.ncores, cfg.W, cfg.NPAD, cfg.B
    H1, H2, C, KI, K2 = cfg.H1, cfg.H2, cfg.C, cfg.KI, cfg.K2
    TT = int(np.asarray(T).sum())
    SLOT = TT * P

    nc = bacc.Bacc("TRN2", target_bir_lowering=False, debug=False,
                   num_devices=NC, num_swdge_queues=4)

    xt4 = nc.dram_tensor("xt4", [P, W, KI, P], BF16, kind="ExternalInput")
    w1d = nc.dram_tensor("w1d", [P, KI, H1], BF16, kind="ExternalInput")
    w2d = nc.dram_tensor("w2d", [P, K2, H2], BF16, kind="ExternalInput")
    wld = nc.dram_tensor("wld", [P, C], BF16, kind="ExternalInput")
    b1b = nc.dram_tensor("b1b", [P, H1], F32, kind="ExternalInput")
    b2b = nc.dram_tensor("b2b", [P, H2], F32, kind="ExternalInput")
    blb = nc.dram_tensor("blb", [P, C], F32, kind="ExternalInput")
    dinvl = nc.dram_tensor("dinvl", [P, W], F32, kind="ExternalInput")
    idx16 = nc.dram_tensor("idx16", [P, SLOT // 16], I16, kind="ExternalInput")
    dcol = nc.dram_tensor("dcol", [P, TT], BF16, kind="ExternalInput")
    iota = nc.dram_tensor("iota", [P, 4 * P], BF16, kind="ExternalInput")
    ident = nc.dram_tensor("ident", [P, P], BF16, kind="ExternalInput")
    outst = nc.dram_tensor("outst", [P, W * C], F32, kind="ExternalOutput")

    # per-window column ranges in idx16 / dcol
    blk_tiles = np.asarray(T)  # [W, 4]
    w_tile_off = np.zeros(W + 1, np.int64)
    np.cumsum(blk_tiles.sum(axis=1), out=w_tile_off[1:])

    rg = [list(range(NC))]

    B4 = B // 4
    with tile.TileContext(nc) as tc:
        with tc.tile_pool(name="const", bufs=1) as cpool, \
             tc.tile_pool(name="dram", bufs=1, space="DRAM") as dram:
            ag1_in = [dram.tile([B4, H1], BF16, name=f"ag1i{i}") for i in range(4)]
            ag1_out = [dram.tile([NPAD // 4, H1], BF16, addr_space="Shared",
                                 name=f"ag1o{i}") for i in range(4)]
            ag2_in = [dram.tile([B4, H2], BF16, name=f"ag2i{i}") for i in range(4)]
            ag2_out = [dram.tile([NPAD // 4, H2], BF16, addr_space="Shared",
                                 name=f"ag2o{i}") for i in range(4)]

            w1sb = cpool.tile([P, KI, H1], BF16)
            nc.sync.dma_start(w1sb[:], w1d[:])
            w2sb = cpool.tile([P, K2, H2], BF16)
            nc.sync.dma_start(w2sb[:], w2d[:])
            wlsb = cpool.tile([P, C], BF16)
            nc.sync.dma_start(wlsb[:], wld[:])
            b1sb = cpool.tile([P, H1], F32)
            nc.sync.dma_start(b1sb[:], b1b[:])
            b2sb = cpool.tile([P, H2], F32)
            nc.sync.dma_start(b2sb[:], b2b[:])
            blsb = cpool.tile([P, C], F32)
            nc.sync.dma_start(blsb[:], blb[:])
            dinvsb = cpool.tile([P, W], F32)
            nc.sync.dma_start(dinvsb[:], dinvl[:])
            iotasb = cpool.tile([P, 4, P], BF16)
            nc.sync.dma_start(iotasb[:], iota[:].rearrange("p (a b) -> p a b", a=4))
            idsb = cpool.tile([P, P], BF16)
            nc.sync.dma_start(idsb[:], ident[:])
            dcolsb = cpool.tile([P, TT], BF16)
            nc.sync.dma_start(dcolsb[:], dcol[:])

            lgst = cpool.tile([P, W * C], F32)
            sst = cpool.tile([P, W], F32)
            outsb = cpool.tile([P, W * C], F32)
            # retained h' windows for the self-loop epilogue term
            h1buf = cpool.tile([P, W, H1], BF16)
            h2buf = cpool.tile([P, W, H2], BF16)

            # ---------------- phase A: h1' = dinv * (x @ W1), allgather
            with tc.tile_pool(name="phA", bufs=3) as sbA, \
                 tc.tile_pool(name="phA_ps", bufs=2, space="PSUM") as psA:
                for w in (range(W) if upto != "noop" else []):
                    xt = sbA.tile([P, KI, P], BF16, tag="xt")
                    nc.sync.dma_start(xt[:], xt4[:, w])
                    hp = psA.tile([P, H1], F32, tag="hp")
                    for c2 in range(KI):
                        nc.tensor.matmul(hp[:], xt[:, c2], w1sb[:, c2],
                                         start=(c2 == 0), stop=(c2 == KI - 1))
                    nc.scalar.activation(h1buf[:, w], hp[:],
                                         mybir.ActivationFunctionType.Copy,
                                         scale=dinvsb[:, w:w + 1])
                    for c4 in range(4):
                        nc.sync.dma_start(ag1_in[c4][w * 32:(w + 1) * 32, :],
                                          h1buf[c4 * 32:(c4 + 1) * 32, w])

            for c4 in (range(4) if upto != "noop" else []):
                if NC == 1:
                    nc.sync.dma_start(ag1_out[c4][:], ag1_in[c4][:])
                else:
                    nc.gpsimd.collective_compute(
                        "AllGather", mybir.AluOpType.bypass,
                        ins=[ag1_in[c4][:]], outs=[ag1_out[c4][:]],
                        replica_groups=rg)

            qctr = [0]

            def aggregate(w, sb, sbS, ps, cls_tabs, F, tag):
                """Gather + S-matmul segment-sum for window w at width F.
                Returns the PSUM accumulator tile."""
                t0 = int(w_tile_off[w])
                ntile_w = int(w_tile_off[w + 1] - w_tile_off[w])
                idxw = sb.tile([P, ntile_w * 8], I16, tag=f"idxw{tag}")
                nc.sync.dma_start(idxw[:], idx16[:, t0 * 8:(t0 + ntile_w) * 8])
                acc = ps.tile([P, F], F32, tag=f"acc{tag}")
                # batched one-hot builds: 4 S tiles per DVE op
                stiles = []
                for bi in range(0, ntile_w, 4):
                    k = min(4, ntile_w - bi)
                    S4 = sbS.tile([P, 4, P], BF16, tag=f"S{tag}")
                    nc.vector.tensor_tensor(
                        S4[:, :k],
                        dcolsb[:, t0 + bi:t0 + bi + k].to_broadcast([P, k, P]),
                        iotasb[:, :k], op=mybir.AluOpType.is_equal)
                    for j in range(k):
                        stiles.append((S4, j))
                ti = 0
                for cl in range(4):
                    for t_op in ops[w][cl]:
                        g = sb.tile([P, cfg.maxt, F], BF16, tag=f"g{tag}")
                        nc.gpsimd.dma_gather(
                            g[:, :t_op], cls_tabs[cl][:],
                            idxw[:, ti * 8:(ti + t_op) * 8],
                            t_op * P, t_op * P, F,
                            queue_num=qctr[0] % 4)
                        qctr[0] += 1
                        for t in range(t_op):
                            S4, j = stiles[ti + t]
                            nc.tensor.matmul(acc[:], S4[:, j], g[:, t],
                                             start=(ti + t == 0),
                                             stop=(ti + t == ntile_w - 1))
                        ti += t_op
                return acc

            # ---------------- phase C/D: aggregate layer1, h2' = dinv*(a1@W2)
            if upto.startswith("CD") or upto == "full":
                with tc.tile_pool(name="phC", bufs=4) as sbC, \
                     tc.tile_pool(name="phC_s", bufs=6) as sbS, \
                     tc.tile_pool(name="phC_ps", bufs=2, space="PSUM") as psC, \
                     tc.tile_pool(name="phD_ps", bufs=2, space="PSUM") as psD:
                    for w in range(W):
                        acc = aggregate(w, sbC, sbS, psC, ag1_out, H1, "1")
                        # self loop: agg = dinv*(acc + h1') ; then + b, relu
                        zs = sbC.tile([P, H1], F32, tag="zs")
                        nc.vector.tensor_tensor(zs[:], acc[:], h1buf[:, w],
                                                op=mybir.AluOpType.add)
                        z = sbC.tile([P, H1], F32, tag="z")
                        nc.vector.tensor_scalar_mul(z[:], zs[:], dinvsb[:, w:w + 1])
                        z2 = sbC.tile([P, H1], F32, tag="z2")
                        nc.vector.tensor_tensor(z2[:], z[:], b1sb[:],
                                                op=mybir.AluOpType.add)
                        a1 = sbC.tile([P, H1], BF16, tag="a1")
                        nc.scalar.activation(a1[:], z2[:],
                                             mybir.ActivationFunctionType.Relu)
                        h2p = psD.tile([P, H2], F32, tag="h2p")
                        for c2 in range(K2):
                            a1tp = psD.tile([P, P], BF16, tag="a1tp")
                            nc.tensor.transpose(a1tp[:], a1[:, c2 * P:(c2 + 1) * P],
                                                idsb[:])
                            a1t = sbC.tile([P, P], BF16, tag="a1t")
                            nc.vector.tensor_copy(a1t[:], a1tp[:])
                            nc.tensor.matmul(h2p[:], a1t[:], w2sb[:, c2],
                                             start=(c2 == 0), stop=(c2 == K2 - 1))
                        nc.scalar.activation(h2buf[:, w], h2p[:],
                                             mybir.ActivationFunctionType.Copy,
                                             scale=dinvsb[:, w:w + 1])
                        for c4 in range(4):
                            nc.sync.dma_start(ag2_in[c4][w * 32:(w + 1) * 32, :],
                                              h2buf[c4 * 32:(c4 + 1) * 32, w])

                for c4 in range(4):
                    if NC == 1:
                        nc.sync.dma_start(ag2_out[c4][:], ag2_in[c4][:])
                    else:
                        nc.gpsimd.collective_compute(
                            "AllGather", mybir.AluOpType.bypass,
                            ins=[ag2_in[c4][:]], outs=[ag2_out[c4][:]],
                            replica_groups=rg)

            # ---------------- phase E/F: aggregate layer2, logits, log_softmax
            if upto == "full":
                with tc.tile_pool(name="phE", bufs=4) as sbE, \
                     tc.tile_pool(name="phE_s", bufs=6) as sbS2, \
                     tc.tile_pool(name="phE_ps", bufs=2, space="PSUM") as psE, \
                     tc.tile_pool(name="phL_ps", bufs=2, space="PSUM") as psL:
                    for w in range(W):
                        acc = aggregate(w, sbE, sbS2, psE, ag2_out, H2, "2")
                        zs = sbE.tile([P, H2], F32, tag="zse")
                        nc.vector.tensor_tensor(zs[:], acc[:], h2buf[:, w],
                                                op=mybir.AluOpType.add)
                        z = sbE.tile([P, H2], F32, tag="ze")
                        nc.vector.tensor_scalar_mul(z[:], zs[:], dinvsb[:, w:w + 1])
                        z2 = sbE.tile([P, H2], F32, tag="z2e")
                        nc.vector.tensor_tensor(z2[:], z[:], b2sb[:],
                                                op=mybir.AluOpType.add)
                        a2 = sbE.tile([P, H2], BF16, tag="a2")
                        nc.scalar.activation(a2[:], z2[:],
                                             mybir.ActivationFunctionType.Relu)
                        a2tp = psL.tile([P, P], BF16, tag="a2tp")
                        nc.tensor.transpose(a2tp[:], a2[:], idsb[:])
                        a2t = sbE.tile([P, P], BF16, tag="a2t")
                        nc.vector.tensor_copy(a2t[:], a2tp[:])
                        lg = psL.tile([P, C], F32, tag="lg")
                        nc.tensor.matmul(lg[:], a2t[:], wlsb[:], start=True, stop=True)
                        nc.vector.tensor_tensor(lgst[:, w * C:(w + 1) * C], lg[:],
                                                blsb[:], op=mybir.AluOpType.add)
                        e = sbE.tile([P, C], F32, tag="e")
                        nc.scalar.activation(e[:], lgst[:, w * C:(w + 1) * C],
                                             mybir.ActivationFunctionType.Exp,
                                             accum_out=sst[:, w:w + 1])
                    lns = cpool.tile([P, W], F32)
                    nc.scalar.activation(lns[:], sst[:],
                                         mybir.ActivationFunctionType.Ln)
                    for w in range(W):
                        nc.vector.tensor_scalar(
                            outsb[:, w * C:(w + 1) * C], lgst[:, w * C:(w + 1) * C],
                            lns[:, w:w + 1], None, op0=mybir.AluOpType.subtract)
                    nc.sync.dma_start(outst[:], outsb[:])
            else:
                # debug variants: dummy output proving the kept phases ran
                nc.vector.memset(outsb[:], 0.0)
                if upto != "noop":
                    probe_src = ag1_out[0] if upto == "A" else ag2_out[0]
                    probe = cpool.tile([P, C], BF16)
                    nc.sync.dma_start(probe[:], probe_src[:P, :C])
                    nc.vector.tensor_copy(outsb[:, :C], probe[:])
                nc.sync.dma_start(outst[:], outsb[:])

    nc.compile()
    return nc

# ---------------------------------------------------------------- entry point
_CACHE = {}


def _get_compiled(cfg, key, T, ops):
    if key not in _CACHE:
        nc = build_kernel(cfg, T, ops)
        nc.m = get_hw_module(nc.m)
        _CACHE[key] = nc
    return _CACHE[key]


def run(cfg, inputs):
    in_maps, meta = preprocess(cfg, **inputs)
    key = (cfg.N, cfg.F_IN, meta["TT"])
    nc = _get_compiled(cfg, key, meta["T"], meta["ops"])
    res = bass_utils.run_bass_kernel_spmd(
        nc, in_maps, core_ids=list(range(cfg.ncores)))
    out = assemble_output(cfg, meta, res.results)
    return out, res


class _TimedRunner:
    """PJRT runner mirroring bass2jax.run_bass_via_pjrt's multi-core branch,
    but with a cached jit and device-resident inputs for repeatable timing."""

    def __init__(self, nc, n_cores):
        import jax
        import concourse.mybir as mb
        from concourse import bass2jax
        from jax.sharding import Mesh, PartitionSpec, NamedSharding
        from jax.experimental.shard_map import shard_map

        bass2jax.install_neuronx_cc_hook()
        partition_name = (nc.partition_id_tensor.name
                          if nc.partition_id_tensor else None)
        in_names, out_names, out_avals, zero_shapes = [], [], [], []
        for alloc in nc.m.functions[0].allocations:
            if not isinstance(alloc, mb.MemoryLocationSet):
                continue
            name = alloc.memorylocations[0].name
            if alloc.kind == "ExternalInput":
                if name != partition_name:
                    in_names.append(name)
            elif alloc.kind == "ExternalOutput":
                out_names.append(name)
                shape = tuple(alloc.tensor_shape)
                dtype = mb.dt.np(alloc.dtype)
                out_avals.append(jax.core.ShapedArray(shape, dtype))
                zero_shapes.append((shape, dtype))
        n_params = len(in_names)
        all_in_names = list(in_names) + list(out_names)
        if partition_name is not None:
            all_in_names.append(partition_name)
        donate = tuple(range(n_params, n_params + len(out_names)))

        def _body(*args):
            operands = list(args)
            if partition_name is not None:
                operands.append(bass2jax.partition_id_tensor())
            outs = bass2jax._bass_exec_p.bind(
                *operands,
                out_avals=tuple(out_avals),
                in_names=tuple(all_in_names),
                out_names=tuple(out_names),
                lowering_input_output_aliases=(),
                sim_require_finite=True,
                sim_require_nnan=True,
                nc=nc,
            )
            return tuple(outs)

        devices = jax.devices()[:n_cores]
        mesh = Mesh(np.asarray(devices), ("core",))
        in_specs = (PartitionSpec("core"),) * (n_params + len(out_names))
        out_specs = (PartitionSpec("core"),) * len(out_names)
        self.fn = jax.jit(
            shard_map(_body, mesh=mesh, in_specs=in_specs,
                      out_specs=out_specs, check_rep=False),
            donate_argnums=donate, keep_unused=True)
        self.jax = jax
        self.mesh = mesh
        self.sharding = NamedSharding(mesh, PartitionSpec("core"))
        self.in_names = in_names
        self.out_names = out_names
        self.zero_shapes = zero_shapes
        self.n_cores = n_cores
        self.dev_inputs = None

    def stage_inputs(self, in_maps):
        concat_in = [
            np.concatenate([np.asarray(in_maps[c][n])
                            for c in range(self.n_cores)], axis=0)
            for n in self.in_names
        ]
        self.dev_inputs = [self.jax.device_put(a, self.sharding)
                           for a in concat_in]
        for a in self.dev_inputs:
            a.block_until_ready()

    def exec_once(self):
        import time
        zeros = [np.zeros((self.n_cores * s[0], *s[1:]), d)
                 for s, d in self.zero_shapes]
        dz = [self.jax.device_put(z, self.sharding) for z in zeros]
        for z in dz:
            z.block_until_ready()
        t0 = time.perf_counter()
        outs = self.fn(*self.dev_inputs, *dz)
        for o in outs:
            o.block_until_ready()
        t1 = time.perf_counter()
        return outs, t1 - t0

    def results(self, outs):
        res = []
        for c in range(self.n_cores):
            m = {}
            for i, n in enumerate(self.out_names):
                full = np.asarray(outs[i])
                per = full.reshape(self.n_cores, -1, *full.shape[1:])[c]
                m[n] = per
            res.append(m)
        return res


def run_timed(cfg, inputs, iters=3):
    in_maps, meta = preprocess(cfg, **inputs)
    key = (cfg.N, cfg.F_IN, meta["TT"])
    nc = _get_compiled(cfg, key, meta["T"], meta["ops"])
    rkey = ("runner",) + key
    if rkey not in _CACHE:
        _CACHE[rkey] = _TimedRunner(nc, cfg.ncores)
    runner = _CACHE[rkey]
    runner.stage_inputs(in_maps)
    times = []
    outs = None
    for _ in range(iters):
        outs, dt = runner.exec_once()
        times.append(dt)
    results = runner.results(outs)
    out = assemble_output(cfg, meta, results)
    return out, times


def kernel(x, edge_index, W1, b1, W2, b2, Wl, bl):
    out, _ = run(FULL, dict(x=x, edge_index=edge_index, W1=W1, b1=b1,
                            W2=W2, b2=b2, Wl=Wl, bl=bl))
    return out



# revision 15
# speedup vs baseline: 13.8879x; 1.2389x over previous
"""Distributed 2-layer GCN (PyG GCNConv-style) on 8 Trainium2 NeuronCores.

Strategy (hardcoded for N=100000, E=3.2M, 512->256->128->4):
  - Nodes are degree-balanced into (ncores*W) windows of 128 nodes; window b is
    owned by core (b % ncores). A node's "global position" is its row in the
    AllGathered feature table, so gathers use plain int positions.
  - Per layer: local dense matmul (bf16 on PE, fp32 PSUM), rows pre-scaled by
    dinv, results AllGathered to a replicated bf16 feature table in DRAM.
  - Aggregation: per (window, class) block of dst-sorted edges, a dma_gather
    (custom SWDGE gather, int16 indices; the table is viewed in 4 strided
    classes of row%4 so indices fit int16) pulls source rows into SBUF; a
    one-hot S matrix built on DVE (is_equal vs iota, 4 tiles per op, bf16)
    feeds a PE matmul S^T @ msgs that segment-sums into the window's PSUM
    accumulator. Gathers stripe across 4 SWDGE queues (2.3x faster Q7
    descriptor generation). Padding slots carry dcol=128 which never
    matches iota -> contribute 0.
  - Self loops are folded into the epilogue: agg = dinv*(acc + h'own) + b
    with h' windows retained in SBUF (saves ~100K gather descriptors).
  - Epilogue: relu; layer 2 repeats; final logits + log softmax.
"""
import math
import numpy as np

import concourse.bass as bass
import concourse.mybir as mybir
import concourse.bass_utils as bass_utils
from concourse import bacc, tile
from concourse.bass_interp import get_hw_module

P = 128
F32 = mybir.dt.float32
BF16 = mybir.dt.bfloat16
I16 = mybir.dt.int16


class Cfg:
    def __init__(self, N, F_IN, H1, H2, C, ncores=8, W=None, maxt=8):
        self.N, self.F_IN, self.H1, self.H2, self.C = N, F_IN, H1, H2, C
        self.ncores = ncores
        B = ncores * P
        self.W = W if W is not None else math.ceil(N / B)
        self.NPAD = self.W * B
        assert self.NPAD >= N and self.NPAD % 4 == 0
        self.CLS = self.NPAD // 4
        assert self.CLS <= 32768
        self.KI = F_IN // P
        self.K2 = H1 // P
        self.maxt = maxt
        self.B = self.W * P  # nodes per core


FULL = Cfg(N=100000, F_IN=512, H1=256, H2=128, C=4)


# ---------------------------------------------------------------- host side
def preprocess(cfg, x, edge_index, W1, b1, W2, b2, Wl, bl):
    N, NC, W, NPAD, B = cfg.N, cfg.ncores, cfg.W, cfg.NPAD, cfg.B
    NW = NC * W

    src = np.asarray(edge_index[0], dtype=np.int64)
    dst = np.asarray(edge_index[1], dtype=np.int64)
    deg = np.bincount(dst, minlength=N).astype(np.float64) + 1.0
    dinv = (1.0 / np.sqrt(deg)).astype(np.float32)

    # node -> global position, degree-balanced across windows (snake fill)
    degall = np.zeros(NPAD, np.int64)
    degall[:N] = deg.astype(np.int64)
    order = np.argsort(-degall, kind="stable")
    i = np.arange(NPAD)
    phase = i % (2 * NW)
    binid = np.where(phase < NW, phase, 2 * NW - 1 - phase)
    by_bin = np.argsort(binid, kind="stable")
    slot = np.empty(NPAD, np.int64)
    slot[by_bin] = i % P  # within each bin, slots fill 0..127 in arrival order
    # position: core = bin % NC, window = bin // NC
    core_of_bin = binid % NC
    w_of_bin = binid // NC
    g_of_i = core_of_bin * B + w_of_bin * P + slot
    pos = np.empty(NPAD, np.int64)
    pos[order] = g_of_i

    node_at = np.empty(NPAD, np.int64)
    node_at[pos] = np.arange(NPAD)

    # edge list; self loops are folded into the epilogue on-device
    S_pos = pos[src]
    D_pos = pos[dst]
    core_e = D_pos // B
    w_e = (D_pos % B) // P
    dcol_e = (D_pos % P).astype(np.float32)
    # class tables: class = slot//32; row in class table = rank*B/4 + w*32 + slot%32
    B4 = B // 4
    s_slot = S_pos % P
    cls_e = (s_slot // 32).astype(np.int64)
    idx16_e = ((S_pos // B) * B4 + ((S_pos % B) // P) * 32
               + (s_slot % 32)).astype(np.int16)

    key = ((core_e * W + w_e) * 4 + cls_e).astype(np.int64)
    ordE = np.argsort(key, kind="stable")
    counts = np.bincount(key, minlength=NC * W * 4).reshape(NC, W, 4)
    T = np.maximum(1, np.ceil(counts / P).astype(np.int64).max(axis=0))  # [W, 4]
    TT = int(T.sum())
    SLOT = TT * P

    # ops schedule per (w, cl): list of tile counts
    ops = [[[] for _ in range(4)] for _ in range(W)]
    for w in range(W):
        for cl in range(4):
            t = int(T[w][cl])
            while t > 0:
                c = min(t, cfg.maxt)
                ops[w][cl].append(c)
                t -= c

    starts = np.zeros(NC * W * 4 + 1, np.int64)
    np.cumsum(counts.reshape(-1), out=starts[1:])
    blk_off = np.zeros((W, 4), np.int64)  # slot offset of each (w, cl) block
    acc_off = 0
    for w in range(W):
        for cl in range(4):
            blk_off[w, cl] = acc_off
            acc_off += int(T[w][cl]) * P

    idx16 = np.zeros((NC, SLOT), np.int16)
    dcol = np.full((NC, SLOT), float(P), np.float32)  # cast bf16 at ship time
    for c in range(NC):
        for w in range(W):
            for cl in range(4):
                k = (c * W + w) * 4 + cl
                s0, s1 = starts[k], starts[k + 1]
                n = s1 - s0
                off = blk_off[w, cl]
                seg = ordE[s0:s1]
                idx16[c, off:off + n] = idx16_e[seg]
                dcol[c, off:off + n] = dcol_e[seg]

    # wrap idx16 per-op: element i of an op at [i%16, i//16], replicated x8
    idx_w = np.zeros((NC, 16, SLOT // 16), np.int16)
    for w in range(W):
        for cl in range(4):
            off = int(blk_off[w, cl])
            for t_op in ops[w][cl]:
                n = t_op * P
                blk = idx16[:, off:off + n].reshape(NC, n // 16, 16)
                idx_w[:, :, off // 16:(off + n) // 16] = blk.transpose(0, 2, 1)
                off += n
    idx_rep = np.tile(idx_w, (1, 8, 1))  # [NC, 128, SLOT//16]

    dcol_t = dcol.reshape(NC, TT, P).transpose(0, 2, 1).copy()  # [NC, 128, TT]

    # x shard, transposed chunk layout: xt4[p, w, c2, m] = x[node(w*128+m), c2*128+p]
    xpad = np.zeros((NPAD, cfg.F_IN), np.float32)
    xpad[:N] = np.asarray(x, np.float32)
    dinvpad = np.ones(NPAD, np.float32)
    dinvpad[:N] = dinv

    xt4 = np.empty((NC, P, W, cfg.KI, P), np.float32)
    dinvl = np.empty((NC, P, W), np.float32)
    for c in range(NC):
        ids = node_at[c * B:(c + 1) * B]
        xl = xpad[ids]  # [B, F_IN]
        xt4[c] = xl.reshape(W, P, cfg.KI, P).transpose(3, 0, 2, 1)
        dinvl[c] = dinvpad[ids].reshape(W, P).T

    w1d = np.asarray(W1, np.float32).reshape(cfg.KI, P, cfg.H1).transpose(1, 0, 2)
    w2d = np.asarray(W2, np.float32).reshape(cfg.K2, P, cfg.H2).transpose(1, 0, 2)
    wld = np.asarray(Wl, np.float32)  # [H2=128, C]
    b1b = np.broadcast_to(np.asarray(b1, np.float32), (P, cfg.H1)).copy()
    b2b = np.broadcast_to(np.asarray(b2, np.float32), (P, cfg.H2)).copy()
    blb = np.broadcast_to(np.asarray(bl, np.float32), (P, cfg.C)).copy()
    iota4 = np.broadcast_to(np.arange(P, dtype=np.float32),
                            (P, 4, P)).reshape(P, 4 * P).copy()
    ident = np.eye(P, dtype=np.float32)

    import ml_dtypes
    bf = lambda a: a.astype(ml_dtypes.bfloat16)

    in_maps = []
    for c in range(NC):
        in_maps.append({
            "xt4": bf(xt4[c]),
            "w1d": bf(w1d), "w2d": bf(w2d), "wld": bf(wld),
            "b1b": b1b, "b2b": b2b, "blb": blb,
            "dinvl": dinvl[c],
            "idx16": idx_rep[c],
            "dcol": dcol_t[c],
            "iota": bf(iota4),
            "ident": bf(ident),
        })

    meta = dict(T=T, ops=ops, pos=pos, node_at=node_at, SLOT=SLOT, TT=TT)
    return in_maps, meta


def assemble_output(cfg, meta, results):
    N, NC, W, C, B = cfg.N, cfg.ncores, cfg.W, cfg.C, cfg.B
    rows = []
    for c in range(NC):
        r = results[c]["outst"].reshape(P, W, C).transpose(1, 0, 2).reshape(B, C)
        rows.append(r)
    allrows = np.concatenate(rows, axis=0)  # [NPAD, C] in position order
    return allrows[meta["pos"][:N]].astype(np.float32)


# ---------------------------------------------------------------- device side
def build_kernel(cfg, T, ops, upto="full"):
    NC, W, NPAD, B = cfg.ncores, cfg.W, cfg.NPAD, cfg.B
    H1, H2, C, KI, K2 = cfg.H1, cfg.H2, cfg.C, cfg.KI, cfg.K2
    TT = int(np.asarray(T).sum())
    SLOT = TT * P

    nc = bacc.Bacc("TRN2", target_bir_lowering=False, debug=False,
                   num_devices=NC, num_swdge_queues=4)

    xt4 = nc.dram_tensor("xt4", [P, W, KI, P], BF16, kind="ExternalInput")
    w1d = nc.dram_tensor("w1d", [P, KI, H1], BF16, kind="ExternalInput")
    w2d = nc.dram_tensor("w2d", [P, K2, H2], BF16, kind="ExternalInput")
    wld = nc.dram_tensor("wld", [P, C], BF16, kind="ExternalInput")
    b1b = nc.dram_tensor("b1b", [P, H1], F32, kind="ExternalInput")
    b2b = nc.dram_tensor("b2b", [P, H2], F32, kind="ExternalInput")
    blb = nc.dram_tensor("blb", [P, C], F32, kind="ExternalInput")
    dinvl = nc.dram_tensor("dinvl", [P, W], F32, kind="ExternalInput")
    idx16 = nc.dram_tensor("idx16", [P, SLOT // 16], I16, kind="ExternalInput")
    dcol = nc.dram_tensor("dcol", [P, TT], F32, kind="ExternalInput")
    iota = nc.dram_tensor("iota", [P, 4 * P], BF16, kind="ExternalInput")
    ident = nc.dram_tensor("ident", [P, P], BF16, kind="ExternalInput")
    outst = nc.dram_tensor("outst", [P, W * C], F32, kind="ExternalOutput")

    # per-window column ranges in idx16 / dcol
    blk_tiles = np.asarray(T)  # [W, 4]
    w_tile_off = np.zeros(W + 1, np.int64)
    np.cumsum(blk_tiles.sum(axis=1), out=w_tile_off[1:])

    rg = [list(range(NC))]

    B4 = B // 4
    with tile.TileContext(nc) as tc:
        with tc.tile_pool(name="const", bufs=1) as cpool, \
             tc.tile_pool(name="dram", bufs=1, space="DRAM") as dram:
            ag1_in = [dram.tile([B4, H1], BF16, name=f"ag1i{i}") for i in range(4)]
            ag1_out = [dram.tile([NPAD // 4, H1], BF16, addr_space="Shared",
                                 name=f"ag1o{i}") for i in range(4)]
            ag2_in = [dram.tile([B4, H2], BF16, name=f"ag2i{i}") for i in range(4)]
            ag2_out = [dram.tile([NPAD // 4, H2], BF16, addr_space="Shared",
                                 name=f"ag2o{i}") for i in range(4)]

            w1sb = cpool.tile([P, KI, H1], BF16)
            nc.sync.dma_start(w1sb[:], w1d[:])
            w2sb = cpool.tile([P, K2, H2], BF16)
            nc.sync.dma_start(w2sb[:], w2d[:])
            wlsb = cpool.tile([P, C], BF16)
            nc.sync.dma_start(wlsb[:], wld[:])
            b1sb = cpool.tile([P, H1], F32)
            nc.sync.dma_start(b1sb[:], b1b[:])
            b2sb = cpool.tile([P, H2], F32)
            nc.sync.dma_start(b2sb[:], b2b[:])
            blsb = cpool.tile([P, C], F32)
            nc.sync.dma_start(blsb[:], blb[:])
            dinvsb = cpool.tile([P, W], F32)
            nc.sync.dma_start(dinvsb[:], dinvl[:])
            iotasb = cpool.tile([P, 4, P], BF16)
            nc.sync.dma_start(iotasb[:], iota[:].rearrange("p (a b) -> p a b", a=4))
            idsb = cpool.tile([P, P], BF16)
            nc.sync.dma_start(idsb[:], ident[:])
            dcolsb = cpool.tile([P, TT], F32)
            nc.sync.dma_start(dcolsb[:], dcol[:])

            lgst = cpool.tile([P, W * C], F32)
            sst = cpool.tile([P, W], F32)
            outsb = cpool.tile([P, W * C], F32)
            # retained h' windows for the self-loop epilogue term
            h1buf = cpool.tile([P, W, H1], BF16)
            h2buf = cpool.tile([P, W, H2], BF16)

            # ---------------- phase A: h1' = dinv * (x @ W1), allgather
            with tc.tile_pool(name="phA", bufs=3) as sbA, \
                 tc.tile_pool(name="phA_ps", bufs=2, space="PSUM") as psA:
                for w in (range(W) if upto != "noop" else []):
                    xt = sbA.tile([P, KI, P], BF16, tag="xt")
                    nc.sync.dma_start(xt[:], xt4[:, w])
                    hp = psA.tile([P, H1], F32, tag="hp")
                    for c2 in range(KI):
                        nc.tensor.matmul(hp[:], xt[:, c2], w1sb[:, c2],
                                         start=(c2 == 0), stop=(c2 == KI - 1))
                    nc.scalar.activation(h1buf[:, w], hp[:],
                                         mybir.ActivationFunctionType.Copy,
                                         scale=dinvsb[:, w:w + 1])
                    for c4 in range(4):
                        nc.sync.dma_start(ag1_in[c4][w * 32:(w + 1) * 32, :],
                                          h1buf[c4 * 32:(c4 + 1) * 32, w])

            for c4 in (range(4) if upto != "noop" else []):
                if NC == 1:
                    nc.sync.dma_start(ag1_out[c4][:], ag1_in[c4][:])
                else:
                    nc.gpsimd.collective_compute(
                        "AllGather", mybir.AluOpType.bypass,
                        ins=[ag1_in[c4][:]], outs=[ag1_out[c4][:]],
                        replica_groups=rg)

            qctr = [0]

            def aggregate(w, sb, sbS, ps, cls_tabs, F, tag):
                """Gather + S-matmul segment-sum for window w at width F.
                Returns the PSUM accumulator tile."""
                t0 = int(w_tile_off[w])
                ntile_w = int(w_tile_off[w + 1] - w_tile_off[w])
                idxw = sb.tile([P, ntile_w * 8], I16, tag=f"idxw{tag}", bufs=6)
                nc.sync.dma_start(idxw[:], idx16[:, t0 * 8:(t0 + ntile_w) * 8])
                acc = ps.tile([P, F], F32, tag=f"acc{tag}")
                ti = 0
                for cl in range(4):
                    for t_op in ops[w][cl]:
                        g = sb.tile([P, cfg.maxt, F], BF16, tag=f"g{tag}",
                                    bufs=10)
                        nc.gpsimd.dma_gather(
                            g[:, :t_op], cls_tabs[cl][:],
                            idxw[:, ti * 8:(ti + t_op) * 8],
                            t_op * P, t_op * P, F,
                            queue_num=qctr[0] % 4)
                        qctr[0] += 1
                        for t in range(t_op):
                            S = sbS.tile([P, P], BF16, tag=f"S{tag}", bufs=8)
                            nc.vector.tensor_scalar(
                                S[:], iotasb[:, 0],
                                dcolsb[:, t0 + ti + t:t0 + ti + t + 1], None,
                                op0=mybir.AluOpType.is_equal)
                            nc.tensor.matmul(acc[:], S[:], g[:, t],
                                             start=(ti + t == 0),
                                             stop=(ti + t == ntile_w - 1))
                        ti += t_op
                return acc

            # ---------------- phase C/D: aggregate layer1, h2' = dinv*(a1@W2)
            if upto.startswith("CD") or upto == "full":
                with tc.tile_pool(name="phC", bufs=4) as sbC, \
                     tc.tile_pool(name="phC_s", bufs=6) as sbS, \
                     tc.tile_pool(name="phC_ps", bufs=2, space="PSUM") as psC, \
                     tc.tile_pool(name="phD_ps", bufs=2, space="PSUM") as psD:
                    for w in range(W):
                        acc = aggregate(w, sbC, sbS, psC, ag1_out, H1, "1")
                        # self loop: agg = dinv*(acc + h1') ; then + b, relu
                        zs = sbC.tile([P, H1], F32, tag="zs")
                        nc.vector.tensor_tensor(zs[:], acc[:], h1buf[:, w],
                                                op=mybir.AluOpType.add)
                        z = sbC.tile([P, H1], F32, tag="z")
                        nc.scalar.activation(z[:], zs[:],
                                             mybir.ActivationFunctionType.Copy,
                                             scale=dinvsb[:, w:w + 1])
                        z2 = sbC.tile([P, H1], F32, tag="z2")
                        nc.vector.tensor_tensor(z2[:], z[:], b1sb[:],
                                                op=mybir.AluOpType.add)
                        a1 = sbC.tile([P, H1], BF16, tag="a1")
                        nc.scalar.activation(a1[:], z2[:],
                                             mybir.ActivationFunctionType.Relu)
                        h2p = psD.tile([P, H2], F32, tag="h2p")
                        for c2 in range(K2):
                            a1tp = psD.tile([P, P], BF16, tag="a1tp")
                            nc.tensor.transpose(a1tp[:], a1[:, c2 * P:(c2 + 1) * P],
                                                idsb[:])
                            a1t = sbC.tile([P, P], BF16, tag="a1t")
                            nc.vector.tensor_copy(a1t[:], a1tp[:])
                            nc.tensor.matmul(h2p[:], a1t[:], w2sb[:, c2],
                                             start=(c2 == 0), stop=(c2 == K2 - 1))
                        nc.scalar.activation(h2buf[:, w], h2p[:],
                                             mybir.ActivationFunctionType.Copy,
                                             scale=dinvsb[:, w:w + 1])
                        for c4 in range(4):
                            nc.sync.dma_start(ag2_in[c4][w * 32:(w + 1) * 32, :],
                                              h2buf[c4 * 32:(c4 + 1) * 32, w])

                for c4 in range(4):
                    if NC == 1:
                        nc.sync.dma_start(ag2_out[c4][:], ag2_in[c4][:])
                    else:
                        nc.gpsimd.collective_compute(
                            "AllGather", mybir.AluOpType.bypass,
                            ins=[ag2_in[c4][:]], outs=[ag2_out[c4][:]],
                            replica_groups=rg)

            # ---------------- phase E/F: aggregate layer2, logits, log_softmax
            if upto == "full":
                with tc.tile_pool(name="phE", bufs=4) as sbE, \
                     tc.tile_pool(name="phE_s", bufs=6) as sbS2, \
                     tc.tile_pool(name="phE_ps", bufs=2, space="PSUM") as psE, \
                     tc.tile_pool(name="phL_ps", bufs=2, space="PSUM") as psL:
                    for w in range(W):
                        acc = aggregate(w, sbE, sbS2, psE, ag2_out, H2, "2")
                        zs = sbE.tile([P, H2], F32, tag="zse")
                        nc.vector.tensor_tensor(zs[:], acc[:], h2buf[:, w],
                                                op=mybir.AluOpType.add)
                        z = sbE.tile([P, H2], F32, tag="ze")
                        nc.scalar.activation(z[:], zs[:],
                                             mybir.ActivationFunctionType.Copy,
                                             scale=dinvsb[:, w:w + 1])
                        z2 = sbE.tile([P, H2], F32, tag="z2e")
                        nc.vector.tensor_tensor(z2[:], z[:], b2sb[:],
                                                op=mybir.AluOpType.add)
                        a2 = sbE.tile([P, H2], BF16, tag="a2")
                        nc.scalar.activation(a2[:], z2[:],
                                             mybir.ActivationFunctionType.Relu)
                        a2tp = psL.tile([P, P], BF16, tag="a2tp")
                        nc.tensor.transpose(a2tp[:], a2[:], idsb[:])
                        a2t = sbE.tile([P, P], BF16, tag="a2t")
                        nc.vector.tensor_copy(a2t[:], a2tp[:])
                        lg = psL.tile([P, C], F32, tag="lg")
                        nc.tensor.matmul(lg[:], a2t[:], wlsb[:], start=True, stop=True)
                        nc.vector.tensor_tensor(lgst[:, w * C:(w + 1) * C], lg[:],
                                                blsb[:], op=mybir.AluOpType.add)
                        e = sbE.tile([P, C], F32, tag="e")
                        nc.scalar.activation(e[:], lgst[:, w * C:(w + 1) * C],
                                             mybir.ActivationFunctionType.Exp,
                                             accum_out=sst[:, w:w + 1])
                    lns = cpool.tile([P, W], F32)
                    nc.scalar.activation(lns[:], sst[:],
                                         mybir.ActivationFunctionType.Ln)
                    for w in range(W):
                        nc.vector.tensor_scalar(
                            outsb[:, w * C:(w + 1) * C], lgst[:, w * C:(w + 1) * C],
                            lns[:, w:w + 1], None, op0=mybir.AluOpType.subtract)
                    nc.sync.dma_start(outst[:], outsb[:])
            else:
                # debug variants: dummy output proving the kept phases ran
                nc.vector.memset(outsb[:], 0.0)
                if upto != "noop":
                    probe_src = ag1_out[0] if upto == "A" else ag2_out[0]
                    probe = cpool.tile([P, C], BF16)
                    nc.sync.dma_start(probe[:], probe_src[:P, :C])
                    nc.vector.tensor_copy(outsb[:, :C], probe[:])
                nc.sync.dma_start(outst[:], outsb[:])

    nc.compile()
    return nc

# ---------------------------------------------------------------- entry point
_CACHE = {}


def _get_compiled(cfg, key, T, ops):
    if key not in _CACHE:
        nc = build_kernel(cfg, T, ops)
        nc.m = get_hw_module(nc.m)
        _CACHE[key] = nc
    return _CACHE[key]


def run(cfg, inputs):
    in_maps, meta = preprocess(cfg, **inputs)
    key = (cfg.N, cfg.F_IN, meta["TT"])
    nc = _get_compiled(cfg, key, meta["T"], meta["ops"])
    res = bass_utils.run_bass_kernel_spmd(
        nc, in_maps, core_ids=list(range(cfg.ncores)))
    out = assemble_output(cfg, meta, res.results)
    return out, res


class _TimedRunner:
    """PJRT runner mirroring bass2jax.run_bass_via_pjrt's multi-core branch,
    but with a cached jit and device-resident inputs for repeatable timing."""

    def __init__(self, nc, n_cores):
        import jax
        import concourse.mybir as mb
        from concourse import bass2jax
        from jax.sharding import Mesh, PartitionSpec, NamedSharding
        from jax.experimental.shard_map import shard_map

        bass2jax.install_neuronx_cc_hook()
        partition_name = (nc.partition_id_tensor.name
                          if nc.partition_id_tensor else None)
        in_names, out_names, out_avals, zero_shapes = [], [], [], []
        for alloc in nc.m.functions[0].allocations:
            if not isinstance(alloc, mb.MemoryLocationSet):
                continue
            name = alloc.memorylocations[0].name
            if alloc.kind == "ExternalInput":
                if name != partition_name:
                    in_names.append(name)
            elif alloc.kind == "ExternalOutput":
                out_names.append(name)
                shape = tuple(alloc.tensor_shape)
                dtype = mb.dt.np(alloc.dtype)
                out_avals.append(jax.core.ShapedArray(shape, dtype))
                zero_shapes.append((shape, dtype))
        n_params = len(in_names)
        all_in_names = list(in_names) + list(out_names)
        if partition_name is not None:
            all_in_names.append(partition_name)
        donate = tuple(range(n_params, n_params + len(out_names)))

        def _body(*args):
            operands = list(args)
            if partition_name is not None:
                operands.append(bass2jax.partition_id_tensor())
            outs = bass2jax._bass_exec_p.bind(
                *operands,
                out_avals=tuple(out_avals),
                in_names=tuple(all_in_names),
                out_names=tuple(out_names),
                lowering_input_output_aliases=(),
                sim_require_finite=True,
                sim_require_nnan=True,
                nc=nc,
            )
            return tuple(outs)

        devices = jax.devices()[:n_cores]
        mesh = Mesh(np.asarray(devices), ("core",))
        in_specs = (PartitionSpec("core"),) * (n_params + len(out_names))
        out_specs = (PartitionSpec("core"),) * len(out_names)
        self.fn = jax.jit(
            shard_map(_body, mesh=mesh, in_specs=in_specs,
                      out_specs=out_specs, check_rep=False),
            donate_argnums=donate, keep_unused=True)
        self.jax = jax
        self.mesh = mesh
        self.sharding = NamedSharding(mesh, PartitionSpec("core"))
        self.in_names = in_names
        self.out_names = out_names
        self.zero_shapes = zero_shapes
        self.n_cores = n_cores
        self.dev_inputs = None

    def stage_inputs(self, in_maps):
        concat_in = [
            np.concatenate([np.asarray(in_maps[c][n])
                            for c in range(self.n_cores)], axis=0)
            for n in self.in_names
        ]
        self.dev_inputs = [self.jax.device_put(a, self.sharding)
                           for a in concat_in]
        for a in self.dev_inputs:
            a.block_until_ready()

    def exec_once(self):
        import time
        zeros = [np.zeros((self.n_cores * s[0], *s[1:]), d)
                 for s, d in self.zero_shapes]
        dz = [self.jax.device_put(z, self.sharding) for z in zeros]
        for z in dz:
            z.block_until_ready()
        t0 = time.perf_counter()
        outs = self.fn(*self.dev_inputs, *dz)
        for o in outs:
            o.block_until_ready()
        t1 = time.perf_counter()
        return outs, t1 - t0

    def results(self, outs):
        res = []
        for c in range(self.n_cores):
            m = {}
            for i, n in enumerate(self.out_names):
                full = np.asarray(outs[i])
                per = full.reshape(self.n_cores, -1, *full.shape[1:])[c]
                m[n] = per
            res.append(m)
        return res


def run_timed(cfg, inputs, iters=3):
    in_maps, meta = preprocess(cfg, **inputs)
    key = (cfg.N, cfg.F_IN, meta["TT"])
    nc = _get_compiled(cfg, key, meta["T"], meta["ops"])
    rkey = ("runner",) + key
    if rkey not in _CACHE:
        _CACHE[rkey] = _TimedRunner(nc, cfg.ncores)
    runner = _CACHE[rkey]
    runner.stage_inputs(in_maps)
    times = []
    outs = None
    for _ in range(iters):
        outs, dt = runner.exec_once()
        times.append(dt)
    results = runner.results(outs)
    out = assemble_output(cfg, meta, results)
    return out, times


def kernel(x, edge_index, W1, b1, W2, b2, Wl, bl):
    out, _ = run(FULL, dict(x=x, edge_index=edge_index, W1=W1, b1=b1,
                            W2=W2, b2=b2, Wl=Wl, bl=bl))
    return out



# revision 18
# speedup vs baseline: 24.5116x; 1.7650x over previous
"""Distributed 2-layer GCN (PyG GCNConv-style) on 8 Trainium2 NeuronCores.

Strategy (hardcoded for N=100000, E=3.2M, 512->256->128->4):
  - Nodes are degree-balanced into (ncores*W) windows of 128 nodes; window b is
    owned by core (b % ncores). A node's "global position" is its row in the
    AllGathered feature table, so gathers use plain int positions.
  - Per layer: local dense matmul (bf16 on PE, fp32 PSUM), rows pre-scaled by
    dinv, results AllGathered to a replicated bf16 feature table in DRAM.
  - Aggregation: per (window, class) block of dst-sorted edges, a dma_gather
    (custom SWDGE gather, int16 indices; the table is viewed in 4 strided
    classes of row%4 so indices fit int16) pulls source rows into SBUF; a
    one-hot S matrix built on DVE (is_equal vs iota, 4 tiles per op, bf16)
    feeds a PE matmul S^T @ msgs that segment-sums into the window's PSUM
    accumulator. Gathers stripe across 4 SWDGE queues (2.3x faster Q7
    descriptor generation). Padding slots carry dcol=128 which never
    matches iota -> contribute 0.
  - Self loops are folded into the epilogue: agg = dinv*(acc + h'own) + b
    with h' windows retained in SBUF (saves ~100K gather descriptors).
  - Epilogue: relu; layer 2 repeats; final logits + log softmax.
"""
import math
import numpy as np

import concourse.bass as bass
import concourse.mybir as mybir
import concourse.bass_utils as bass_utils
from concourse import bacc, tile
from concourse.bass_interp import get_hw_module

P = 128
F32 = mybir.dt.float32
BF16 = mybir.dt.bfloat16
I16 = mybir.dt.int16


class Cfg:
    def __init__(self, N, F_IN, H1, H2, C, ncores=8, W=None, maxt=8):
        self.N, self.F_IN, self.H1, self.H2, self.C = N, F_IN, H1, H2, C
        self.ncores = ncores
        B = ncores * P
        self.W = W if W is not None else math.ceil(N / B)
        self.NPAD = self.W * B
        assert self.NPAD >= N and self.NPAD % 4 == 0
        self.CLS = self.NPAD // 4
        assert self.CLS <= 32768
        self.KI = F_IN // P
        self.K2 = H1 // P
        self.maxt = maxt
        self.B = self.W * P  # nodes per core


FULL = Cfg(N=100000, F_IN=512, H1=256, H2=128, C=4)


# ---------------------------------------------------------------- host side
def preprocess(cfg, x, edge_index, W1, b1, W2, b2, Wl, bl):
    N, NC, W, NPAD, B = cfg.N, cfg.ncores, cfg.W, cfg.NPAD, cfg.B
    NW = NC * W

    src = np.asarray(edge_index[0], dtype=np.int64)
    dst = np.asarray(edge_index[1], dtype=np.int64)
    deg = np.bincount(dst, minlength=N).astype(np.float64) + 1.0
    dinv = (1.0 / np.sqrt(deg)).astype(np.float32)

    # node -> global position, degree-balanced across windows (snake fill)
    degall = np.zeros(NPAD, np.int64)
    degall[:N] = deg.astype(np.int64)
    order = np.argsort(-degall, kind="stable")
    i = np.arange(NPAD)
    phase = i % (2 * NW)
    binid = np.where(phase < NW, phase, 2 * NW - 1 - phase)
    by_bin = np.argsort(binid, kind="stable")
    slot = np.empty(NPAD, np.int64)
    slot[by_bin] = i % P  # within each bin, slots fill 0..127 in arrival order
    # position: core = bin % NC, window = bin // NC
    core_of_bin = binid % NC
    w_of_bin = binid // NC
    g_of_i = core_of_bin * B + w_of_bin * P + slot
    pos = np.empty(NPAD, np.int64)
    pos[order] = g_of_i

    node_at = np.empty(NPAD, np.int64)
    node_at[pos] = np.arange(NPAD)

    # edge list; self loops are folded into the epilogue on-device
    S_pos = pos[src]
    D_pos = pos[dst]
    core_e = D_pos // B
    w_e = (D_pos % B) // P
    dcol_e = (D_pos % P).astype(np.float32)
    # class tables: class = slot//32; row in class table = rank*B/4 + w*32 + slot%32
    B4 = B // 4
    s_slot = S_pos % P
    cls_e = (s_slot // 32).astype(np.int64)
    idx16_e = ((S_pos // B) * B4 + ((S_pos % B) // P) * 32
               + (s_slot % 32)).astype(np.int16)

    key = ((core_e * W + w_e) * 4 + cls_e).astype(np.int64)
    ordE = np.argsort(key, kind="stable")
    counts = np.bincount(key, minlength=NC * W * 4).reshape(NC, W, 4)
    T = np.maximum(1, np.ceil(counts / P).astype(np.int64).max(axis=0))  # [W, 4]
    TT = int(T.sum())
    SLOT = TT * P

    # ops schedule per (w, cl): list of tile counts. Ops of >=4 tiles
    # (>=512 idx) pay no SWDGE fixed overhead; split e.g. 9 -> [5, 4].
    ops = [[[] for _ in range(4)] for _ in range(W)]
    for w in range(W):
        for cl in range(4):
            t = int(T[w][cl])
            while t > 0:
                c = min(t, cfg.maxt)
                if t > cfg.maxt and t - cfg.maxt < 4:
                    c = t - 4
                ops[w][cl].append(c)
                t -= c

    starts = np.zeros(NC * W * 4 + 1, np.int64)
    np.cumsum(counts.reshape(-1), out=starts[1:])
    blk_off = np.zeros((W, 4), np.int64)  # slot offset of each (w, cl) block
    acc_off = 0
    for w in range(W):
        for cl in range(4):
            blk_off[w, cl] = acc_off
            acc_off += int(T[w][cl]) * P

    idx16 = np.zeros((NC, SLOT), np.int16)
    dcol = np.full((NC, SLOT), float(P), np.float32)  # cast bf16 at ship time
    for c in range(NC):
        for w in range(W):
            for cl in range(4):
                k = (c * W + w) * 4 + cl
                s0, s1 = starts[k], starts[k + 1]
                n = s1 - s0
                off = blk_off[w, cl]
                seg = ordE[s0:s1]
                idx16[c, off:off + n] = idx16_e[seg]
                dcol[c, off:off + n] = dcol_e[seg]

    # wrap idx16 per-op: element i of an op at [i%16, i//16], replicated x8
    idx_w = np.zeros((NC, 16, SLOT // 16), np.int16)
    for w in range(W):
        for cl in range(4):
            off = int(blk_off[w, cl])
            for t_op in ops[w][cl]:
                n = t_op * P
                blk = idx16[:, off:off + n].reshape(NC, n // 16, 16)
                idx_w[:, :, off // 16:(off + n) // 16] = blk.transpose(0, 2, 1)
                off += n
    idx_rep = np.tile(idx_w, (1, 8, 1))  # [NC, 128, SLOT//16]

    dcol_t = dcol.reshape(NC, TT, P).transpose(0, 2, 1).copy()  # [NC, 128, TT]

    # x shard, transposed chunk layout: xt4[p, w, c2, m] = x[node(w*128+m), c2*128+p]
    xpad = np.zeros((NPAD, cfg.F_IN), np.float32)
    xpad[:N] = np.asarray(x, np.float32)
    dinvpad = np.ones(NPAD, np.float32)
    dinvpad[:N] = dinv

    xt4 = np.empty((NC, P, W, cfg.KI, P), np.float32)
    dinvl = np.empty((NC, P, W), np.float32)
    for c in range(NC):
        ids = node_at[c * B:(c + 1) * B]
        xl = xpad[ids]  # [B, F_IN]
        xt4[c] = xl.reshape(W, P, cfg.KI, P).transpose(3, 0, 2, 1)
        dinvl[c] = dinvpad[ids].reshape(W, P).T

    w1d = np.asarray(W1, np.float32).reshape(cfg.KI, P, cfg.H1).transpose(1, 0, 2)
    w2d = np.asarray(W2, np.float32).reshape(cfg.K2, P, cfg.H2).transpose(1, 0, 2)
    wld = np.asarray(Wl, np.float32)  # [H2=128, C]
    b1b = np.broadcast_to(np.asarray(b1, np.float32), (P, cfg.H1)).copy()
    b2b = np.broadcast_to(np.asarray(b2, np.float32), (P, cfg.H2)).copy()
    blb = np.broadcast_to(np.asarray(bl, np.float32), (P, cfg.C)).copy()
    iota4 = np.broadcast_to(np.arange(P, dtype=np.float32),
                            (P, 4, P)).reshape(P, 4 * P).copy()
    ident = np.eye(P, dtype=np.float32)

    import ml_dtypes
    bf = lambda a: a.astype(ml_dtypes.bfloat16)

    in_maps = []
    for c in range(NC):
        in_maps.append({
            "xt4": bf(xt4[c]),
            "w1d": bf(w1d), "w2d": bf(w2d), "wld": bf(wld),
            "b1b": b1b, "b2b": b2b, "blb": blb,
            "dinvl": dinvl[c],
            "idx16": idx_rep[c],
            "dcol": bf(dcol_t[c]),
            "iota": bf(iota4),
            "ident": bf(ident),
        })

    meta = dict(T=T, ops=ops, pos=pos, node_at=node_at, SLOT=SLOT, TT=TT)
    return in_maps, meta


def assemble_output(cfg, meta, results):
    N, NC, W, C, B = cfg.N, cfg.ncores, cfg.W, cfg.C, cfg.B
    rows = []
    for c in range(NC):
        r = results[c]["outst"].reshape(P, W, C).transpose(1, 0, 2).reshape(B, C)
        rows.append(r)
    allrows = np.concatenate(rows, axis=0)  # [NPAD, C] in position order
    return allrows[meta["pos"][:N]].astype(np.float32)


# ---------------------------------------------------------------- device side
def build_kernel(cfg, T, ops, upto="full"):
    NC, W, NPAD, B = cfg.ncores, cfg.W, cfg.NPAD, cfg.B
    H1, H2, C, KI, K2 = cfg.H1, cfg.H2, cfg.C, cfg.KI, cfg.K2
    TT = int(np.asarray(T).sum())
    SLOT = TT * P

    nc = bacc.Bacc("TRN2", target_bir_lowering=False, debug=False,
                   num_devices=NC, num_swdge_queues=4)

    xt4 = nc.dram_tensor("xt4", [P, W, KI, P], BF16, kind="ExternalInput")
    w1d = nc.dram_tensor("w1d", [P, KI, H1], BF16, kind="ExternalInput")
    w2d = nc.dram_tensor("w2d", [P, K2, H2], BF16, kind="ExternalInput")
    wld = nc.dram_tensor("wld", [P, C], BF16, kind="ExternalInput")
    b1b = nc.dram_tensor("b1b", [P, H1], F32, kind="ExternalInput")
    b2b = nc.dram_tensor("b2b", [P, H2], F32, kind="ExternalInput")
    blb = nc.dram_tensor("blb", [P, C], F32, kind="ExternalInput")
    dinvl = nc.dram_tensor("dinvl", [P, W], F32, kind="ExternalInput")
    idx16 = nc.dram_tensor("idx16", [P, SLOT // 16], I16, kind="ExternalInput")
    dcol = nc.dram_tensor("dcol", [P, TT], BF16, kind="ExternalInput")
    iota = nc.dram_tensor("iota", [P, 4 * P], BF16, kind="ExternalInput")
    ident = nc.dram_tensor("ident", [P, P], BF16, kind="ExternalInput")
    outst = nc.dram_tensor("outst", [P, W * C], F32, kind="ExternalOutput")

    # per-window column ranges in idx16 / dcol
    blk_tiles = np.asarray(T)  # [W, 4]
    w_tile_off = np.zeros(W + 1, np.int64)
    np.cumsum(blk_tiles.sum(axis=1), out=w_tile_off[1:])

    rg = [list(range(NC))]

    B4 = B // 4
    with tile.TileContext(nc) as tc:
        with tc.tile_pool(name="const", bufs=1) as cpool, \
             tc.tile_pool(name="dram", bufs=1, space="DRAM") as dram:
            ag1_in = [dram.tile([B4, H1], BF16, name=f"ag1i{i}") for i in range(4)]
            ag1_out = [dram.tile([NPAD // 4, H1], BF16, addr_space="Shared",
                                 name=f"ag1o{i}") for i in range(4)]
            ag2_in = [dram.tile([B4, H2], BF16, name=f"ag2i{i}") for i in range(4)]
            ag2_out = [dram.tile([NPAD // 4, H2], BF16, addr_space="Shared",
                                 name=f"ag2o{i}") for i in range(4)]

            w1sb = cpool.tile([P, KI, H1], BF16)
            nc.sync.dma_start(w1sb[:], w1d[:])
            w2sb = cpool.tile([P, K2, H2], BF16)
            nc.sync.dma_start(w2sb[:], w2d[:])
            wlsb = cpool.tile([P, C], BF16)
            nc.sync.dma_start(wlsb[:], wld[:])
            b1sb = cpool.tile([P, H1], F32)
            nc.sync.dma_start(b1sb[:], b1b[:])
            b2sb = cpool.tile([P, H2], F32)
            nc.sync.dma_start(b2sb[:], b2b[:])
            blsb = cpool.tile([P, C], F32)
            nc.sync.dma_start(blsb[:], blb[:])
            dinvsb = cpool.tile([P, W], F32)
            nc.sync.dma_start(dinvsb[:], dinvl[:])
            iotasb = cpool.tile([P, 4, P], BF16)
            nc.sync.dma_start(iotasb[:], iota[:].rearrange("p (a b) -> p a b", a=4))
            idsb = cpool.tile([P, P], BF16)
            nc.sync.dma_start(idsb[:], ident[:])
            dcolsb = cpool.tile([P, TT], BF16)
            nc.sync.dma_start(dcolsb[:], dcol[:])

            lgst = cpool.tile([P, W * C], F32)
            sst = cpool.tile([P, W], F32)
            outsb = cpool.tile([P, W * C], F32)
            # retained h' windows for the self-loop epilogue term
            h1buf = cpool.tile([P, W, H1], BF16)
            h2buf = cpool.tile([P, W, H2], BF16)

            # ---------------- phase A: h1' = dinv * (x @ W1), allgather
            with tc.tile_pool(name="phA", bufs=3) as sbA, \
                 tc.tile_pool(name="phA_ps", bufs=2, space="PSUM") as psA:
                for w in (range(W) if upto != "noop" else []):
                    xt = sbA.tile([P, KI, P], BF16, tag="xt")
                    nc.sync.dma_start(xt[:], xt4[:, w])
                    hp = psA.tile([P, H1], F32, tag="hp")
                    for c2 in range(KI):
                        nc.tensor.matmul(hp[:], xt[:, c2], w1sb[:, c2],
                                         start=(c2 == 0), stop=(c2 == KI - 1))
                    nc.scalar.activation(h1buf[:, w], hp[:],
                                         mybir.ActivationFunctionType.Copy,
                                         scale=dinvsb[:, w:w + 1])
                    for c4 in range(4):
                        nc.sync.dma_start(ag1_in[c4][w * 32:(w + 1) * 32, :],
                                          h1buf[c4 * 32:(c4 + 1) * 32, w])

            for c4 in (range(4) if upto != "noop" else []):
                if NC == 1:
                    nc.sync.dma_start(ag1_out[c4][:], ag1_in[c4][:])
                else:
                    nc.gpsimd.collective_compute(
                        "AllGather", mybir.AluOpType.bypass,
                        ins=[ag1_in[c4][:]], outs=[ag1_out[c4][:]],
                        replica_groups=rg)

            qctr = [0]

            def aggregate(w, sb, sbS, ps, cls_tabs, F, tag):
                """Gather + S-matmul segment-sum for window w at width F.
                Returns the PSUM accumulator tile."""
                t0 = int(w_tile_off[w])
                ntile_w = int(w_tile_off[w + 1] - w_tile_off[w])
                idxw = sb.tile([P, ntile_w * 8], I16, tag=f"idxw{tag}", bufs=6)
                nc.sync.dma_start(idxw[:], idx16[:, t0 * 8:(t0 + ntile_w) * 8])
                acc = ps.tile([P, F], F32, tag=f"acc{tag}")
                # batched one-hot builds: 4 S tiles per DVE op
                stiles = []
                for bi in range(0, ntile_w, 4):
                    k = min(4, ntile_w - bi)
                    S4 = sbS.tile([P, 4, P], BF16, tag=f"S{tag}", bufs=8)
                    nc.vector.tensor_tensor(
                        S4[:, :k],
                        dcolsb[:, t0 + bi:t0 + bi + k].to_broadcast([P, k, P]),
                        iotasb[:, :k], op=mybir.AluOpType.is_equal)
                    for j in range(k):
                        stiles.append((S4, j))
                ti = 0
                for cl in range(4):
                    for t_op in ops[w][cl]:
                        g = sb.tile([P, cfg.maxt, F], BF16, tag=f"g{tag}",
                                    bufs=10)
                        nc.gpsimd.dma_gather(
                            g[:, :t_op], cls_tabs[cl][:],
                            idxw[:, ti * 8:(ti + t_op) * 8],
                            t_op * P, t_op * P, F,
                            queue_num=qctr[0] % 4)
                        qctr[0] += 1
                        for t in range(t_op):
                            S4, j = stiles[ti + t]
                            nc.tensor.matmul(acc[:], S4[:, j], g[:, t],
                                             start=(ti + t == 0),
                                             stop=(ti + t == ntile_w - 1))
                        ti += t_op
                return acc

            # ---------------- phase C/D: aggregate layer1, h2' = dinv*(a1@W2)
            if upto.startswith("CD") or upto == "full":
                with tc.tile_pool(name="phC", bufs=4) as sbC, \
                     tc.tile_pool(name="phC_s", bufs=6) as sbS, \
                     tc.tile_pool(name="phC_ps", bufs=2, space="PSUM") as psC, \
                     tc.tile_pool(name="phD_ps", bufs=2, space="PSUM") as psD:
                    for w in range(W):
                        acc = aggregate(w, sbC, sbS, psC, ag1_out, H1, "1")
                        # self loop: agg = dinv*(acc + h1') ; then + b, relu
                        zs = sbC.tile([P, H1], F32, tag="zs")
                        nc.vector.tensor_tensor(zs[:], acc[:], h1buf[:, w],
                                                op=mybir.AluOpType.add)
                        z = sbC.tile([P, H1], F32, tag="z")
                        nc.scalar.activation(z[:], zs[:],
                                             mybir.ActivationFunctionType.Copy,
                                             scale=dinvsb[:, w:w + 1])
                        z2 = sbC.tile([P, H1], F32, tag="z2")
                        nc.vector.tensor_tensor(z2[:], z[:], b1sb[:],
                                                op=mybir.AluOpType.add)
                        a1 = sbC.tile([P, H1], BF16, tag="a1")
                        nc.scalar.activation(a1[:], z2[:],
                                             mybir.ActivationFunctionType.Relu)
                        h2p = psD.tile([P, H2], F32, tag="h2p")
                        for c2 in range(K2):
                            a1tp = psD.tile([P, P], BF16, tag="a1tp")
                            nc.tensor.transpose(a1tp[:], a1[:, c2 * P:(c2 + 1) * P],
                                                idsb[:])
                            a1t = sbC.tile([P, P], BF16, tag="a1t")
                            nc.vector.tensor_copy(a1t[:], a1tp[:])
                            nc.tensor.matmul(h2p[:], a1t[:], w2sb[:, c2],
                                             start=(c2 == 0), stop=(c2 == K2 - 1))
                        nc.scalar.activation(h2buf[:, w], h2p[:],
                                             mybir.ActivationFunctionType.Copy,
                                             scale=dinvsb[:, w:w + 1])
                        for c4 in range(4):
                            nc.sync.dma_start(ag2_in[c4][w * 32:(w + 1) * 32, :],
                                              h2buf[c4 * 32:(c4 + 1) * 32, w])

                for c4 in range(4):
                    if NC == 1:
                        nc.sync.dma_start(ag2_out[c4][:], ag2_in[c4][:])
                    else:
                        nc.gpsimd.collective_compute(
                            "AllGather", mybir.AluOpType.bypass,
                            ins=[ag2_in[c4][:]], outs=[ag2_out[c4][:]],
                            replica_groups=rg)

            # ---------------- phase E/F: aggregate layer2, logits, log_softmax
            if upto == "full":
                with tc.tile_pool(name="phE", bufs=4) as sbE, \
                     tc.tile_pool(name="phE_s", bufs=6) as sbS2, \
                     tc.tile_pool(name="phE_ps", bufs=2, space="PSUM") as psE, \
                     tc.tile_pool(name="phL_ps", bufs=2, space="PSUM") as psL:
                    for w in range(W):
                        acc = aggregate(w, sbE, sbS2, psE, ag2_out, H2, "2")
                        zs = sbE.tile([P, H2], F32, tag="zse")
                        nc.vector.tensor_tensor(zs[:], acc[:], h2buf[:, w],
                                                op=mybir.AluOpType.add)
                        z = sbE.tile([P, H2], F32, tag="ze")
                        nc.scalar.activation(z[:], zs[:],
                                             mybir.ActivationFunctionType.Copy,
                                             scale=dinvsb[:, w:w + 1])
                        z2 = sbE.tile([P, H2], F32, tag="z2e")
                        nc.vector.tensor_tensor(z2[:], z[:], b2sb[:],
                                                op=mybir.AluOpType.add)
                        a2 = sbE.tile([P, H2], BF16, tag="a2")
                        nc.scalar.activation(a2[:], z2[:],
                                             mybir.ActivationFunctionType.Relu)
                        a2tp = psL.tile([P, P], BF16, tag="a2tp")
                        nc.tensor.transpose(a2tp[:], a2[:], idsb[:])
                        a2t = sbE.tile([P, P], BF16, tag="a2t")
                        nc.vector.tensor_copy(a2t[:], a2tp[:])
                        lg = psL.tile([P, C], F32, tag="lg")
                        nc.tensor.matmul(lg[:], a2t[:], wlsb[:], start=True, stop=True)
                        nc.vector.tensor_tensor(lgst[:, w * C:(w + 1) * C], lg[:],
                                                blsb[:], op=mybir.AluOpType.add)
                        e = sbE.tile([P, C], F32, tag="e")
                        nc.scalar.activation(e[:], lgst[:, w * C:(w + 1) * C],
                                             mybir.ActivationFunctionType.Exp,
                                             accum_out=sst[:, w:w + 1])
                    lns = cpool.tile([P, W], F32)
                    nc.scalar.activation(lns[:], sst[:],
                                         mybir.ActivationFunctionType.Ln)
                    for w in range(W):
                        nc.vector.tensor_scalar(
                            outsb[:, w * C:(w + 1) * C], lgst[:, w * C:(w + 1) * C],
                            lns[:, w:w + 1], None, op0=mybir.AluOpType.subtract)
                    nc.sync.dma_start(outst[:], outsb[:])
            else:
                # debug variants: dummy output proving the kept phases ran
                nc.vector.memset(outsb[:], 0.0)
                if upto != "noop":
                    probe_src = ag1_out[0] if upto == "A" else ag2_out[0]
                    probe = cpool.tile([P, C], BF16)
                    nc.sync.dma_start(probe[:], probe_src[:P, :C])
                    nc.vector.tensor_copy(outsb[:, :C], probe[:])
                nc.sync.dma_start(outst[:], outsb[:])

    nc.compile()
    return nc

# ---------------------------------------------------------------- entry point
_CACHE = {}


def _get_compiled(cfg, key, T, ops):
    if key not in _CACHE:
        nc = build_kernel(cfg, T, ops)
        nc.m = get_hw_module(nc.m)
        _CACHE[key] = nc
    return _CACHE[key]


def run(cfg, inputs):
    in_maps, meta = preprocess(cfg, **inputs)
    key = (cfg.N, cfg.F_IN, meta["TT"])
    nc = _get_compiled(cfg, key, meta["T"], meta["ops"])
    res = bass_utils.run_bass_kernel_spmd(
        nc, in_maps, core_ids=list(range(cfg.ncores)))
    out = assemble_output(cfg, meta, res.results)
    return out, res


class _TimedRunner:
    """PJRT runner mirroring bass2jax.run_bass_via_pjrt's multi-core branch,
    but with a cached jit and device-resident inputs for repeatable timing."""

    def __init__(self, nc, n_cores):
        import jax
        import concourse.mybir as mb
        from concourse import bass2jax
        from jax.sharding import Mesh, PartitionSpec, NamedSharding
        from jax.experimental.shard_map import shard_map

        bass2jax.install_neuronx_cc_hook()
        partition_name = (nc.partition_id_tensor.name
                          if nc.partition_id_tensor else None)
        in_names, out_names, out_avals, zero_shapes = [], [], [], []
        for alloc in nc.m.functions[0].allocations:
            if not isinstance(alloc, mb.MemoryLocationSet):
                continue
            name = alloc.memorylocations[0].name
            if alloc.kind == "ExternalInput":
                if name != partition_name:
                    in_names.append(name)
            elif alloc.kind == "ExternalOutput":
                out_names.append(name)
                shape = tuple(alloc.tensor_shape)
                dtype = mb.dt.np(alloc.dtype)
                out_avals.append(jax.core.ShapedArray(shape, dtype))
                zero_shapes.append((shape, dtype))
        n_params = len(in_names)
        all_in_names = list(in_names) + list(out_names)
        if partition_name is not None:
            all_in_names.append(partition_name)
        donate = tuple(range(n_params, n_params + len(out_names)))

        def _body(*args):
            operands = list(args)
            if partition_name is not None:
                operands.append(bass2jax.partition_id_tensor())
            outs = bass2jax._bass_exec_p.bind(
                *operands,
                out_avals=tuple(out_avals),
                in_names=tuple(all_in_names),
                out_names=tuple(out_names),
                lowering_input_output_aliases=(),
                sim_require_finite=True,
                sim_require_nnan=True,
                nc=nc,
            )
            return tuple(outs)

        devices = jax.devices()[:n_cores]
        mesh = Mesh(np.asarray(devices), ("core",))
        in_specs = (PartitionSpec("core"),) * (n_params + len(out_names))
        out_specs = (PartitionSpec("core"),) * len(out_names)
        self.fn = jax.jit(
            shard_map(_body, mesh=mesh, in_specs=in_specs,
                      out_specs=out_specs, check_rep=False),
            donate_argnums=donate, keep_unused=True)
        self.jax = jax
        self.mesh = mesh
        self.sharding = NamedSharding(mesh, PartitionSpec("core"))
        self.in_names = in_names
        self.out_names = out_names
        self.zero_shapes = zero_shapes
        self.n_cores = n_cores
        self.dev_inputs = None

    def stage_inputs(self, in_maps):
        concat_in = [
            np.concatenate([np.asarray(in_maps[c][n])
                            for c in range(self.n_cores)], axis=0)
            for n in self.in_names
        ]
        self.dev_inputs = [self.jax.device_put(a, self.sharding)
                           for a in concat_in]
        for a in self.dev_inputs:
            a.block_until_ready()

    def exec_once(self):
        import time
        zeros = [np.zeros((self.n_cores * s[0], *s[1:]), d)
                 for s, d in self.zero_shapes]
        dz = [self.jax.device_put(z, self.sharding) for z in zeros]
        for z in dz:
            z.block_until_ready()
        t0 = time.perf_counter()
        outs = self.fn(*self.dev_inputs, *dz)
        for o in outs:
            o.block_until_ready()
        t1 = time.perf_counter()
        return outs, t1 - t0

    def results(self, outs):
        res = []
        for c in range(self.n_cores):
            m = {}
            for i, n in enumerate(self.out_names):
                full = np.asarray(outs[i])
                per = full.reshape(self.n_cores, -1, *full.shape[1:])[c]
                m[n] = per
            res.append(m)
        return res


def run_timed(cfg, inputs, iters=3):
    in_maps, meta = preprocess(cfg, **inputs)
    key = (cfg.N, cfg.F_IN, meta["TT"])
    nc = _get_compiled(cfg, key, meta["T"], meta["ops"])
    rkey = ("runner",) + key
    if rkey not in _CACHE:
        _CACHE[rkey] = _TimedRunner(nc, cfg.ncores)
    runner = _CACHE[rkey]
    runner.stage_inputs(in_maps)
    times = []
    outs = None
    for _ in range(iters):
        outs, dt = runner.exec_once()
        times.append(dt)
    results = runner.results(outs)
    out = assemble_output(cfg, meta, results)
    return out, times


def kernel(x, edge_index, W1, b1, W2, b2, Wl, bl):
    out, _ = run(FULL, dict(x=x, edge_index=edge_index, W1=W1, b1=b1,
                            W2=W2, b2=b2, Wl=Wl, bl=bl))
    return out



# revision 25
# speedup vs baseline: 26.0227x; 1.0616x over previous
"""Distributed 2-layer GCN (PyG GCNConv-style) on 8 Trainium2 NeuronCores.

Strategy (hardcoded for N=100000, E=3.2M, 512->256->128->4):
  - Nodes are degree-balanced into (ncores*W) windows of 128 nodes; window b is
    owned by core (b % ncores). A node's "global position" is its row in the
    AllGathered feature table, so gathers use plain int positions.
  - Per layer: local dense matmul (bf16 on PE, fp32 PSUM), rows pre-scaled by
    dinv, results AllGathered to a replicated bf16 feature table in DRAM.
  - Aggregation: per (window, class) block of dst-sorted edges, a dma_gather
    (custom SWDGE gather, int16 indices; the table is viewed in 4 strided
    classes of row%4 so indices fit int16) pulls source rows into SBUF; a
    one-hot S matrix built on DVE (is_equal vs iota, 4 tiles per op, bf16)
    feeds a PE matmul S^T @ msgs that segment-sums into the window's PSUM
    accumulator. Gathers stripe across 4 SWDGE queues (2.3x faster Q7
    descriptor generation). Padding slots carry dcol=128 which never
    matches iota -> contribute 0.
  - Self loops are folded into the epilogue: agg = dinv*(acc + h'own) + b
    with h' windows retained in SBUF (saves ~100K gather descriptors).
  - Epilogue: relu; layer 2 repeats; final logits + log softmax.
"""
import math
import numpy as np

import concourse.bass as bass
import concourse.mybir as mybir
import concourse.bass_utils as bass_utils
from concourse import bacc, tile
from concourse.bass_interp import get_hw_module

P = 128
F32 = mybir.dt.float32
BF16 = mybir.dt.bfloat16
I16 = mybir.dt.int16


class Cfg:
    def __init__(self, N, F_IN, H1, H2, C, ncores=8, W=None, maxt=8):
        self.N, self.F_IN, self.H1, self.H2, self.C = N, F_IN, H1, H2, C
        self.ncores = ncores
        B = ncores * P
        self.W = W if W is not None else math.ceil(N / B)
        self.NPAD = self.W * B
        assert self.NPAD >= N and self.NPAD % 4 == 0
        self.CLS = self.NPAD // 4
        assert self.CLS <= 32768
        self.KI = F_IN // P
        self.K2 = H1 // P
        self.maxt = maxt
        self.B = self.W * P  # nodes per core


FULL = Cfg(N=100000, F_IN=512, H1=256, H2=128, C=4)


# ---------------------------------------------------------------- host side
def preprocess(cfg, x, edge_index, W1, b1, W2, b2, Wl, bl):
    N, NC, W, NPAD, B = cfg.N, cfg.ncores, cfg.W, cfg.NPAD, cfg.B
    NW = NC * W

    src = np.asarray(edge_index[0], dtype=np.int64)
    dst = np.asarray(edge_index[1], dtype=np.int64)
    deg = np.bincount(dst, minlength=N).astype(np.float64) + 1.0
    dinv = (1.0 / np.sqrt(deg)).astype(np.float32)

    # node -> global position, degree-balanced across windows (snake fill)
    degall = np.zeros(NPAD, np.int64)
    degall[:N] = deg.astype(np.int64)
    order = np.argsort(-degall, kind="stable")
    i = np.arange(NPAD)
    phase = i % (2 * NW)
    binid = np.where(phase < NW, phase, 2 * NW - 1 - phase)
    by_bin = np.argsort(binid, kind="stable")
    slot = np.empty(NPAD, np.int64)
    slot[by_bin] = i % P  # within each bin, slots fill 0..127 in arrival order
    # position: core = bin % NC, window = bin // NC
    core_of_bin = binid % NC
    w_of_bin = binid // NC
    g_of_i = core_of_bin * B + w_of_bin * P + slot
    pos = np.empty(NPAD, np.int64)
    pos[order] = g_of_i

    node_at = np.empty(NPAD, np.int64)
    node_at[pos] = np.arange(NPAD)

    # edge list; self loops are folded into the epilogue on-device
    S_pos = pos[src]
    D_pos = pos[dst]
    core_e = D_pos // B
    w_e = (D_pos % B) // P
    dcol_e = (D_pos % P).astype(np.float32)
    # class tables: class = src_window % 4 (so each class's AllGather can
    # fire as soon as its quarter of windows is computed). Row in class
    # table = rank*B4cl + (w//4)*128 + slot.
    W_cl = [len(range(cl, W, 4)) for cl in range(4)]
    B4cl = [wc * P for wc in W_cl]
    s_slot = S_pos % P
    w_src = (S_pos % B) // P
    cls_e = (w_src % 4).astype(np.int64)
    b4_of = np.asarray(B4cl, np.int64)[cls_e]
    idx16_e = ((S_pos // B) * b4_of + (w_src // 4) * P
               + s_slot).astype(np.int16)

    key = ((core_e * W + w_e) * 4 + cls_e).astype(np.int64)
    ordE = np.argsort(key, kind="stable")
    counts = np.bincount(key, minlength=NC * W * 4).reshape(NC, W, 4)
    T = np.maximum(1, np.ceil(counts / P).astype(np.int64).max(axis=0))  # [W, 4]
    TT = int(T.sum())
    SLOT = TT * P

    # ops schedule per (w, cl): list of tile counts. Ops of >=4 tiles
    # (>=512 idx) pay no SWDGE fixed overhead; split e.g. 9 -> [5, 4].
    ops = [[[] for _ in range(4)] for _ in range(W)]
    for w in range(W):
        for cl in range(4):
            t = int(T[w][cl])
            while t > 0:
                c = min(t, cfg.maxt)
                if t > cfg.maxt and t - cfg.maxt < 4:
                    c = t - 4
                ops[w][cl].append(c)
                t -= c

    starts = np.zeros(NC * W * 4 + 1, np.int64)
    np.cumsum(counts.reshape(-1), out=starts[1:])
    blk_off = np.zeros((W, 4), np.int64)  # slot offset of each (w, cl) block
    acc_off = 0
    for w in range(W):
        for cl in range(4):
            blk_off[w, cl] = acc_off
            acc_off += int(T[w][cl]) * P

    idx16 = np.zeros((NC, SLOT), np.int16)
    dcol = np.full((NC, SLOT), float(P), np.float32)  # cast bf16 at ship time
    for c in range(NC):
        for w in range(W):
            for cl in range(4):
                k = (c * W + w) * 4 + cl
                s0, s1 = starts[k], starts[k + 1]
                n = s1 - s0
                off = blk_off[w, cl]
                seg = ordE[s0:s1]
                idx16[c, off:off + n] = idx16_e[seg]
                dcol[c, off:off + n] = dcol_e[seg]

    # wrap idx16 per-op: element i of an op at [i%16, i//16], replicated x8
    idx_w = np.zeros((NC, 16, SLOT // 16), np.int16)
    for w in range(W):
        for cl in range(4):
            off = int(blk_off[w, cl])
            for t_op in ops[w][cl]:
                n = t_op * P
                blk = idx16[:, off:off + n].reshape(NC, n // 16, 16)
                idx_w[:, :, off // 16:(off + n) // 16] = blk.transpose(0, 2, 1)
                off += n
    idx_rep = np.tile(idx_w, (1, 8, 1))  # [NC, 128, SLOT//16]

    dcol_t = dcol.reshape(NC, TT, P).transpose(0, 2, 1).copy()  # [NC, 128, TT]

    # x shard, transposed chunk layout: xt4[p, i, c2, m] = x[node(w*128+m), c2*128+p]
    # with i indexing windows in class-major (worder) order for batched loads.
    worder = [w for cl in range(4) for w in range(cl, W, 4)]
    xpad = np.zeros((NPAD, cfg.F_IN), np.float32)
    xpad[:N] = np.asarray(x, np.float32)
    dinvpad = np.ones(NPAD, np.float32)
    dinvpad[:N] = dinv

    xt4 = np.empty((NC, P, W, cfg.KI, P), np.float32)
    dinvl = np.empty((NC, P, W), np.float32)
    for c in range(NC):
        ids = node_at[c * B:(c + 1) * B]
        xl = xpad[ids]  # [B, F_IN]
        xt4[c] = xl.reshape(W, P, cfg.KI, P).transpose(3, 0, 2, 1)[:, worder]
        dinvl[c] = dinvpad[ids].reshape(W, P).T

    w1d = np.asarray(W1, np.float32).reshape(cfg.KI, P, cfg.H1).transpose(1, 0, 2)
    w2d = np.asarray(W2, np.float32).reshape(cfg.K2, P, cfg.H2).transpose(1, 0, 2)
    wld = np.asarray(Wl, np.float32)  # [H2=128, C]
    b1b = np.broadcast_to(np.asarray(b1, np.float32), (P, cfg.H1)).copy()
    b2b = np.broadcast_to(np.asarray(b2, np.float32), (P, cfg.H2)).copy()
    blb = np.broadcast_to(np.asarray(bl, np.float32), (P, cfg.C)).copy()
    iota4 = np.broadcast_to(np.arange(P, dtype=np.float32),
                            (P, 4, P)).reshape(P, 4 * P).copy()
    ident = np.eye(P, dtype=np.float32)

    import ml_dtypes
    bf = lambda a: a.astype(ml_dtypes.bfloat16)

    in_maps = []
    for c in range(NC):
        in_maps.append({
            "xt4": bf(xt4[c]),
            "w1d": bf(w1d), "w2d": bf(w2d), "wld": bf(wld),
            "b1b": b1b, "b2b": b2b, "blb": blb,
            "dinvl": dinvl[c],
            "idx16": idx_rep[c],
            "dcol": bf(dcol_t[c]),
            "iota": bf(iota4),
            "ident": bf(ident),
        })

    meta = dict(T=T, ops=ops, pos=pos, node_at=node_at, SLOT=SLOT, TT=TT)
    return in_maps, meta


def assemble_output(cfg, meta, results):
    N, NC, W, C, B = cfg.N, cfg.ncores, cfg.W, cfg.C, cfg.B
    rows = []
    for c in range(NC):
        r = results[c]["outst"].reshape(P, W, C).transpose(1, 0, 2).reshape(B, C)
        rows.append(r)
    allrows = np.concatenate(rows, axis=0)  # [NPAD, C] in position order
    return allrows[meta["pos"][:N]].astype(np.float32)


# ---------------------------------------------------------------- device side
def build_kernel(cfg, T, ops, upto="full"):
    NC, W, NPAD, B = cfg.ncores, cfg.W, cfg.NPAD, cfg.B
    H1, H2, C, KI, K2 = cfg.H1, cfg.H2, cfg.C, cfg.KI, cfg.K2
    TT = int(np.asarray(T).sum())
    SLOT = TT * P

    nc = bacc.Bacc("TRN2", target_bir_lowering=False, debug=False,
                   num_devices=NC, num_swdge_queues=4)

    xt4 = nc.dram_tensor("xt4", [P, W, KI, P], BF16, kind="ExternalInput")
    w1d = nc.dram_tensor("w1d", [P, KI, H1], BF16, kind="ExternalInput")
    w2d = nc.dram_tensor("w2d", [P, K2, H2], BF16, kind="ExternalInput")
    wld = nc.dram_tensor("wld", [P, C], BF16, kind="ExternalInput")
    b1b = nc.dram_tensor("b1b", [P, H1], F32, kind="ExternalInput")
    b2b = nc.dram_tensor("b2b", [P, H2], F32, kind="ExternalInput")
    blb = nc.dram_tensor("blb", [P, C], F32, kind="ExternalInput")
    dinvl = nc.dram_tensor("dinvl", [P, W], F32, kind="ExternalInput")
    idx16 = nc.dram_tensor("idx16", [P, SLOT // 16], I16, kind="ExternalInput")
    dcol = nc.dram_tensor("dcol", [P, TT], BF16, kind="ExternalInput")
    iota = nc.dram_tensor("iota", [P, 4 * P], BF16, kind="ExternalInput")
    ident = nc.dram_tensor("ident", [P, P], BF16, kind="ExternalInput")
    outst = nc.dram_tensor("outst", [P, W * C], F32, kind="ExternalOutput")

    # per-window column ranges in idx16 / dcol
    blk_tiles = np.asarray(T)  # [W, 4]
    w_tile_off = np.zeros(W + 1, np.int64)
    np.cumsum(blk_tiles.sum(axis=1), out=w_tile_off[1:])

    rg = [list(range(NC))]

    # class = src_window % 4; class-major window order so each class's
    # AllGather fires as soon as its quarter of windows is computed
    worder = [w for cl in range(4) for w in range(cl, W, 4)]
    W_cl = [len(range(cl, W, 4)) for cl in range(4)]
    B4cl = [wc * P for wc in W_cl]
    last_w_of_cl = {cl: [w for w in range(cl, W, 4)][-1] for cl in range(4)}
    with tile.TileContext(nc) as tc:
        with tc.tile_pool(name="const", bufs=1) as cpool, \
             tc.tile_pool(name="dram", bufs=1, space="DRAM") as dram:
            ag1_in = [dram.tile([B4cl[i], H1], BF16, name=f"ag1i{i}")
                      for i in range(4)]
            ag1_out = [dram.tile([NC * B4cl[i], H1], BF16, addr_space="Shared",
                                 name=f"ag1o{i}") for i in range(4)]
            ag2_in = [dram.tile([B4cl[i], H2], BF16, name=f"ag2i{i}")
                      for i in range(4)]
            ag2_out = [dram.tile([NC * B4cl[i], H2], BF16, addr_space="Shared",
                                 name=f"ag2o{i}") for i in range(4)]

            w1sb = cpool.tile([P, KI, H1], BF16)
            nc.sync.dma_start(w1sb[:], w1d[:])
            w2sb = cpool.tile([P, K2, H2], BF16)
            nc.sync.dma_start(w2sb[:], w2d[:])
            wlsb = cpool.tile([P, C], BF16)
            nc.sync.dma_start(wlsb[:], wld[:])
            b1sb = cpool.tile([P, H1], F32)
            nc.sync.dma_start(b1sb[:], b1b[:])
            b2sb = cpool.tile([P, H2], F32)
            nc.sync.dma_start(b2sb[:], b2b[:])
            blsb = cpool.tile([P, C], F32)
            nc.sync.dma_start(blsb[:], blb[:])
            dinvsb = cpool.tile([P, W], F32)
            nc.sync.dma_start(dinvsb[:], dinvl[:])
            iotasb = cpool.tile([P, 4, P], BF16)
            nc.sync.dma_start(iotasb[:], iota[:].rearrange("p (a b) -> p a b", a=4))
            idsb = cpool.tile([P, P], BF16)
            nc.sync.dma_start(idsb[:], ident[:])
            dcolsb = cpool.tile([P, TT], BF16)
            nc.sync.dma_start(dcolsb[:], dcol[:])

            lgst = cpool.tile([P, W * C], F32)
            sst = cpool.tile([P, W], F32)
            outsb = cpool.tile([P, W * C], F32)
            # retained h' windows for the self-loop epilogue term
            h1buf = cpool.tile([P, W, H1], BF16)
            h2buf = cpool.tile([P, W, H2], BF16)

            def allgather(cl, ag_in, ag_out):
                if NC == 1:
                    nc.sync.dma_start(ag_out[cl][:], ag_in[cl][:])
                else:
                    nc.gpsimd.collective_compute(
                        "AllGather", mybir.AluOpType.bypass,
                        ins=[ag_in[cl][:]], outs=[ag_out[cl][:]],
                        replica_groups=rg)

            # ---------------- phase A: h1' = dinv * (x @ W1), allgather per
            # class as soon as its windows are done (class-major worder)
            XB = 4  # windows per xt4 load
            with tc.tile_pool(name="phA", bufs=3) as sbA, \
                 tc.tile_pool(name="phA_ps", bufs=2, space="PSUM") as psA:
                for i0 in (range(0, W, XB) if upto != "noop" else []):
                    nb = min(XB, W - i0)
                    xt = sbA.tile([P, XB, KI, P], BF16, tag="xt")
                    nc.sync.dma_start(xt[:, :nb], xt4[:, i0:i0 + nb])
                    for k in range(nb):
                        w = worder[i0 + k]
                        cl = w % 4
                        hp = psA.tile([P, H1], F32, tag="hp")
                        for c2 in range(KI):
                            nc.tensor.matmul(hp[:], xt[:, k, c2], w1sb[:, c2],
                                             start=(c2 == 0),
                                             stop=(c2 == KI - 1))
                        nc.scalar.activation(h1buf[:, w], hp[:],
                                             mybir.ActivationFunctionType.Copy,
                                             scale=dinvsb[:, w:w + 1])
                        widx = w // 4
                        nc.sync.dma_start(
                            ag1_in[cl][widx * P:(widx + 1) * P, :],
                            h1buf[:, w])
                        if w == last_w_of_cl[cl]:
                            allgather(cl, ag1_in, ag1_out)

            qctr = [0]

            def aggregate(w, sb, sbS, ps, cls_tabs, F, tag):
                """Gather + S-matmul segment-sum for window w at width F.
                Returns the PSUM accumulator tile."""
                t0 = int(w_tile_off[w])
                ntile_w = int(w_tile_off[w + 1] - w_tile_off[w])
                idxw = sb.tile([P, ntile_w * 8], I16, tag=f"idxw{tag}", bufs=6)
                nc.sync.dma_start(idxw[:], idx16[:, t0 * 8:(t0 + ntile_w) * 8])
                acc = ps.tile([P, F], F32, tag=f"acc{tag}")
                # batched one-hot builds: 4 S tiles per DVE op
                stiles = []
                for bi in range(0, ntile_w, 4):
                    k = min(4, ntile_w - bi)
                    S4 = sbS.tile([P, 4, P], BF16, tag=f"S{tag}", bufs=8)
                    nc.vector.tensor_tensor(
                        S4[:, :k],
                        dcolsb[:, t0 + bi:t0 + bi + k].to_broadcast([P, k, P]),
                        iotasb[:, :k], op=mybir.AluOpType.is_equal)
                    for j in range(k):
                        stiles.append((S4, j))
                ti = 0
                for cl in range(4):
                    for t_op in ops[w][cl]:
                        g = sb.tile([P, cfg.maxt, F], BF16, tag=f"g{tag}",
                                    bufs=10)
                        nc.gpsimd.dma_gather(
                            g[:, :t_op], cls_tabs[cl][:],
                            idxw[:, ti * 8:(ti + t_op) * 8],
                            t_op * P, t_op * P, F,
                            queue_num=qctr[0] % 4)
                        qctr[0] += 1
                        for t in range(t_op):
                            S4, j = stiles[ti + t]
                            nc.tensor.matmul(acc[:], S4[:, j], g[:, t],
                                             start=(ti + t == 0),
                                             stop=(ti + t == ntile_w - 1))
                        ti += t_op
                return acc

            # ---------------- phase C/D: aggregate layer1, h2' = dinv*(a1@W2)
            if upto.startswith("CD") or upto == "full":
                with tc.tile_pool(name="phC", bufs=4) as sbC, \
                     tc.tile_pool(name="phC_s", bufs=6) as sbS, \
                     tc.tile_pool(name="phC_ps", bufs=2, space="PSUM") as psC, \
                     tc.tile_pool(name="phD_ps", bufs=2, space="PSUM") as psD:
                    for w in worder:
                        acc = aggregate(w, sbC, sbS, psC, ag1_out, H1, "1")
                        # self loop: agg = dinv*(acc + h1') ; then + b, relu
                        zs = sbC.tile([P, H1], F32, tag="zs")
                        nc.vector.tensor_tensor(zs[:], acc[:], h1buf[:, w],
                                                op=mybir.AluOpType.add)
                        z = sbC.tile([P, H1], F32, tag="z")
                        nc.scalar.activation(z[:], zs[:],
                                             mybir.ActivationFunctionType.Copy,
                                             scale=dinvsb[:, w:w + 1])
                        z2 = sbC.tile([P, H1], F32, tag="z2")
                        nc.vector.tensor_tensor(z2[:], z[:], b1sb[:],
                                                op=mybir.AluOpType.add)
                        a1 = sbC.tile([P, H1], BF16, tag="a1")
                        nc.scalar.activation(a1[:], z2[:],
                                             mybir.ActivationFunctionType.Relu)
                        h2p = psD.tile([P, H2], F32, tag="h2p")
                        for c2 in range(K2):
                            a1tp = psD.tile([P, P], BF16, tag="a1tp")
                            nc.tensor.transpose(a1tp[:], a1[:, c2 * P:(c2 + 1) * P],
                                                idsb[:])
                            a1t = sbC.tile([P, P], BF16, tag="a1t")
                            nc.vector.tensor_copy(a1t[:], a1tp[:])
                            nc.tensor.matmul(h2p[:], a1t[:], w2sb[:, c2],
                                             start=(c2 == 0), stop=(c2 == K2 - 1))
                        nc.scalar.activation(h2buf[:, w], h2p[:],
                                             mybir.ActivationFunctionType.Copy,
                                             scale=dinvsb[:, w:w + 1])
                        cl = w % 4
                        widx = w // 4
                        nc.sync.dma_start(
                            ag2_in[cl][widx * P:(widx + 1) * P, :],
                            h2buf[:, w])
                        if w == last_w_of_cl[cl]:
                            allgather(cl, ag2_in, ag2_out)

            # ---------------- phase E/F: aggregate layer2, logits, log_softmax
            if upto == "full":
                with tc.tile_pool(name="phE", bufs=4) as sbE, \
                     tc.tile_pool(name="phE_s", bufs=6) as sbS2, \
                     tc.tile_pool(name="phE_ps", bufs=2, space="PSUM") as psE, \
                     tc.tile_pool(name="phL_ps", bufs=2, space="PSUM") as psL:
                    for w in worder:
                        acc = aggregate(w, sbE, sbS2, psE, ag2_out, H2, "2")
                        zs = sbE.tile([P, H2], F32, tag="zse")
                        nc.vector.tensor_tensor(zs[:], acc[:], h2buf[:, w],
                                                op=mybir.AluOpType.add)
                        z = sbE.tile([P, H2], F32, tag="ze")
                        nc.scalar.activation(z[:], zs[:],
                                             mybir.ActivationFunctionType.Copy,
                                             scale=dinvsb[:, w:w + 1])
                        z2 = sbE.tile([P, H2], F32, tag="z2e")
                        nc.vector.tensor_tensor(z2[:], z[:], b2sb[:],
                                                op=mybir.AluOpType.add)
                        a2 = sbE.tile([P, H2], BF16, tag="a2")
                        nc.scalar.activation(a2[:], z2[:],
                                             mybir.ActivationFunctionType.Relu)
                        a2tp = psL.tile([P, P], BF16, tag="a2tp")
                        nc.tensor.transpose(a2tp[:], a2[:], idsb[:])
                        a2t = sbE.tile([P, P], BF16, tag="a2t")
                        nc.vector.tensor_copy(a2t[:], a2tp[:])
                        lg = psL.tile([P, C], F32, tag="lg")
                        nc.tensor.matmul(lg[:], a2t[:], wlsb[:], start=True, stop=True)
                        nc.vector.tensor_tensor(lgst[:, w * C:(w + 1) * C], lg[:],
                                                blsb[:], op=mybir.AluOpType.add)
                        e = sbE.tile([P, C], F32, tag="e")
                        nc.scalar.activation(e[:], lgst[:, w * C:(w + 1) * C],
                                             mybir.ActivationFunctionType.Exp,
                                             accum_out=sst[:, w:w + 1])
                    lns = cpool.tile([P, W], F32)
                    nc.scalar.activation(lns[:], sst[:],
                                         mybir.ActivationFunctionType.Ln)
                    for w in range(W):
                        nc.vector.tensor_scalar(
                            outsb[:, w * C:(w + 1) * C], lgst[:, w * C:(w + 1) * C],
                            lns[:, w:w + 1], None, op0=mybir.AluOpType.subtract)
                    nc.sync.dma_start(outst[:], outsb[:])
            else:
                # debug variants: dummy output proving the kept phases ran
                nc.vector.memset(outsb[:], 0.0)
                if upto != "noop":
                    probe_src = ag1_out[0] if upto == "A" else ag2_out[0]
                    probe = cpool.tile([P, C], BF16)
                    nc.sync.dma_start(probe[:], probe_src[:P, :C])
                    nc.vector.tensor_copy(outsb[:, :C], probe[:])
                nc.sync.dma_start(outst[:], outsb[:])

    nc.compile()
    return nc

# ---------------------------------------------------------------- entry point
_CACHE = {}


def _get_compiled(cfg, key, T, ops):
    if key not in _CACHE:
        nc = build_kernel(cfg, T, ops)
        nc.m = get_hw_module(nc.m)
        _CACHE[key] = nc
    return _CACHE[key]


def run(cfg, inputs):
    in_maps, meta = preprocess(cfg, **inputs)
    key = (cfg.N, cfg.F_IN, meta["TT"])
    nc = _get_compiled(cfg, key, meta["T"], meta["ops"])
    res = bass_utils.run_bass_kernel_spmd(
        nc, in_maps, core_ids=list(range(cfg.ncores)))
    out = assemble_output(cfg, meta, res.results)
    return out, res


class _TimedRunner:
    """PJRT runner mirroring bass2jax.run_bass_via_pjrt's multi-core branch,
    but with a cached jit and device-resident inputs for repeatable timing."""

    def __init__(self, nc, n_cores):
        import jax
        import concourse.mybir as mb
        from concourse import bass2jax
        from jax.sharding import Mesh, PartitionSpec, NamedSharding
        from jax.experimental.shard_map import shard_map

        bass2jax.install_neuronx_cc_hook()
        partition_name = (nc.partition_id_tensor.name
                          if nc.partition_id_tensor else None)
        in_names, out_names, out_avals, zero_shapes = [], [], [], []
        for alloc in nc.m.functions[0].allocations:
            if not isinstance(alloc, mb.MemoryLocationSet):
                continue
            name = alloc.memorylocations[0].name
            if alloc.kind == "ExternalInput":
                if name != partition_name:
                    in_names.append(name)
            elif alloc.kind == "ExternalOutput":
                out_names.append(name)
                shape = tuple(alloc.tensor_shape)
                dtype = mb.dt.np(alloc.dtype)
                out_avals.append(jax.core.ShapedArray(shape, dtype))
                zero_shapes.append((shape, dtype))
        n_params = len(in_names)
        all_in_names = list(in_names) + list(out_names)
        if partition_name is not None:
            all_in_names.append(partition_name)
        donate = tuple(range(n_params, n_params + len(out_names)))

        def _body(*args):
            operands = list(args)
            if partition_name is not None:
                operands.append(bass2jax.partition_id_tensor())
            outs = bass2jax._bass_exec_p.bind(
                *operands,
                out_avals=tuple(out_avals),
                in_names=tuple(all_in_names),
                out_names=tuple(out_names),
                lowering_input_output_aliases=(),
                sim_require_finite=True,
                sim_require_nnan=True,
                nc=nc,
            )
            return tuple(outs)

        devices = jax.devices()[:n_cores]
        mesh = Mesh(np.asarray(devices), ("core",))
        in_specs = (PartitionSpec("core"),) * (n_params + len(out_names))
        out_specs = (PartitionSpec("core"),) * len(out_names)
        self.fn = jax.jit(
            shard_map(_body, mesh=mesh, in_specs=in_specs,
                      out_specs=out_specs, check_rep=False),
            donate_argnums=donate, keep_unused=True)
        self.jax = jax
        self.mesh = mesh
        self.sharding = NamedSharding(mesh, PartitionSpec("core"))
        self.in_names = in_names
        self.out_names = out_names
        self.zero_shapes = zero_shapes
        self.n_cores = n_cores
        self.dev_inputs = None

    def stage_inputs(self, in_maps):
        concat_in = [
            np.concatenate([np.asarray(in_maps[c][n])
                            for c in range(self.n_cores)], axis=0)
            for n in self.in_names
        ]
        self.dev_inputs = [self.jax.device_put(a, self.sharding)
                           for a in concat_in]
        for a in self.dev_inputs:
            a.block_until_ready()

    def exec_once(self):
        import time
        zeros = [np.zeros((self.n_cores * s[0], *s[1:]), d)
                 for s, d in self.zero_shapes]
        dz = [self.jax.device_put(z, self.sharding) for z in zeros]
        for z in dz:
            z.block_until_ready()
        t0 = time.perf_counter()
        outs = self.fn(*self.dev_inputs, *dz)
        for o in outs:
            o.block_until_ready()
        t1 = time.perf_counter()
        return outs, t1 - t0

    def results(self, outs):
        res = []
        for c in range(self.n_cores):
            m = {}
            for i, n in enumerate(self.out_names):
                full = np.asarray(outs[i])
                per = full.reshape(self.n_cores, -1, *full.shape[1:])[c]
                m[n] = per
            res.append(m)
        return res


def run_timed(cfg, inputs, iters=3):
    in_maps, meta = preprocess(cfg, **inputs)
    key = (cfg.N, cfg.F_IN, meta["TT"])
    nc = _get_compiled(cfg, key, meta["T"], meta["ops"])
    rkey = ("runner",) + key
    if rkey not in _CACHE:
        _CACHE[rkey] = _TimedRunner(nc, cfg.ncores)
    runner = _CACHE[rkey]
    runner.stage_inputs(in_maps)
    times = []
    outs = None
    for _ in range(iters):
        outs, dt = runner.exec_once()
        times.append(dt)
    results = runner.results(outs)
    out = assemble_output(cfg, meta, results)
    return out, times


def kernel(x, edge_index, W1, b1, W2, b2, Wl, bl):
    out, _ = run(FULL, dict(x=x, edge_index=edge_index, W1=W1, b1=b1,
                            W2=W2, b2=b2, Wl=Wl, bl=bl))
    return out



# revision 26
# speedup vs baseline: 26.1628x; 1.0054x over previous
"""Distributed 2-layer GCN (PyG GCNConv-style) on 8 Trainium2 NeuronCores.

Strategy (hardcoded for N=100000, E=3.2M, 512->256->128->4):
  - Nodes are degree-balanced into (ncores*W) windows of 128 nodes; window b is
    owned by core (b % ncores). A node's "global position" is its row in the
    AllGathered feature table, so gathers use plain int positions.
  - Per layer: local dense matmul (bf16 on PE, fp32 PSUM), rows pre-scaled by
    dinv, results AllGathered to a replicated bf16 feature table in DRAM.
  - Aggregation: per (window, class) block of dst-sorted edges, a dma_gather
    (custom SWDGE gather, int16 indices; the table is viewed in 4 strided
    classes of row%4 so indices fit int16) pulls source rows into SBUF; a
    one-hot S matrix built on DVE (is_equal vs iota, 4 tiles per op, bf16)
    feeds a PE matmul S^T @ msgs that segment-sums into the window's PSUM
    accumulator. Gathers stripe across 4 SWDGE queues (2.3x faster Q7
    descriptor generation). Padding slots carry dcol=128 which never
    matches iota -> contribute 0.
  - Self loops are folded into the epilogue: agg = dinv*(acc + h'own) + b
    with h' windows retained in SBUF (saves ~100K gather descriptors).
  - Epilogue: relu; layer 2 repeats; final logits + log softmax.
"""
import math
import numpy as np

import concourse.bass as bass
import concourse.mybir as mybir
import concourse.bass_utils as bass_utils
from concourse import bacc, tile
from concourse.bass_interp import get_hw_module

P = 128
F32 = mybir.dt.float32
BF16 = mybir.dt.bfloat16
I16 = mybir.dt.int16


class Cfg:
    def __init__(self, N, F_IN, H1, H2, C, ncores=8, W=None, maxt=8):
        self.N, self.F_IN, self.H1, self.H2, self.C = N, F_IN, H1, H2, C
        self.ncores = ncores
        B = ncores * P
        self.W = W if W is not None else math.ceil(N / B)
        self.NPAD = self.W * B
        assert self.NPAD >= N and self.NPAD % 4 == 0
        self.CLS = self.NPAD // 4
        assert self.CLS <= 32768
        self.KI = F_IN // P
        self.K2 = H1 // P
        self.maxt = maxt
        self.B = self.W * P  # nodes per core


FULL = Cfg(N=100000, F_IN=512, H1=256, H2=128, C=4)


# ---------------------------------------------------------------- host side
def preprocess(cfg, x, edge_index, W1, b1, W2, b2, Wl, bl):
    N, NC, W, NPAD, B = cfg.N, cfg.ncores, cfg.W, cfg.NPAD, cfg.B
    NW = NC * W

    src = np.asarray(edge_index[0], dtype=np.int64)
    dst = np.asarray(edge_index[1], dtype=np.int64)
    deg = np.bincount(dst, minlength=N).astype(np.float64) + 1.0
    dinv = (1.0 / np.sqrt(deg)).astype(np.float32)

    # node -> global position, degree-balanced across windows (snake fill)
    degall = np.zeros(NPAD, np.int64)
    degall[:N] = deg.astype(np.int64)
    order = np.argsort(-degall, kind="stable")
    i = np.arange(NPAD)
    phase = i % (2 * NW)
    binid = np.where(phase < NW, phase, 2 * NW - 1 - phase)
    by_bin = np.argsort(binid, kind="stable")
    slot = np.empty(NPAD, np.int64)
    slot[by_bin] = i % P  # within each bin, slots fill 0..127 in arrival order
    # position: core = bin % NC, window = bin // NC
    core_of_bin = binid % NC
    w_of_bin = binid // NC
    g_of_i = core_of_bin * B + w_of_bin * P + slot
    pos = np.empty(NPAD, np.int64)
    pos[order] = g_of_i

    node_at = np.empty(NPAD, np.int64)
    node_at[pos] = np.arange(NPAD)

    # edge list; self loops are folded into the epilogue on-device
    S_pos = pos[src]
    D_pos = pos[dst]
    core_e = D_pos // B
    w_e = (D_pos % B) // P
    dcol_e = (D_pos % P).astype(np.float32)
    # class tables: class = src_window % 4 (so each class's AllGather can
    # fire as soon as its quarter of windows is computed). Row in class
    # table = rank*B4cl + (w//4)*128 + slot.
    W_cl = [len(range(cl, W, 4)) for cl in range(4)]
    B4cl = [wc * P for wc in W_cl]
    s_slot = S_pos % P
    w_src = (S_pos % B) // P
    cls_e = (w_src % 4).astype(np.int64)
    b4_of = np.asarray(B4cl, np.int64)[cls_e]
    idx16_e = ((S_pos // B) * b4_of + (w_src // 4) * P
               + s_slot).astype(np.int16)

    key = ((core_e * W + w_e) * 4 + cls_e).astype(np.int64)
    ordE = np.argsort(key, kind="stable")
    counts = np.bincount(key, minlength=NC * W * 4).reshape(NC, W, 4)
    T = np.maximum(1, np.ceil(counts / P).astype(np.int64).max(axis=0))  # [W, 4]
    TT = int(T.sum())
    SLOT = TT * P

    # ops schedule per (w, cl): list of tile counts. Ops of >=4 tiles
    # (>=512 idx) pay no SWDGE fixed overhead; split e.g. 9 -> [5, 4].
    ops = [[[] for _ in range(4)] for _ in range(W)]
    for w in range(W):
        for cl in range(4):
            t = int(T[w][cl])
            while t > 0:
                c = min(t, cfg.maxt)
                if t > cfg.maxt and t - cfg.maxt < 4:
                    c = t - 4
                ops[w][cl].append(c)
                t -= c

    starts = np.zeros(NC * W * 4 + 1, np.int64)
    np.cumsum(counts.reshape(-1), out=starts[1:])
    blk_off = np.zeros((W, 4), np.int64)  # slot offset of each (w, cl) block
    acc_off = 0
    for w in range(W):
        for cl in range(4):
            blk_off[w, cl] = acc_off
            acc_off += int(T[w][cl]) * P

    idx16 = np.zeros((NC, SLOT), np.int16)
    dcol = np.full((NC, SLOT), float(P), np.float32)  # cast bf16 at ship time
    for c in range(NC):
        for w in range(W):
            for cl in range(4):
                k = (c * W + w) * 4 + cl
                s0, s1 = starts[k], starts[k + 1]
                n = s1 - s0
                off = blk_off[w, cl]
                seg = ordE[s0:s1]
                idx16[c, off:off + n] = idx16_e[seg]
                dcol[c, off:off + n] = dcol_e[seg]

    # wrap idx16 per-op: element i of an op at [i%16, i//16], replicated x8
    idx_w = np.zeros((NC, 16, SLOT // 16), np.int16)
    for w in range(W):
        for cl in range(4):
            off = int(blk_off[w, cl])
            for t_op in ops[w][cl]:
                n = t_op * P
                blk = idx16[:, off:off + n].reshape(NC, n // 16, 16)
                idx_w[:, :, off // 16:(off + n) // 16] = blk.transpose(0, 2, 1)
                off += n
    idx_rep = np.tile(idx_w, (1, 8, 1))  # [NC, 128, SLOT//16]

    dcol_t = dcol.reshape(NC, TT, P).transpose(0, 2, 1).copy()  # [NC, 128, TT]

    # x shard, transposed chunk layout: xt4[p, i, c2, m] = x[node(w*128+m), c2*128+p]
    # with i indexing windows in class-major (worder) order for batched loads.
    worder = [w for cl in range(4) for w in range(cl, W, 4)]
    xpad = np.zeros((NPAD, cfg.F_IN), np.float32)
    xpad[:N] = np.asarray(x, np.float32)
    dinvpad = np.ones(NPAD, np.float32)
    dinvpad[:N] = dinv

    xt4 = np.empty((NC, P, W, cfg.KI, P), np.float32)
    dinvl = np.empty((NC, P, W), np.float32)
    for c in range(NC):
        ids = node_at[c * B:(c + 1) * B]
        xl = xpad[ids]  # [B, F_IN]
        xt4[c] = xl.reshape(W, P, cfg.KI, P).transpose(3, 0, 2, 1)[:, worder]
        dinvl[c] = dinvpad[ids].reshape(W, P).T

    w1d = np.asarray(W1, np.float32).reshape(cfg.KI, P, cfg.H1).transpose(1, 0, 2)
    w2d = np.asarray(W2, np.float32).reshape(cfg.K2, P, cfg.H2).transpose(1, 0, 2)
    wld = np.asarray(Wl, np.float32)  # [H2=128, C]
    b1b = np.broadcast_to(np.asarray(b1, np.float32), (P, cfg.H1)).copy()
    b2b = np.broadcast_to(np.asarray(b2, np.float32), (P, cfg.H2)).copy()
    blb = np.broadcast_to(np.asarray(bl, np.float32), (P, cfg.C)).copy()
    iota4 = np.broadcast_to(np.arange(P, dtype=np.float32),
                            (P, 4, P)).reshape(P, 4 * P).copy()
    ident = np.eye(P, dtype=np.float32)

    import ml_dtypes
    bf = lambda a: a.astype(ml_dtypes.bfloat16)

    in_maps = []
    for c in range(NC):
        in_maps.append({
            "xt4": bf(xt4[c]),
            "w1d": bf(w1d), "w2d": bf(w2d), "wld": bf(wld),
            "b1b": b1b, "b2b": b2b, "blb": blb,
            "dinvl": dinvl[c],
            "idx16": idx_rep[c],
            "dcol": bf(dcol_t[c]),
            "iota": bf(iota4),
            "ident": bf(ident),
        })

    meta = dict(T=T, ops=ops, pos=pos, node_at=node_at, SLOT=SLOT, TT=TT)
    return in_maps, meta


def assemble_output(cfg, meta, results):
    N, NC, W, C, B = cfg.N, cfg.ncores, cfg.W, cfg.C, cfg.B
    rows = []
    for c in range(NC):
        r = results[c]["outst"].reshape(P, W, C).transpose(1, 0, 2).reshape(B, C)
        rows.append(r)
    allrows = np.concatenate(rows, axis=0)  # [NPAD, C] in position order
    return allrows[meta["pos"][:N]].astype(np.float32)


# ---------------------------------------------------------------- device side
def build_kernel(cfg, T, ops, upto="full"):
    NC, W, NPAD, B = cfg.ncores, cfg.W, cfg.NPAD, cfg.B
    H1, H2, C, KI, K2 = cfg.H1, cfg.H2, cfg.C, cfg.KI, cfg.K2
    TT = int(np.asarray(T).sum())
    SLOT = TT * P

    nc = bacc.Bacc("TRN2", target_bir_lowering=False, debug=False,
                   num_devices=NC, num_swdge_queues=4)

    xt4 = nc.dram_tensor("xt4", [P, W, KI, P], BF16, kind="ExternalInput")
    w1d = nc.dram_tensor("w1d", [P, KI, H1], BF16, kind="ExternalInput")
    w2d = nc.dram_tensor("w2d", [P, K2, H2], BF16, kind="ExternalInput")
    wld = nc.dram_tensor("wld", [P, C], BF16, kind="ExternalInput")
    b1b = nc.dram_tensor("b1b", [P, H1], F32, kind="ExternalInput")
    b2b = nc.dram_tensor("b2b", [P, H2], F32, kind="ExternalInput")
    blb = nc.dram_tensor("blb", [P, C], F32, kind="ExternalInput")
    dinvl = nc.dram_tensor("dinvl", [P, W], F32, kind="ExternalInput")
    idx16 = nc.dram_tensor("idx16", [P, SLOT // 16], I16, kind="ExternalInput")
    dcol = nc.dram_tensor("dcol", [P, TT], BF16, kind="ExternalInput")
    iota = nc.dram_tensor("iota", [P, 4 * P], BF16, kind="ExternalInput")
    ident = nc.dram_tensor("ident", [P, P], BF16, kind="ExternalInput")
    outst = nc.dram_tensor("outst", [P, W * C], F32, kind="ExternalOutput")

    # per-window column ranges in idx16 / dcol
    blk_tiles = np.asarray(T)  # [W, 4]
    w_tile_off = np.zeros(W + 1, np.int64)
    np.cumsum(blk_tiles.sum(axis=1), out=w_tile_off[1:])

    rg = [list(range(NC))]

    # class = src_window % 4; class-major window order so each class's
    # AllGather fires as soon as its quarter of windows is computed
    worder = [w for cl in range(4) for w in range(cl, W, 4)]
    W_cl = [len(range(cl, W, 4)) for cl in range(4)]
    B4cl = [wc * P for wc in W_cl]
    last_w_of_cl = {cl: [w for w in range(cl, W, 4)][-1] for cl in range(4)}
    with tile.TileContext(nc) as tc:
        with tc.tile_pool(name="const", bufs=1) as cpool, \
             tc.tile_pool(name="dram", bufs=1, space="DRAM") as dram:
            ag1_in = [dram.tile([B4cl[i], H1], BF16, name=f"ag1i{i}")
                      for i in range(4)]
            ag1_out = [dram.tile([NC * B4cl[i], H1], BF16, addr_space="Shared",
                                 name=f"ag1o{i}") for i in range(4)]
            ag2_in = [dram.tile([B4cl[i], H2], BF16, name=f"ag2i{i}")
                      for i in range(4)]
            ag2_out = [dram.tile([NC * B4cl[i], H2], BF16, addr_space="Shared",
                                 name=f"ag2o{i}") for i in range(4)]

            w1sb = cpool.tile([P, KI, H1], BF16)
            nc.sync.dma_start(w1sb[:], w1d[:])
            w2sb = cpool.tile([P, K2, H2], BF16)
            nc.sync.dma_start(w2sb[:], w2d[:])
            wlsb = cpool.tile([P, C], BF16)
            nc.sync.dma_start(wlsb[:], wld[:])
            b1sb = cpool.tile([P, H1], F32)
            nc.sync.dma_start(b1sb[:], b1b[:])
            b2sb = cpool.tile([P, H2], F32)
            nc.sync.dma_start(b2sb[:], b2b[:])
            blsb = cpool.tile([P, C], F32)
            nc.sync.dma_start(blsb[:], blb[:])
            dinvsb = cpool.tile([P, W], F32)
            nc.sync.dma_start(dinvsb[:], dinvl[:])
            iotasb = cpool.tile([P, 4, P], BF16)
            nc.sync.dma_start(iotasb[:], iota[:].rearrange("p (a b) -> p a b", a=4))
            idsb = cpool.tile([P, P], BF16)
            nc.sync.dma_start(idsb[:], ident[:])
            dcolsb = cpool.tile([P, TT], BF16)
            nc.sync.dma_start(dcolsb[:], dcol[:])

            lgst = cpool.tile([P, W * C], F32)
            sst = cpool.tile([P, W], F32)
            outsb = cpool.tile([P, W * C], F32)
            # retained h' windows for the self-loop epilogue term
            h1buf = cpool.tile([P, W, H1], BF16)
            h2buf = cpool.tile([P, W, H2], BF16)

            def allgather(cl, ag_in, ag_out):
                if NC == 1:
                    nc.sync.dma_start(ag_out[cl][:], ag_in[cl][:])
                else:
                    nc.gpsimd.collective_compute(
                        "AllGather", mybir.AluOpType.bypass,
                        ins=[ag_in[cl][:]], outs=[ag_out[cl][:]],
                        replica_groups=rg)

            # ---------------- phase A: h1' = dinv * (x @ W1), allgather per
            # class as soon as its windows are done (class-major worder)
            XB = 4  # windows per xt4 load
            with tc.tile_pool(name="phA", bufs=3) as sbA, \
                 tc.tile_pool(name="phA_ps", bufs=2, space="PSUM") as psA:
                for i0 in (range(0, W, XB) if upto != "noop" else []):
                    nb = min(XB, W - i0)
                    xt = sbA.tile([P, XB, KI, P], BF16, tag="xt")
                    nc.sync.dma_start(xt[:, :nb], xt4[:, i0:i0 + nb])
                    for k in range(nb):
                        w = worder[i0 + k]
                        cl = w % 4
                        hp = psA.tile([P, H1], F32, tag="hp")
                        for c2 in range(KI):
                            nc.tensor.matmul(hp[:], xt[:, k, c2], w1sb[:, c2],
                                             start=(c2 == 0),
                                             stop=(c2 == KI - 1))
                        nc.scalar.activation(h1buf[:, w], hp[:],
                                             mybir.ActivationFunctionType.Copy,
                                             scale=dinvsb[:, w:w + 1])
                        widx = w // 4
                        nc.sync.dma_start(
                            ag1_in[cl][widx * P:(widx + 1) * P, :],
                            h1buf[:, w])
                        if w == last_w_of_cl[cl]:
                            allgather(cl, ag1_in, ag1_out)

            qctr = [0]

            def aggregate(w, sb, sbS, ps, cls_tabs, F, tag):
                """Gather + S-matmul segment-sum for window w at width F.
                Returns the PSUM accumulator tile."""
                t0 = int(w_tile_off[w])
                ntile_w = int(w_tile_off[w + 1] - w_tile_off[w])
                idxw = sb.tile([P, ntile_w * 8], I16, tag=f"idxw{tag}", bufs=6)
                nc.sync.dma_start(idxw[:], idx16[:, t0 * 8:(t0 + ntile_w) * 8])
                acc = ps.tile([P, F], F32, tag=f"acc{tag}", bufs=3)
                # batched one-hot builds: 4 S tiles per DVE op
                stiles = []
                for bi in range(0, ntile_w, 4):
                    k = min(4, ntile_w - bi)
                    S4 = sbS.tile([P, 4, P], BF16, tag=f"S{tag}", bufs=10)
                    nc.vector.tensor_tensor(
                        S4[:, :k],
                        dcolsb[:, t0 + bi:t0 + bi + k].to_broadcast([P, k, P]),
                        iotasb[:, :k], op=mybir.AluOpType.is_equal)
                    for j in range(k):
                        stiles.append((S4, j))
                ti = 0
                for cl in range(4):
                    for t_op in ops[w][cl]:
                        g = sb.tile([P, cfg.maxt, F], BF16, tag=f"g{tag}",
                                    bufs=12)
                        nc.gpsimd.dma_gather(
                            g[:, :t_op], cls_tabs[cl][:],
                            idxw[:, ti * 8:(ti + t_op) * 8],
                            t_op * P, t_op * P, F,
                            queue_num=qctr[0] % 4)
                        qctr[0] += 1
                        for t in range(t_op):
                            S4, j = stiles[ti + t]
                            nc.tensor.matmul(acc[:], S4[:, j], g[:, t],
                                             start=(ti + t == 0),
                                             stop=(ti + t == ntile_w - 1))
                        ti += t_op
                return acc

            # ---------------- phase C/D: aggregate layer1, h2' = dinv*(a1@W2)
            if upto.startswith("CD") or upto == "full":
                with tc.tile_pool(name="phC", bufs=4) as sbC, \
                     tc.tile_pool(name="phC_s", bufs=6) as sbS, \
                     tc.tile_pool(name="phC_ps", bufs=2, space="PSUM") as psC, \
                     tc.tile_pool(name="phD_ps", bufs=2, space="PSUM") as psD:
                    for w in worder:
                        acc = aggregate(w, sbC, sbS, psC, ag1_out, H1, "1")
                        # self loop: agg = dinv*(acc + h1') ; then + b, relu
                        zs = sbC.tile([P, H1], F32, tag="zs")
                        nc.vector.tensor_tensor(zs[:], acc[:], h1buf[:, w],
                                                op=mybir.AluOpType.add)
                        z = sbC.tile([P, H1], F32, tag="z")
                        nc.scalar.activation(z[:], zs[:],
                                             mybir.ActivationFunctionType.Copy,
                                             scale=dinvsb[:, w:w + 1])
                        z2 = sbC.tile([P, H1], F32, tag="z2")
                        nc.vector.tensor_tensor(z2[:], z[:], b1sb[:],
                                                op=mybir.AluOpType.add)
                        a1 = sbC.tile([P, H1], BF16, tag="a1")
                        nc.scalar.activation(a1[:], z2[:],
                                             mybir.ActivationFunctionType.Relu)
                        h2p = psD.tile([P, H2], F32, tag="h2p")
                        for c2 in range(K2):
                            a1tp = psD.tile([P, P], BF16, tag="a1tp")
                            nc.tensor.transpose(a1tp[:], a1[:, c2 * P:(c2 + 1) * P],
                                                idsb[:])
                            a1t = sbC.tile([P, P], BF16, tag="a1t")
                            nc.vector.tensor_copy(a1t[:], a1tp[:])
                            nc.tensor.matmul(h2p[:], a1t[:], w2sb[:, c2],
                                             start=(c2 == 0), stop=(c2 == K2 - 1))
                        nc.scalar.activation(h2buf[:, w], h2p[:],
                                             mybir.ActivationFunctionType.Copy,
                                             scale=dinvsb[:, w:w + 1])
                        cl = w % 4
                        widx = w // 4
                        nc.sync.dma_start(
                            ag2_in[cl][widx * P:(widx + 1) * P, :],
                            h2buf[:, w])
                        if w == last_w_of_cl[cl]:
                            allgather(cl, ag2_in, ag2_out)

            # ---------------- phase E/F: aggregate layer2, logits, log_softmax
            if upto == "full":
                with tc.tile_pool(name="phE", bufs=4) as sbE, \
                     tc.tile_pool(name="phE_s", bufs=6) as sbS2, \
                     tc.tile_pool(name="phE_ps", bufs=2, space="PSUM") as psE, \
                     tc.tile_pool(name="phL_ps", bufs=2, space="PSUM") as psL:
                    for w in worder:
                        acc = aggregate(w, sbE, sbS2, psE, ag2_out, H2, "2")
                        zs = sbE.tile([P, H2], F32, tag="zse")
                        nc.vector.tensor_tensor(zs[:], acc[:], h2buf[:, w],
                                                op=mybir.AluOpType.add)
                        z = sbE.tile([P, H2], F32, tag="ze")
                        nc.scalar.activation(z[:], zs[:],
                                             mybir.ActivationFunctionType.Copy,
                                             scale=dinvsb[:, w:w + 1])
                        z2 = sbE.tile([P, H2], F32, tag="z2e")
                        nc.vector.tensor_tensor(z2[:], z[:], b2sb[:],
                                                op=mybir.AluOpType.add)
                        a2 = sbE.tile([P, H2], BF16, tag="a2")
                        nc.scalar.activation(a2[:], z2[:],
                                             mybir.ActivationFunctionType.Relu)
                        a2tp = psL.tile([P, P], BF16, tag="a2tp")
                        nc.tensor.transpose(a2tp[:], a2[:], idsb[:])
                        a2t = sbE.tile([P, P], BF16, tag="a2t")
                        nc.vector.tensor_copy(a2t[:], a2tp[:])
                        lg = psL.tile([P, C], F32, tag="lg")
                        nc.tensor.matmul(lg[:], a2t[:], wlsb[:], start=True, stop=True)
                        nc.vector.tensor_tensor(lgst[:, w * C:(w + 1) * C], lg[:],
                                                blsb[:], op=mybir.AluOpType.add)
                        e = sbE.tile([P, C], F32, tag="e")
                        nc.scalar.activation(e[:], lgst[:, w * C:(w + 1) * C],
                                             mybir.ActivationFunctionType.Exp,
                                             accum_out=sst[:, w:w + 1])
                    lns = cpool.tile([P, W], F32)
                    nc.scalar.activation(lns[:], sst[:],
                                         mybir.ActivationFunctionType.Ln)
                    for w in range(W):
                        nc.vector.tensor_scalar(
                            outsb[:, w * C:(w + 1) * C], lgst[:, w * C:(w + 1) * C],
                            lns[:, w:w + 1], None, op0=mybir.AluOpType.subtract)
                    nc.sync.dma_start(outst[:], outsb[:])
            else:
                # debug variants: dummy output proving the kept phases ran
                nc.vector.memset(outsb[:], 0.0)
                if upto != "noop":
                    probe_src = ag1_out[0] if upto == "A" else ag2_out[0]
                    probe = cpool.tile([P, C], BF16)
                    nc.sync.dma_start(probe[:], probe_src[:P, :C])
                    nc.vector.tensor_copy(outsb[:, :C], probe[:])
                nc.sync.dma_start(outst[:], outsb[:])

    nc.compile()
    return nc

# ---------------------------------------------------------------- entry point
_CACHE = {}


def _get_compiled(cfg, key, T, ops):
    if key not in _CACHE:
        nc = build_kernel(cfg, T, ops)
        nc.m = get_hw_module(nc.m)
        _CACHE[key] = nc
    return _CACHE[key]


def run(cfg, inputs):
    in_maps, meta = preprocess(cfg, **inputs)
    key = (cfg.N, cfg.F_IN, meta["TT"])
    nc = _get_compiled(cfg, key, meta["T"], meta["ops"])
    res = bass_utils.run_bass_kernel_spmd(
        nc, in_maps, core_ids=list(range(cfg.ncores)))
    out = assemble_output(cfg, meta, res.results)
    return out, res


class _TimedRunner:
    """PJRT runner mirroring bass2jax.run_bass_via_pjrt's multi-core branch,
    but with a cached jit and device-resident inputs for repeatable timing."""

    def __init__(self, nc, n_cores):
        import jax
        import concourse.mybir as mb
        from concourse import bass2jax
        from jax.sharding import Mesh, PartitionSpec, NamedSharding
        from jax.experimental.shard_map import shard_map

        bass2jax.install_neuronx_cc_hook()
        partition_name = (nc.partition_id_tensor.name
                          if nc.partition_id_tensor else None)
        in_names, out_names, out_avals, zero_shapes = [], [], [], []
        for alloc in nc.m.functions[0].allocations:
            if not isinstance(alloc, mb.MemoryLocationSet):
                continue
            name = alloc.memorylocations[0].name
            if alloc.kind == "ExternalInput":
                if name != partition_name:
                    in_names.append(name)
            elif alloc.kind == "ExternalOutput":
                out_names.append(name)
                shape = tuple(alloc.tensor_shape)
                dtype = mb.dt.np(alloc.dtype)
                out_avals.append(jax.core.ShapedArray(shape, dtype))
                zero_shapes.append((shape, dtype))
        n_params = len(in_names)
        all_in_names = list(in_names) + list(out_names)
        if partition_name is not None:
            all_in_names.append(partition_name)
        donate = tuple(range(n_params, n_params + len(out_names)))

        def _body(*args):
            operands = list(args)
            if partition_name is not None:
                operands.append(bass2jax.partition_id_tensor())
            outs = bass2jax._bass_exec_p.bind(
                *operands,
                out_avals=tuple(out_avals),
                in_names=tuple(all_in_names),
                out_names=tuple(out_names),
                lowering_input_output_aliases=(),
                sim_require_finite=True,
                sim_require_nnan=True,
                nc=nc,
            )
            return tuple(outs)

        devices = jax.devices()[:n_cores]
        mesh = Mesh(np.asarray(devices), ("core",))
        in_specs = (PartitionSpec("core"),) * (n_params + len(out_names))
        out_specs = (PartitionSpec("core"),) * len(out_names)
        self.fn = jax.jit(
            shard_map(_body, mesh=mesh, in_specs=in_specs,
                      out_specs=out_specs, check_rep=False),
            donate_argnums=donate, keep_unused=True)
        self.jax = jax
        self.mesh = mesh
        self.sharding = NamedSharding(mesh, PartitionSpec("core"))
        self.in_names = in_names
        self.out_names = out_names
        self.zero_shapes = zero_shapes
        self.n_cores = n_cores
        self.dev_inputs = None

    def stage_inputs(self, in_maps):
        concat_in = [
            np.concatenate([np.asarray(in_maps[c][n])
                            for c in range(self.n_cores)], axis=0)
            for n in self.in_names
        ]
        self.dev_inputs = [self.jax.device_put(a, self.sharding)
                           for a in concat_in]
        for a in self.dev_inputs:
            a.block_until_ready()

    def exec_once(self):
        import time
        zeros = [np.zeros((self.n_cores * s[0], *s[1:]), d)
                 for s, d in self.zero_shapes]
        dz = [self.jax.device_put(z, self.sharding) for z in zeros]
        for z in dz:
            z.block_until_ready()
        t0 = time.perf_counter()
        outs = self.fn(*self.dev_inputs, *dz)
        for o in outs:
            o.block_until_ready()
        t1 = time.perf_counter()
        return outs, t1 - t0

    def results(self, outs):
        res = []
        for c in range(self.n_cores):
            m = {}
            for i, n in enumerate(self.out_names):
                full = np.asarray(outs[i])
                per = full.reshape(self.n_cores, -1, *full.shape[1:])[c]
                m[n] = per
            res.append(m)
        return res


def run_timed(cfg, inputs, iters=3):
    in_maps, meta = preprocess(cfg, **inputs)
    key = (cfg.N, cfg.F_IN, meta["TT"])
    nc = _get_compiled(cfg, key, meta["T"], meta["ops"])
    rkey = ("runner",) + key
    if rkey not in _CACHE:
        _CACHE[rkey] = _TimedRunner(nc, cfg.ncores)
    runner = _CACHE[rkey]
    runner.stage_inputs(in_maps)
    times = []
    outs = None
    for _ in range(iters):
        outs, dt = runner.exec_once()
        times.append(dt)
    results = runner.results(outs)
    out = assemble_output(cfg, meta, results)
    return out, times


def kernel(x, edge_index, W1, b1, W2, b2, Wl, bl):
    out, _ = run(FULL, dict(x=x, edge_index=edge_index, W1=W1, b1=b1,
                            W2=W2, b2=b2, Wl=Wl, bl=bl))
    return out



# revision 30
# speedup vs baseline: 26.2819x; 1.0046x over previous
"""Distributed 2-layer GCN (PyG GCNConv-style) on 8 Trainium2 NeuronCores.

Strategy (hardcoded for N=100000, E=3.2M, 512->256->128->4):
  - Nodes are degree-balanced into (ncores*W) windows of 128 nodes; window b is
    owned by core (b % ncores). A node's "global position" is its row in the
    AllGathered feature table, so gathers use plain int positions.
  - Per layer: local dense matmul (bf16 on PE, fp32 PSUM), rows pre-scaled by
    dinv, results AllGathered to a replicated bf16 feature table in DRAM.
  - Per layer the replicated feature table is split into 4 classes by
    src_window % 4 so table rows fit int16 gather indices AND each class's
    AllGather fires as soon as its quarter of windows is computed
    (class-major window order overlaps collectives with compute).
  - Aggregation: per (window, class) block of dst-sorted edges, a dma_gather
    (SWDGE, int16 indices) pulls source rows into SBUF; a one-hot S matrix
    built on DVE (is_equal vs iota, 4 tiles per batched op, bf16) feeds a
    PE matmul S^T @ msgs that segment-sums into the window's PSUM
    accumulator. Gathers stripe across 4 SWDGE queues (9.2 -> 4ns per
    descriptor on the Q7) and ops are split >=4 tiles (>=512 idx) to dodge
    the per-op fixed cost. Padding slots carry dcol=128 which never
    matches iota -> contribute 0.
  - Self loops are folded into the epilogue: agg = dinv*(acc + h'own) + b
    with h' windows retained in SBUF (saves ~100K gather descriptors).
  - Epilogue: relu; layer 2 repeats; final logits + log softmax.
  Profiled HW exec time on 8 cores: ~3.03 ms (baseline session start: 8.8 ms).
"""
import math
import numpy as np

import concourse.bass as bass
import concourse.mybir as mybir
import concourse.bass_utils as bass_utils
from concourse import bacc, tile
from concourse.bass_interp import get_hw_module

P = 128
F32 = mybir.dt.float32
BF16 = mybir.dt.bfloat16
I16 = mybir.dt.int16


class Cfg:
    def __init__(self, N, F_IN, H1, H2, C, ncores=8, W=None, maxt=8):
        self.N, self.F_IN, self.H1, self.H2, self.C = N, F_IN, H1, H2, C
        self.ncores = ncores
        B = ncores * P
        self.W = W if W is not None else math.ceil(N / B)
        self.NPAD = self.W * B
        assert self.NPAD >= N and self.NPAD % 4 == 0
        self.CLS = self.NPAD // 4
        assert self.CLS <= 32768
        self.KI = F_IN // P
        self.K2 = H1 // P
        self.maxt = maxt
        self.B = self.W * P  # nodes per core


FULL = Cfg(N=100000, F_IN=512, H1=256, H2=128, C=4)


# ---------------------------------------------------------------- host side
def preprocess(cfg, x, edge_index, W1, b1, W2, b2, Wl, bl):
    N, NC, W, NPAD, B = cfg.N, cfg.ncores, cfg.W, cfg.NPAD, cfg.B
    NW = NC * W

    src = np.asarray(edge_index[0], dtype=np.int64)
    dst = np.asarray(edge_index[1], dtype=np.int64)
    deg = np.bincount(dst, minlength=N).astype(np.float64) + 1.0
    dinv = (1.0 / np.sqrt(deg)).astype(np.float32)

    # node -> global position, degree-balanced across windows (snake fill)
    degall = np.zeros(NPAD, np.int64)
    degall[:N] = deg.astype(np.int64)
    order = np.argsort(-degall, kind="stable")
    i = np.arange(NPAD)
    phase = i % (2 * NW)
    binid = np.where(phase < NW, phase, 2 * NW - 1 - phase)
    by_bin = np.argsort(binid, kind="stable")
    slot = np.empty(NPAD, np.int64)
    slot[by_bin] = i % P  # within each bin, slots fill 0..127 in arrival order
    # position: core = bin % NC, window = bin // NC
    core_of_bin = binid % NC
    w_of_bin = binid // NC
    g_of_i = core_of_bin * B + w_of_bin * P + slot
    pos = np.empty(NPAD, np.int64)
    pos[order] = g_of_i

    node_at = np.empty(NPAD, np.int64)
    node_at[pos] = np.arange(NPAD)

    # edge list; self loops are folded into the epilogue on-device
    S_pos = pos[src]
    D_pos = pos[dst]
    core_e = D_pos // B
    w_e = (D_pos % B) // P
    dcol_e = (D_pos % P).astype(np.float32)
    # class tables: class = src_window % 4 (so each class's AllGather can
    # fire as soon as its quarter of windows is computed). Row in class
    # table = rank*B4cl + (w//4)*128 + slot.
    W_cl = [len(range(cl, W, 4)) for cl in range(4)]
    B4cl = [wc * P for wc in W_cl]
    s_slot = S_pos % P
    w_src = (S_pos % B) // P
    cls_e = (w_src % 4).astype(np.int64)
    b4_of = np.asarray(B4cl, np.int64)[cls_e]
    idx16_e = ((S_pos // B) * b4_of + (w_src // 4) * P
               + s_slot).astype(np.int16)

    key = ((core_e * W + w_e) * 4 + cls_e).astype(np.int64)
    ordE = np.argsort(key, kind="stable")
    counts = np.bincount(key, minlength=NC * W * 4).reshape(NC, W, 4)
    T = np.maximum(1, np.ceil(counts / P).astype(np.int64).max(axis=0))  # [W, 4]
    TT = int(T.sum())
    SLOT = TT * P

    # ops schedule per (w, cl): list of tile counts. Ops of >=4 tiles
    # (>=512 idx) pay no SWDGE fixed overhead; split e.g. 9 -> [5, 4].
    ops = [[[] for _ in range(4)] for _ in range(W)]
    for w in range(W):
        for cl in range(4):
            t = int(T[w][cl])
            while t > 0:
                c = min(t, cfg.maxt)
                if t > cfg.maxt and t - cfg.maxt < 4:
                    c = t - 4
                ops[w][cl].append(c)
                t -= c

    starts = np.zeros(NC * W * 4 + 1, np.int64)
    np.cumsum(counts.reshape(-1), out=starts[1:])
    blk_off = np.zeros((W, 4), np.int64)  # slot offset of each (w, cl) block
    acc_off = 0
    for w in range(W):
        for cl in range(4):
            blk_off[w, cl] = acc_off
            acc_off += int(T[w][cl]) * P

    idx16 = np.zeros((NC, SLOT), np.int16)
    dcol = np.full((NC, SLOT), float(P), np.float32)  # cast bf16 at ship time
    for c in range(NC):
        for w in range(W):
            for cl in range(4):
                k = (c * W + w) * 4 + cl
                s0, s1 = starts[k], starts[k + 1]
                n = s1 - s0
                off = blk_off[w, cl]
                seg = ordE[s0:s1]
                idx16[c, off:off + n] = idx16_e[seg]
                dcol[c, off:off + n] = dcol_e[seg]

    # wrap idx16 per-op: element i of an op at [i%16, i//16], replicated x8
    idx_w = np.zeros((NC, 16, SLOT // 16), np.int16)
    for w in range(W):
        for cl in range(4):
            off = int(blk_off[w, cl])
            for t_op in ops[w][cl]:
                n = t_op * P
                blk = idx16[:, off:off + n].reshape(NC, n // 16, 16)
                idx_w[:, :, off // 16:(off + n) // 16] = blk.transpose(0, 2, 1)
                off += n
    idx_rep = np.tile(idx_w, (1, 8, 1))  # [NC, 128, SLOT//16]

    dcol_t = dcol.reshape(NC, TT, P).transpose(0, 2, 1).copy()  # [NC, 128, TT]

    # x shard, transposed chunk layout: xt4[p, i, c2, m] = x[node(w*128+m), c2*128+p]
    # with i indexing windows in class-major (worder) order for batched loads.
    worder = [w for cl in range(4) for w in range(cl, W, 4)]
    xpad = np.zeros((NPAD, cfg.F_IN), np.float32)
    xpad[:N] = np.asarray(x, np.float32)
    dinvpad = np.ones(NPAD, np.float32)
    dinvpad[:N] = dinv

    xt4 = np.empty((NC, P, W, cfg.KI, P), np.float32)
    dinvl = np.empty((NC, P, W), np.float32)
    for c in range(NC):
        ids = node_at[c * B:(c + 1) * B]
        xl = xpad[ids]  # [B, F_IN]
        xt4[c] = xl.reshape(W, P, cfg.KI, P).transpose(3, 0, 2, 1)[:, worder]
        dinvl[c] = dinvpad[ids].reshape(W, P).T

    w1d = np.asarray(W1, np.float32).reshape(cfg.KI, P, cfg.H1).transpose(1, 0, 2)
    w2d = np.asarray(W2, np.float32).reshape(cfg.K2, P, cfg.H2).transpose(1, 0, 2)
    wld = np.asarray(Wl, np.float32)  # [H2=128, C]
    b1b = np.broadcast_to(np.asarray(b1, np.float32), (P, cfg.H1)).copy()
    b2b = np.broadcast_to(np.asarray(b2, np.float32), (P, cfg.H2)).copy()
    blb = np.broadcast_to(np.asarray(bl, np.float32), (P, cfg.C)).copy()
    iota4 = np.broadcast_to(np.arange(P, dtype=np.float32),
                            (P, 4, P)).reshape(P, 4 * P).copy()
    ident = np.eye(P, dtype=np.float32)

    import ml_dtypes
    bf = lambda a: a.astype(ml_dtypes.bfloat16)

    in_maps = []
    for c in range(NC):
        in_maps.append({
            "xt4": bf(xt4[c]),
            "w1d": bf(w1d), "w2d": bf(w2d), "wld": bf(wld),
            "b1b": b1b, "b2b": b2b, "blb": blb,
            "dinvl": dinvl[c],
            "idx16": idx_rep[c],
            "dcol": bf(dcol_t[c]),
            "iota": bf(iota4),
            "ident": bf(ident),
        })

    meta = dict(T=T, ops=ops, pos=pos, node_at=node_at, SLOT=SLOT, TT=TT)
    return in_maps, meta


def assemble_output(cfg, meta, results):
    N, NC, W, C, B = cfg.N, cfg.ncores, cfg.W, cfg.C, cfg.B
    rows = []
    for c in range(NC):
        r = results[c]["outst"].reshape(P, W, C).transpose(1, 0, 2).reshape(B, C)
        rows.append(r)
    allrows = np.concatenate(rows, axis=0)  # [NPAD, C] in position order
    return allrows[meta["pos"][:N]].astype(np.float32)


# ---------------------------------------------------------------- device side
def build_kernel(cfg, T, ops, upto="full"):
    NC, W, NPAD, B = cfg.ncores, cfg.W, cfg.NPAD, cfg.B
    H1, H2, C, KI, K2 = cfg.H1, cfg.H2, cfg.C, cfg.KI, cfg.K2
    TT = int(np.asarray(T).sum())
    SLOT = TT * P

    nc = bacc.Bacc("TRN2", target_bir_lowering=False, debug=False,
                   num_devices=NC, num_swdge_queues=4)

    xt4 = nc.dram_tensor("xt4", [P, W, KI, P], BF16, kind="ExternalInput")
    w1d = nc.dram_tensor("w1d", [P, KI, H1], BF16, kind="ExternalInput")
    w2d = nc.dram_tensor("w2d", [P, K2, H2], BF16, kind="ExternalInput")
    wld = nc.dram_tensor("wld", [P, C], BF16, kind="ExternalInput")
    b1b = nc.dram_tensor("b1b", [P, H1], F32, kind="ExternalInput")
    b2b = nc.dram_tensor("b2b", [P, H2], F32, kind="ExternalInput")
    blb = nc.dram_tensor("blb", [P, C], F32, kind="ExternalInput")
    dinvl = nc.dram_tensor("dinvl", [P, W], F32, kind="ExternalInput")
    idx16 = nc.dram_tensor("idx16", [P, SLOT // 16], I16, kind="ExternalInput")
    dcol = nc.dram_tensor("dcol", [P, TT], BF16, kind="ExternalInput")
    iota = nc.dram_tensor("iota", [P, 4 * P], BF16, kind="ExternalInput")
    ident = nc.dram_tensor("ident", [P, P], BF16, kind="ExternalInput")
    outst = nc.dram_tensor("outst", [P, W * C], F32, kind="ExternalOutput")

    # per-window column ranges in idx16 / dcol
    blk_tiles = np.asarray(T)  # [W, 4]
    w_tile_off = np.zeros(W + 1, np.int64)
    np.cumsum(blk_tiles.sum(axis=1), out=w_tile_off[1:])

    rg = [list(range(NC))]

    # class = src_window % 4; class-major window order so each class's
    # AllGather fires as soon as its quarter of windows is computed
    worder = [w for cl in range(4) for w in range(cl, W, 4)]
    W_cl = [len(range(cl, W, 4)) for cl in range(4)]
    B4cl = [wc * P for wc in W_cl]
    last_w_of_cl = {cl: [w for w in range(cl, W, 4)][-1] for cl in range(4)}
    with tile.TileContext(nc) as tc:
        with tc.tile_pool(name="const", bufs=1) as cpool, \
             tc.tile_pool(name="dram", bufs=1, space="DRAM") as dram:
            ag1_in = [dram.tile([B4cl[i], H1], BF16, name=f"ag1i{i}")
                      for i in range(4)]
            ag1_out = [dram.tile([NC * B4cl[i], H1], BF16, addr_space="Shared",
                                 name=f"ag1o{i}") for i in range(4)]
            ag2_in = [dram.tile([B4cl[i], H2], BF16, name=f"ag2i{i}")
                      for i in range(4)]
            ag2_out = [dram.tile([NC * B4cl[i], H2], BF16, addr_space="Shared",
                                 name=f"ag2o{i}") for i in range(4)]

            w1sb = cpool.tile([P, KI, H1], BF16)
            nc.sync.dma_start(w1sb[:], w1d[:])
            w2sb = cpool.tile([P, K2, H2], BF16)
            nc.sync.dma_start(w2sb[:], w2d[:])
            wlsb = cpool.tile([P, C], BF16)
            nc.sync.dma_start(wlsb[:], wld[:])
            b1sb = cpool.tile([P, H1], F32)
            nc.sync.dma_start(b1sb[:], b1b[:])
            b2sb = cpool.tile([P, H2], F32)
            nc.sync.dma_start(b2sb[:], b2b[:])
            blsb = cpool.tile([P, C], F32)
            nc.sync.dma_start(blsb[:], blb[:])
            dinvsb = cpool.tile([P, W], F32)
            nc.sync.dma_start(dinvsb[:], dinvl[:])
            iotasb = cpool.tile([P, 4, P], BF16)
            nc.sync.dma_start(iotasb[:], iota[:].rearrange("p (a b) -> p a b", a=4))
            idsb = cpool.tile([P, P], BF16)
            nc.sync.dma_start(idsb[:], ident[:])
            dcolsb = cpool.tile([P, TT], BF16)
            nc.sync.dma_start(dcolsb[:], dcol[:])

            lgst = cpool.tile([P, W * C], F32)
            sst = cpool.tile([P, W], F32)
            outsb = cpool.tile([P, W * C], F32)
            # retained h' windows for the self-loop epilogue term
            h1buf = cpool.tile([P, W, H1], BF16)
            h2buf = cpool.tile([P, W, H2], BF16)

            def allgather(cl, ag_in, ag_out):
                if NC == 1:
                    nc.sync.dma_start(ag_out[cl][:], ag_in[cl][:])
                else:
                    nc.gpsimd.collective_compute(
                        "AllGather", mybir.AluOpType.bypass,
                        ins=[ag_in[cl][:]], outs=[ag_out[cl][:]],
                        replica_groups=rg)

            # ---------------- phase A: h1' = dinv * (x @ W1), allgather per
            # class as soon as its windows are done (class-major worder)
            XB = 8  # windows per xt4 load
            with tc.tile_pool(name="phA", bufs=3) as sbA, \
                 tc.tile_pool(name="phA_ps", bufs=3, space="PSUM") as psA:
                for i0 in (range(0, W, XB) if upto != "noop" else []):
                    nb = min(XB, W - i0)
                    xt = sbA.tile([P, XB, KI, P], BF16, tag="xt")
                    nc.sync.dma_start(xt[:, :nb], xt4[:, i0:i0 + nb])
                    for k in range(nb):
                        w = worder[i0 + k]
                        cl = w % 4
                        hp = psA.tile([P, H1], F32, tag="hp")
                        for c2 in range(KI):
                            nc.tensor.matmul(hp[:], xt[:, k, c2], w1sb[:, c2],
                                             start=(c2 == 0),
                                             stop=(c2 == KI - 1))
                        # staging ring decouples the act->DMA chain from the
                        # persistent h1buf (tile-granular deps would
                        # serialize phase A on it)
                        h1p = sbA.tile([P, H1], BF16, tag="h1p", bufs=4)
                        nc.scalar.activation(h1p[:], hp[:],
                                             mybir.ActivationFunctionType.Copy,
                                             scale=dinvsb[:, w:w + 1])
                        nc.vector.tensor_copy(h1buf[:, w], h1p[:])
                        widx = w // 4
                        nc.sync.dma_start(
                            ag1_in[cl][widx * P:(widx + 1) * P, :],
                            h1p[:])
                        if w == last_w_of_cl[cl]:
                            allgather(cl, ag1_in, ag1_out)

            qctr = [0]
            # pre-set num_idxs registers once (else every gather emits a
            # GpSimd MOVE on the bottleneck engine)
            nreg = {}
            for w in range(W):
                for cl in range(4):
                    for t_op in ops[w][cl]:
                        if t_op not in nreg:
                            nreg[t_op] = nc.gpsimd.to_reg(t_op * P)

            def aggregate(w, sb, sbS, ps, cls_tabs, F, tag):
                """Gather + S-matmul segment-sum for window w at width F.
                Returns the PSUM accumulator tile."""
                t0 = int(w_tile_off[w])
                ntile_w = int(w_tile_off[w + 1] - w_tile_off[w])
                idxw = sb.tile([P, ntile_w * 8], I16, tag=f"idxw{tag}", bufs=6)
                nc.sync.dma_start(idxw[:], idx16[:, t0 * 8:(t0 + ntile_w) * 8])
                acc = ps.tile([P, F], F32, tag=f"acc{tag}", bufs=3)
                # batched one-hot builds: 4 S tiles per DVE op
                stiles = []
                for bi in range(0, ntile_w, 4):
                    k = min(4, ntile_w - bi)
                    S4 = sbS.tile([P, 4, P], BF16, tag=f"S{tag}", bufs=10)
                    nc.vector.tensor_tensor(
                        S4[:, :k],
                        dcolsb[:, t0 + bi:t0 + bi + k].to_broadcast([P, k, P]),
                        iotasb[:, :k], op=mybir.AluOpType.is_equal)
                    for j in range(k):
                        stiles.append((S4, j))
                ti = 0
                for cl in range(4):
                    for t_op in ops[w][cl]:
                        g = sb.tile([P, cfg.maxt, F], BF16, tag=f"g{tag}",
                                    bufs=12)
                        nc.gpsimd.dma_gather(
                            g[:, :t_op], cls_tabs[cl][:],
                            idxw[:, ti * 8:(ti + t_op) * 8],
                            t_op * P, nreg[t_op], F,
                            queue_num=qctr[0] % 4)
                        qctr[0] += 1
                        for t in range(t_op):
                            S4, j = stiles[ti + t]
                            nc.tensor.matmul(acc[:], S4[:, j], g[:, t],
                                             start=(ti + t == 0),
                                             stop=(ti + t == ntile_w - 1))
                        ti += t_op
                return acc

            # ---------------- phase C/D: aggregate layer1, h2' = dinv*(a1@W2)
            if upto.startswith("CD") or upto == "full":
                with tc.tile_pool(name="phC", bufs=4) as sbC, \
                     tc.tile_pool(name="phC_s", bufs=6) as sbS, \
                     tc.tile_pool(name="phC_ps", bufs=2, space="PSUM") as psC, \
                     tc.tile_pool(name="phD_ps", bufs=2, space="PSUM") as psD:
                    for w in worder:
                        acc = aggregate(w, sbC, sbS, psC, ag1_out, H1, "1")
                        # self loop: agg = dinv*(acc + h1') ; then + b, relu
                        zs = sbC.tile([P, H1], F32, tag="zs")
                        nc.vector.tensor_tensor(zs[:], acc[:], h1buf[:, w],
                                                op=mybir.AluOpType.add)
                        z = sbC.tile([P, H1], F32, tag="z")
                        nc.scalar.activation(z[:], zs[:],
                                             mybir.ActivationFunctionType.Copy,
                                             scale=dinvsb[:, w:w + 1])
                        z2 = sbC.tile([P, H1], F32, tag="z2")
                        nc.vector.tensor_tensor(z2[:], z[:], b1sb[:],
                                                op=mybir.AluOpType.add)
                        a1 = sbC.tile([P, H1], BF16, tag="a1")
                        nc.scalar.activation(a1[:], z2[:],
                                             mybir.ActivationFunctionType.Relu)
                        h2p = psD.tile([P, H2], F32, tag="h2p")
                        for c2 in range(K2):
                            a1tp = psD.tile([P, P], BF16, tag="a1tp")
                            nc.tensor.transpose(a1tp[:], a1[:, c2 * P:(c2 + 1) * P],
                                                idsb[:])
                            a1t = sbC.tile([P, P], BF16, tag="a1t")
                            nc.vector.tensor_copy(a1t[:], a1tp[:])
                            nc.tensor.matmul(h2p[:], a1t[:], w2sb[:, c2],
                                             start=(c2 == 0), stop=(c2 == K2 - 1))
                        nc.scalar.activation(h2buf[:, w], h2p[:],
                                             mybir.ActivationFunctionType.Copy,
                                             scale=dinvsb[:, w:w + 1])
                        cl = w % 4
                        widx = w // 4
                        nc.sync.dma_start(
                            ag2_in[cl][widx * P:(widx + 1) * P, :],
                            h2buf[:, w])
                        if w == last_w_of_cl[cl]:
                            allgather(cl, ag2_in, ag2_out)

            # ---------------- phase E/F: aggregate layer2, logits, log_softmax
            if upto == "full":
                with tc.tile_pool(name="phE", bufs=4) as sbE, \
                     tc.tile_pool(name="phE_s", bufs=6) as sbS2, \
                     tc.tile_pool(name="phE_ps", bufs=2, space="PSUM") as psE, \
                     tc.tile_pool(name="phL_ps", bufs=2, space="PSUM") as psL:
                    for w in worder:
                        acc = aggregate(w, sbE, sbS2, psE, ag2_out, H2, "2")
                        zs = sbE.tile([P, H2], F32, tag="zse")
                        nc.vector.tensor_tensor(zs[:], acc[:], h2buf[:, w],
                                                op=mybir.AluOpType.add)
                        z = sbE.tile([P, H2], F32, tag="ze")
                        nc.scalar.activation(z[:], zs[:],
                                             mybir.ActivationFunctionType.Copy,
                                             scale=dinvsb[:, w:w + 1])
                        z2 = sbE.tile([P, H2], F32, tag="z2e")
                        nc.vector.tensor_tensor(z2[:], z[:], b2sb[:],
                                                op=mybir.AluOpType.add)
                        a2 = sbE.tile([P, H2], BF16, tag="a2")
                        nc.scalar.activation(a2[:], z2[:],
                                             mybir.ActivationFunctionType.Relu)
                        a2tp = psL.tile([P, P], BF16, tag="a2tp")
                        nc.tensor.transpose(a2tp[:], a2[:], idsb[:])
                        a2t = sbE.tile([P, P], BF16, tag="a2t")
                        nc.vector.tensor_copy(a2t[:], a2tp[:])
                        lg = psL.tile([P, C], F32, tag="lg")
                        nc.tensor.matmul(lg[:], a2t[:], wlsb[:], start=True, stop=True)
                        nc.vector.tensor_tensor(lgst[:, w * C:(w + 1) * C], lg[:],
                                                blsb[:], op=mybir.AluOpType.add)
                        e = sbE.tile([P, C], F32, tag="e")
                        nc.scalar.activation(e[:], lgst[:, w * C:(w + 1) * C],
                                             mybir.ActivationFunctionType.Exp,
                                             accum_out=sst[:, w:w + 1])
                    lns = cpool.tile([P, W], F32)
                    nc.scalar.activation(lns[:], sst[:],
                                         mybir.ActivationFunctionType.Ln)
                    for w in range(W):
                        nc.vector.tensor_scalar(
                            outsb[:, w * C:(w + 1) * C], lgst[:, w * C:(w + 1) * C],
                            lns[:, w:w + 1], None, op0=mybir.AluOpType.subtract)
                    nc.sync.dma_start(outst[:], outsb[:])
            else:
                # debug variants: dummy output proving the kept phases ran
                nc.vector.memset(outsb[:], 0.0)
                if upto != "noop":
                    probe_src = ag1_out[0] if upto == "A" else ag2_out[0]
                    probe = cpool.tile([P, C], BF16)
                    nc.sync.dma_start(probe[:], probe_src[:P, :C])
                    nc.vector.tensor_copy(outsb[:, :C], probe[:])
                nc.sync.dma_start(outst[:], outsb[:])

    nc.compile()
    return nc

# ---------------------------------------------------------------- entry point
_CACHE = {}


def _get_compiled(cfg, key, T, ops):
    if key not in _CACHE:
        nc = build_kernel(cfg, T, ops)
        nc.m = get_hw_module(nc.m)
        _CACHE[key] = nc
    return _CACHE[key]


def run(cfg, inputs):
    in_maps, meta = preprocess(cfg, **inputs)
    key = (cfg.N, cfg.F_IN, meta["TT"])
    nc = _get_compiled(cfg, key, meta["T"], meta["ops"])
    res = bass_utils.run_bass_kernel_spmd(
        nc, in_maps, core_ids=list(range(cfg.ncores)))
    out = assemble_output(cfg, meta, res.results)
    return out, res


class _TimedRunner:
    """PJRT runner mirroring bass2jax.run_bass_via_pjrt's multi-core branch,
    but with a cached jit and device-resident inputs for repeatable timing."""

    def __init__(self, nc, n_cores):
        import jax
        import concourse.mybir as mb
        from concourse import bass2jax
        from jax.sharding import Mesh, PartitionSpec, NamedSharding
        from jax.experimental.shard_map import shard_map

        bass2jax.install_neuronx_cc_hook()
        partition_name = (nc.partition_id_tensor.name
                          if nc.partition_id_tensor else None)
        in_names, out_names, out_avals, zero_shapes = [], [], [], []
        for alloc in nc.m.functions[0].allocations:
            if not isinstance(alloc, mb.MemoryLocationSet):
                continue
            name = alloc.memorylocations[0].name
            if alloc.kind == "ExternalInput":
                if name != partition_name:
                    in_names.append(name)
            elif alloc.kind == "ExternalOutput":
                out_names.append(name)
                shape = tuple(alloc.tensor_shape)
                dtype = mb.dt.np(alloc.dtype)
                out_avals.append(jax.core.ShapedArray(shape, dtype))
                zero_shapes.append((shape, dtype))
        n_params = len(in_names)
        all_in_names = list(in_names) + list(out_names)
        if partition_name is not None:
            all_in_names.append(partition_name)
        donate = tuple(range(n_params, n_params + len(out_names)))

        def _body(*args):
            operands = list(args)
            if partition_name is not None:
                operands.append(bass2jax.partition_id_tensor())
            outs = bass2jax._bass_exec_p.bind(
                *operands,
                out_avals=tuple(out_avals),
                in_names=tuple(all_in_names),
                out_names=tuple(out_names),
                lowering_input_output_aliases=(),
                sim_require_finite=True,
                sim_require_nnan=True,
                nc=nc,
            )
            return tuple(outs)

        devices = jax.devices()[:n_cores]
        mesh = Mesh(np.asarray(devices), ("core",))
        in_specs = (PartitionSpec("core"),) * (n_params + len(out_names))
        out_specs = (PartitionSpec("core"),) * len(out_names)
        self.fn = jax.jit(
            shard_map(_body, mesh=mesh, in_specs=in_specs,
                      out_specs=out_specs, check_rep=False),
            donate_argnums=donate, keep_unused=True)
        self.jax = jax
        self.mesh = mesh
        self.sharding = NamedSharding(mesh, PartitionSpec("core"))
        self.in_names = in_names
        self.out_names = out_names
        self.zero_shapes = zero_shapes
        self.n_cores = n_cores
        self.dev_inputs = None

    def stage_inputs(self, in_maps):
        concat_in = [
            np.concatenate([np.asarray(in_maps[c][n])
                            for c in range(self.n_cores)], axis=0)
            for n in self.in_names
        ]
        self.dev_inputs = [self.jax.device_put(a, self.sharding)
                           for a in concat_in]
        for a in self.dev_inputs:
            a.block_until_ready()

    def exec_once(self):
        import time
        zeros = [np.zeros((self.n_cores * s[0], *s[1:]), d)
                 for s, d in self.zero_shapes]
        dz = [self.jax.device_put(z, self.sharding) for z in zeros]
        for z in dz:
            z.block_until_ready()
        t0 = time.perf_counter()
        outs = self.fn(*self.dev_inputs, *dz)
        for o in outs:
            o.block_until_ready()
        t1 = time.perf_counter()
        return outs, t1 - t0

    def results(self, outs):
        res = []
        for c in range(self.n_cores):
            m = {}
            for i, n in enumerate(self.out_names):
                full = np.asarray(outs[i])
                per = full.reshape(self.n_cores, -1, *full.shape[1:])[c]
                m[n] = per
            res.append(m)
        return res


def run_timed(cfg, inputs, iters=3):
    in_maps, meta = preprocess(cfg, **inputs)
    key = (cfg.N, cfg.F_IN, meta["TT"])
    nc = _get_compiled(cfg, key, meta["T"], meta["ops"])
    rkey = ("runner",) + key
    if rkey not in _CACHE:
        _CACHE[rkey] = _TimedRunner(nc, cfg.ncores)
    runner = _CACHE[rkey]
    runner.stage_inputs(in_maps)
    times = []
    outs = None
    for _ in range(iters):
        outs, dt = runner.exec_once()
        times.append(dt)
    results = runner.results(outs)
    out = assemble_output(cfg, meta, results)
    return out, times


def kernel(x, edge_index, W1, b1, W2, b2, Wl, bl):
    out, _ = run(FULL, dict(x=x, edge_index=edge_index, W1=W1, b1=b1,
                            W2=W2, b2=b2, Wl=Wl, bl=bl))
    return out



# revision 31
# speedup vs baseline: 26.4680x; 1.0071x over previous
"""Distributed 2-layer GCN (PyG GCNConv-style) on 8 Trainium2 NeuronCores.

Strategy (hardcoded for N=100000, E=3.2M, 512->256->128->4):
  - Nodes are degree-balanced into (ncores*W) windows of 128 nodes; window b is
    owned by core (b % ncores). A node's "global position" is its row in the
    AllGathered feature table, so gathers use plain int positions.
  - Per layer: local dense matmul (bf16 on PE, fp32 PSUM), rows pre-scaled by
    dinv, results AllGathered to a replicated bf16 feature table in DRAM.
  - Per layer the replicated feature table is split into 4 classes by
    src_window % 4 so table rows fit int16 gather indices AND each class's
    AllGather fires as soon as its quarter of windows is computed
    (class-major window order overlaps collectives with compute).
  - Aggregation: per (window, class) block of dst-sorted edges, a dma_gather
    (SWDGE, int16 indices) pulls source rows into SBUF; a one-hot S matrix
    built on DVE (is_equal vs iota, 4 tiles per batched op, bf16) feeds a
    PE matmul S^T @ msgs that segment-sums into the window's PSUM
    accumulator. Gathers stripe across 4 SWDGE queues (9.2 -> 4ns per
    descriptor on the Q7) and ops are split >=4 tiles (>=512 idx) to dodge
    the per-op fixed cost. Padding slots carry dcol=128 which never
    matches iota -> contribute 0.
  - Self loops are folded into the epilogue: agg = dinv*(acc + h'own) + b
    with h' windows retained in SBUF (saves ~100K gather descriptors).
  - Epilogue: relu; layer 2 repeats; final logits + log softmax.
  Profiled HW exec time on 8 cores: ~3.03 ms (baseline session start: 8.8 ms).
"""
import math
import numpy as np

import concourse.bass as bass
import concourse.mybir as mybir
import concourse.bass_utils as bass_utils
from concourse import bacc, tile
from concourse.bass_interp import get_hw_module

P = 128
F32 = mybir.dt.float32
BF16 = mybir.dt.bfloat16
I16 = mybir.dt.int16


class Cfg:
    def __init__(self, N, F_IN, H1, H2, C, ncores=8, W=None, maxt=8):
        self.N, self.F_IN, self.H1, self.H2, self.C = N, F_IN, H1, H2, C
        self.ncores = ncores
        B = ncores * P
        self.W = W if W is not None else math.ceil(N / B)
        self.NPAD = self.W * B
        assert self.NPAD >= N and self.NPAD % 4 == 0
        self.CLS = self.NPAD // 4
        assert self.CLS <= 32768
        self.KI = F_IN // P
        self.K2 = H1 // P
        self.maxt = maxt
        self.B = self.W * P  # nodes per core


FULL = Cfg(N=100000, F_IN=512, H1=256, H2=128, C=4)


# ---------------------------------------------------------------- host side
def preprocess(cfg, x, edge_index, W1, b1, W2, b2, Wl, bl):
    N, NC, W, NPAD, B = cfg.N, cfg.ncores, cfg.W, cfg.NPAD, cfg.B
    NW = NC * W

    src = np.asarray(edge_index[0], dtype=np.int64)
    dst = np.asarray(edge_index[1], dtype=np.int64)
    deg = np.bincount(dst, minlength=N).astype(np.float64) + 1.0
    dinv = (1.0 / np.sqrt(deg)).astype(np.float32)

    # node -> global position, degree-balanced across windows (snake fill)
    degall = np.zeros(NPAD, np.int64)
    degall[:N] = deg.astype(np.int64)
    order = np.argsort(-degall, kind="stable")
    i = np.arange(NPAD)
    phase = i % (2 * NW)
    binid = np.where(phase < NW, phase, 2 * NW - 1 - phase)
    by_bin = np.argsort(binid, kind="stable")
    slot = np.empty(NPAD, np.int64)
    slot[by_bin] = i % P  # within each bin, slots fill 0..127 in arrival order
    # position: core = bin % NC, window = bin // NC
    core_of_bin = binid % NC
    w_of_bin = binid // NC
    g_of_i = core_of_bin * B + w_of_bin * P + slot
    pos = np.empty(NPAD, np.int64)
    pos[order] = g_of_i

    node_at = np.empty(NPAD, np.int64)
    node_at[pos] = np.arange(NPAD)

    # edge list; self loops are folded into the epilogue on-device
    S_pos = pos[src]
    D_pos = pos[dst]
    core_e = D_pos // B
    w_e = (D_pos % B) // P
    dcol_e = (D_pos % P).astype(np.float32)
    # class tables: class = src_window % 4 (so each class's AllGather can
    # fire as soon as its quarter of windows is computed). Row in class
    # table = rank*B4cl + (w//4)*128 + slot.
    W_cl = [len(range(cl, W, 4)) for cl in range(4)]
    B4cl = [wc * P for wc in W_cl]
    s_slot = S_pos % P
    w_src = (S_pos % B) // P
    cls_e = (w_src % 4).astype(np.int64)
    b4_of = np.asarray(B4cl, np.int64)[cls_e]
    idx16_e = ((S_pos // B) * b4_of + (w_src // 4) * P
               + s_slot).astype(np.int16)

    key = ((core_e * W + w_e) * 4 + cls_e).astype(np.int64)
    ordE = np.argsort(key, kind="stable")
    counts = np.bincount(key, minlength=NC * W * 4).reshape(NC, W, 4)
    T = np.maximum(1, np.ceil(counts / P).astype(np.int64).max(axis=0))  # [W, 4]
    TT = int(T.sum())
    SLOT = TT * P

    # ops schedule per (w, cl): list of tile counts. Ops of >=4 tiles
    # (>=512 idx) pay no SWDGE fixed overhead; split e.g. 9 -> [5, 4].
    ops = [[[] for _ in range(4)] for _ in range(W)]
    for w in range(W):
        for cl in range(4):
            t = int(T[w][cl])
            while t > 0:
                c = min(t, cfg.maxt)
                if t > cfg.maxt and t - cfg.maxt < 4:
                    c = t - 4
                ops[w][cl].append(c)
                t -= c

    starts = np.zeros(NC * W * 4 + 1, np.int64)
    np.cumsum(counts.reshape(-1), out=starts[1:])
    blk_off = np.zeros((W, 4), np.int64)  # slot offset of each (w, cl) block
    acc_off = 0
    for w in range(W):
        for cl in range(4):
            blk_off[w, cl] = acc_off
            acc_off += int(T[w][cl]) * P

    idx16 = np.zeros((NC, SLOT), np.int16)
    dcol = np.full((NC, SLOT), float(P), np.float32)  # cast bf16 at ship time
    for c in range(NC):
        for w in range(W):
            for cl in range(4):
                k = (c * W + w) * 4 + cl
                s0, s1 = starts[k], starts[k + 1]
                n = s1 - s0
                off = blk_off[w, cl]
                seg = ordE[s0:s1]
                idx16[c, off:off + n] = idx16_e[seg]
                dcol[c, off:off + n] = dcol_e[seg]

    # wrap idx16 per-op: element i of an op at [i%16, i//16], replicated x8
    idx_w = np.zeros((NC, 16, SLOT // 16), np.int16)
    for w in range(W):
        for cl in range(4):
            off = int(blk_off[w, cl])
            for t_op in ops[w][cl]:
                n = t_op * P
                blk = idx16[:, off:off + n].reshape(NC, n // 16, 16)
                idx_w[:, :, off // 16:(off + n) // 16] = blk.transpose(0, 2, 1)
                off += n
    idx_rep = np.tile(idx_w, (1, 8, 1))  # [NC, 128, SLOT//16]

    dcol_t = dcol.reshape(NC, TT, P).transpose(0, 2, 1).copy()  # [NC, 128, TT]

    # x shard, transposed chunk layout: xt4[p, i, c2, m] = x[node(w*128+m), c2*128+p]
    # with i indexing windows in class-major (worder) order for batched loads.
    worder = [w for cl in range(4) for w in range(cl, W, 4)]
    xpad = np.zeros((NPAD, cfg.F_IN), np.float32)
    xpad[:N] = np.asarray(x, np.float32)
    dinvpad = np.ones(NPAD, np.float32)
    dinvpad[:N] = dinv

    xt4 = np.empty((NC, P, W, cfg.KI, P), np.float32)
    dinvl = np.empty((NC, P, W), np.float32)
    for c in range(NC):
        ids = node_at[c * B:(c + 1) * B]
        xl = xpad[ids]  # [B, F_IN]
        xt4[c] = xl.reshape(W, P, cfg.KI, P).transpose(3, 0, 2, 1)[:, worder]
        dinvl[c] = dinvpad[ids].reshape(W, P).T

    w1d = np.asarray(W1, np.float32).reshape(cfg.KI, P, cfg.H1).transpose(1, 0, 2)
    w2d = np.asarray(W2, np.float32).reshape(cfg.K2, P, cfg.H2).transpose(1, 0, 2)
    wld = np.asarray(Wl, np.float32)  # [H2=128, C]
    b1b = np.broadcast_to(np.asarray(b1, np.float32), (P, cfg.H1)).copy()
    b2b = np.broadcast_to(np.asarray(b2, np.float32), (P, cfg.H2)).copy()
    blb = np.broadcast_to(np.asarray(bl, np.float32), (P, cfg.C)).copy()
    iota4 = np.broadcast_to(np.arange(P, dtype=np.float32),
                            (P, 4, P)).reshape(P, 4 * P).copy()
    ident = np.eye(P, dtype=np.float32)

    import ml_dtypes
    bf = lambda a: a.astype(ml_dtypes.bfloat16)

    in_maps = []
    for c in range(NC):
        in_maps.append({
            "xt4": bf(xt4[c]),
            "w1d": bf(w1d), "w2d": bf(w2d), "wld": bf(wld),
            "b1b": b1b, "b2b": b2b, "blb": blb,
            "dinvl": dinvl[c],
            "idx16": idx_rep[c],
            "dcol": bf(dcol_t[c]),
            "iota": bf(iota4),
            "ident": bf(ident),
        })

    meta = dict(T=T, ops=ops, pos=pos, node_at=node_at, SLOT=SLOT, TT=TT)
    return in_maps, meta


def assemble_output(cfg, meta, results):
    N, NC, W, C, B = cfg.N, cfg.ncores, cfg.W, cfg.C, cfg.B
    rows = []
    for c in range(NC):
        r = results[c]["outst"].reshape(P, W, C).transpose(1, 0, 2).reshape(B, C)
        rows.append(r)
    allrows = np.concatenate(rows, axis=0)  # [NPAD, C] in position order
    return allrows[meta["pos"][:N]].astype(np.float32)


# ---------------------------------------------------------------- device side
def build_kernel(cfg, T, ops, upto="full"):
    NC, W, NPAD, B = cfg.ncores, cfg.W, cfg.NPAD, cfg.B
    H1, H2, C, KI, K2 = cfg.H1, cfg.H2, cfg.C, cfg.KI, cfg.K2
    TT = int(np.asarray(T).sum())
    SLOT = TT * P

    nc = bacc.Bacc("TRN2", target_bir_lowering=False, debug=False,
                   num_devices=NC, num_swdge_queues=4)

    xt4 = nc.dram_tensor("xt4", [P, W, KI, P], BF16, kind="ExternalInput")
    w1d = nc.dram_tensor("w1d", [P, KI, H1], BF16, kind="ExternalInput")
    w2d = nc.dram_tensor("w2d", [P, K2, H2], BF16, kind="ExternalInput")
    wld = nc.dram_tensor("wld", [P, C], BF16, kind="ExternalInput")
    b1b = nc.dram_tensor("b1b", [P, H1], F32, kind="ExternalInput")
    b2b = nc.dram_tensor("b2b", [P, H2], F32, kind="ExternalInput")
    blb = nc.dram_tensor("blb", [P, C], F32, kind="ExternalInput")
    dinvl = nc.dram_tensor("dinvl", [P, W], F32, kind="ExternalInput")
    idx16 = nc.dram_tensor("idx16", [P, SLOT // 16], I16, kind="ExternalInput")
    dcol = nc.dram_tensor("dcol", [P, TT], BF16, kind="ExternalInput")
    iota = nc.dram_tensor("iota", [P, 4 * P], BF16, kind="ExternalInput")
    ident = nc.dram_tensor("ident", [P, P], BF16, kind="ExternalInput")
    outst = nc.dram_tensor("outst", [P, W * C], F32, kind="ExternalOutput")

    # per-window column ranges in idx16 / dcol
    blk_tiles = np.asarray(T)  # [W, 4]
    w_tile_off = np.zeros(W + 1, np.int64)
    np.cumsum(blk_tiles.sum(axis=1), out=w_tile_off[1:])

    rg = [list(range(NC))]

    # class = src_window % 4; class-major window order so each class's
    # AllGather fires as soon as its quarter of windows is computed
    worder = [w for cl in range(4) for w in range(cl, W, 4)]
    W_cl = [len(range(cl, W, 4)) for cl in range(4)]
    B4cl = [wc * P for wc in W_cl]
    last_w_of_cl = {cl: [w for w in range(cl, W, 4)][-1] for cl in range(4)}
    with tile.TileContext(nc) as tc:
        with tc.tile_pool(name="const", bufs=1) as cpool, \
             tc.tile_pool(name="dram", bufs=1, space="DRAM") as dram:
            ag1_in = [dram.tile([B4cl[i], H1], BF16, name=f"ag1i{i}")
                      for i in range(4)]
            ag1_out = [dram.tile([NC * B4cl[i], H1], BF16, addr_space="Shared",
                                 name=f"ag1o{i}") for i in range(4)]
            ag2_in = [dram.tile([B4cl[i], H2], BF16, name=f"ag2i{i}")
                      for i in range(4)]
            ag2_out = [dram.tile([NC * B4cl[i], H2], BF16, addr_space="Shared",
                                 name=f"ag2o{i}") for i in range(4)]

            w1sb = cpool.tile([P, KI, H1], BF16)
            nc.sync.dma_start(w1sb[:], w1d[:])
            w2sb = cpool.tile([P, K2, H2], BF16)
            nc.sync.dma_start(w2sb[:], w2d[:])
            wlsb = cpool.tile([P, C], BF16)
            nc.sync.dma_start(wlsb[:], wld[:])
            b1sb = cpool.tile([P, H1], F32)
            nc.sync.dma_start(b1sb[:], b1b[:])
            b2sb = cpool.tile([P, H2], F32)
            nc.sync.dma_start(b2sb[:], b2b[:])
            blsb = cpool.tile([P, C], F32)
            nc.sync.dma_start(blsb[:], blb[:])
            dinvsb = cpool.tile([P, W], F32)
            nc.sync.dma_start(dinvsb[:], dinvl[:])
            iotasb = cpool.tile([P, 4, P], BF16)
            nc.sync.dma_start(iotasb[:], iota[:].rearrange("p (a b) -> p a b", a=4))
            idsb = cpool.tile([P, P], BF16)
            nc.sync.dma_start(idsb[:], ident[:])
            dcolsb = cpool.tile([P, TT], BF16)
            nc.sync.dma_start(dcolsb[:], dcol[:])

            lgst = cpool.tile([P, W * C], F32)
            sst = cpool.tile([P, W], F32)
            outsb = cpool.tile([P, W * C], F32)
            # retained h' windows for the self-loop epilogue term
            h1buf = cpool.tile([P, W, H1], BF16)
            h2buf = cpool.tile([P, W, H2], BF16)

            def allgather(cl, ag_in, ag_out):
                if NC == 1:
                    nc.sync.dma_start(ag_out[cl][:], ag_in[cl][:])
                else:
                    nc.gpsimd.collective_compute(
                        "AllGather", mybir.AluOpType.bypass,
                        ins=[ag_in[cl][:]], outs=[ag_out[cl][:]],
                        replica_groups=rg)

            # ---------------- phase A: h1' = dinv * (x @ W1), allgather per
            # class as soon as its windows are done (class-major worder)
            XB = 8  # windows per xt4 load
            with tc.tile_pool(name="phA", bufs=3) as sbA, \
                 tc.tile_pool(name="phA_ps", bufs=3, space="PSUM") as psA:
                for i0 in (range(0, W, XB) if upto != "noop" else []):
                    nb = min(XB, W - i0)
                    xt = sbA.tile([P, XB, KI, P], BF16, tag="xt")
                    nc.sync.dma_start(xt[:, :nb], xt4[:, i0:i0 + nb])
                    for k in range(nb):
                        w = worder[i0 + k]
                        cl = w % 4
                        hp = psA.tile([P, H1], F32, tag="hp")
                        for c2 in range(KI):
                            nc.tensor.matmul(hp[:], xt[:, k, c2], w1sb[:, c2],
                                             start=(c2 == 0),
                                             stop=(c2 == KI - 1))
                        # staging ring decouples the act->DMA chain from the
                        # persistent h1buf (tile-granular deps would
                        # serialize phase A on it)
                        h1p = sbA.tile([P, H1], BF16, tag="h1p", bufs=4)
                        nc.scalar.activation(h1p[:], hp[:],
                                             mybir.ActivationFunctionType.Copy,
                                             scale=dinvsb[:, w:w + 1])
                        nc.vector.tensor_copy(h1buf[:, w], h1p[:])
                        widx = w // 4
                        nc.sync.dma_start(
                            ag1_in[cl][widx * P:(widx + 1) * P, :],
                            h1p[:])
                        if w == last_w_of_cl[cl]:
                            allgather(cl, ag1_in, ag1_out)

            qctr = [0]
            # pre-set num_idxs registers once (else every gather emits a
            # GpSimd MOVE on the bottleneck engine)
            nreg = {}
            for w in range(W):
                for cl in range(4):
                    for t_op in ops[w][cl]:
                        if t_op not in nreg:
                            nreg[t_op] = nc.gpsimd.to_reg(t_op * P)

            def aggregate(w, sb, sbS, ps, cls_tabs, F, tag, gbufs=12):
                """Gather + S-matmul segment-sum for window w at width F.
                Returns the PSUM accumulator tile."""
                t0 = int(w_tile_off[w])
                ntile_w = int(w_tile_off[w + 1] - w_tile_off[w])
                idxw = sb.tile([P, ntile_w * 8], I16, tag=f"idxw{tag}", bufs=6)
                nc.sync.dma_start(idxw[:], idx16[:, t0 * 8:(t0 + ntile_w) * 8])
                acc = ps.tile([P, F], F32, tag=f"acc{tag}", bufs=3)
                # batched one-hot builds: 4 S tiles per DVE op
                stiles = []
                for bi in range(0, ntile_w, 4):
                    k = min(4, ntile_w - bi)
                    S4 = sbS.tile([P, 4, P], BF16, tag=f"S{tag}", bufs=10)
                    nc.vector.tensor_tensor(
                        S4[:, :k],
                        dcolsb[:, t0 + bi:t0 + bi + k].to_broadcast([P, k, P]),
                        iotasb[:, :k], op=mybir.AluOpType.is_equal)
                    for j in range(k):
                        stiles.append((S4, j))
                ti = 0
                for cl in range(4):
                    for t_op in ops[w][cl]:
                        g = sb.tile([P, cfg.maxt, F], BF16, tag=f"g{tag}",
                                    bufs=gbufs)
                        nc.gpsimd.dma_gather(
                            g[:, :t_op], cls_tabs[cl][:],
                            idxw[:, ti * 8:(ti + t_op) * 8],
                            t_op * P, nreg[t_op], F,
                            queue_num=qctr[0] % 4)
                        qctr[0] += 1
                        for t in range(t_op):
                            S4, j = stiles[ti + t]
                            nc.tensor.matmul(acc[:], S4[:, j], g[:, t],
                                             start=(ti + t == 0),
                                             stop=(ti + t == ntile_w - 1))
                        ti += t_op
                return acc

            # ---------------- phase C/D: aggregate layer1, h2' = dinv*(a1@W2)
            if upto.startswith("CD") or upto == "full":
                with tc.tile_pool(name="phC", bufs=4) as sbC, \
                     tc.tile_pool(name="phC_s", bufs=6) as sbS, \
                     tc.tile_pool(name="phC_ps", bufs=2, space="PSUM") as psC, \
                     tc.tile_pool(name="phD_ps", bufs=2, space="PSUM") as psD:
                    for w in worder:
                        acc = aggregate(w, sbC, sbS, psC, ag1_out, H1, "1", gbufs=14)
                        # self loop: agg = dinv*(acc + h1') ; then + b, relu
                        zs = sbC.tile([P, H1], F32, tag="zs")
                        nc.vector.tensor_tensor(zs[:], acc[:], h1buf[:, w],
                                                op=mybir.AluOpType.add)
                        z = sbC.tile([P, H1], F32, tag="z")
                        nc.scalar.activation(z[:], zs[:],
                                             mybir.ActivationFunctionType.Copy,
                                             scale=dinvsb[:, w:w + 1])
                        z2 = sbC.tile([P, H1], F32, tag="z2")
                        nc.vector.tensor_tensor(z2[:], z[:], b1sb[:],
                                                op=mybir.AluOpType.add)
                        a1 = sbC.tile([P, H1], BF16, tag="a1")
                        nc.scalar.activation(a1[:], z2[:],
                                             mybir.ActivationFunctionType.Relu)
                        h2p = psD.tile([P, H2], F32, tag="h2p")
                        for c2 in range(K2):
                            a1tp = psD.tile([P, P], BF16, tag="a1tp")
                            nc.tensor.transpose(a1tp[:], a1[:, c2 * P:(c2 + 1) * P],
                                                idsb[:])
                            a1t = sbC.tile([P, P], BF16, tag="a1t")
                            nc.vector.tensor_copy(a1t[:], a1tp[:])
                            nc.tensor.matmul(h2p[:], a1t[:], w2sb[:, c2],
                                             start=(c2 == 0), stop=(c2 == K2 - 1))
                        nc.scalar.activation(h2buf[:, w], h2p[:],
                                             mybir.ActivationFunctionType.Copy,
                                             scale=dinvsb[:, w:w + 1])
                        cl = w % 4
                        widx = w // 4
                        nc.sync.dma_start(
                            ag2_in[cl][widx * P:(widx + 1) * P, :],
                            h2buf[:, w])
                        if w == last_w_of_cl[cl]:
                            allgather(cl, ag2_in, ag2_out)

            # ---------------- phase E/F: aggregate layer2, logits, log_softmax
            if upto == "full":
                with tc.tile_pool(name="phE", bufs=4) as sbE, \
                     tc.tile_pool(name="phE_s", bufs=6) as sbS2, \
                     tc.tile_pool(name="phE_ps", bufs=2, space="PSUM") as psE, \
                     tc.tile_pool(name="phL_ps", bufs=2, space="PSUM") as psL:
                    for w in worder:
                        acc = aggregate(w, sbE, sbS2, psE, ag2_out, H2, "2")
                        zs = sbE.tile([P, H2], F32, tag="zse")
                        nc.vector.tensor_tensor(zs[:], acc[:], h2buf[:, w],
                                                op=mybir.AluOpType.add)
                        z = sbE.tile([P, H2], F32, tag="ze")
                        nc.scalar.activation(z[:], zs[:],
                                             mybir.ActivationFunctionType.Copy,
                                             scale=dinvsb[:, w:w + 1])
                        z2 = sbE.tile([P, H2], F32, tag="z2e")
                        nc.vector.tensor_tensor(z2[:], z[:], b2sb[:],
                                                op=mybir.AluOpType.add)
                        a2 = sbE.tile([P, H2], BF16, tag="a2")
                        nc.scalar.activation(a2[:], z2[:],
                                             mybir.ActivationFunctionType.Relu)
                        a2tp = psL.tile([P, P], BF16, tag="a2tp")
                        nc.tensor.transpose(a2tp[:], a2[:], idsb[:])
                        a2t = sbE.tile([P, P], BF16, tag="a2t")
                        nc.vector.tensor_copy(a2t[:], a2tp[:])
                        lg = psL.tile([P, C], F32, tag="lg")
                        nc.tensor.matmul(lg[:], a2t[:], wlsb[:], start=True, stop=True)
                        nc.vector.tensor_tensor(lgst[:, w * C:(w + 1) * C], lg[:],
                                                blsb[:], op=mybir.AluOpType.add)
                        e = sbE.tile([P, C], F32, tag="e")
                        nc.scalar.activation(e[:], lgst[:, w * C:(w + 1) * C],
                                             mybir.ActivationFunctionType.Exp,
                                             accum_out=sst[:, w:w + 1])
                    lns = cpool.tile([P, W], F32)
                    nc.scalar.activation(lns[:], sst[:],
                                         mybir.ActivationFunctionType.Ln)
                    for w in range(W):
                        nc.vector.tensor_scalar(
                            outsb[:, w * C:(w + 1) * C], lgst[:, w * C:(w + 1) * C],
                            lns[:, w:w + 1], None, op0=mybir.AluOpType.subtract)
                    nc.sync.dma_start(outst[:], outsb[:])
            else:
                # debug variants: dummy output proving the kept phases ran
                nc.vector.memset(outsb[:], 0.0)
                if upto != "noop":
                    probe_src = ag1_out[0] if upto == "A" else ag2_out[0]
                    probe = cpool.tile([P, C], BF16)
                    nc.sync.dma_start(probe[:], probe_src[:P, :C])
                    nc.vector.tensor_copy(outsb[:, :C], probe[:])
                nc.sync.dma_start(outst[:], outsb[:])

    nc.compile()
    return nc

# ---------------------------------------------------------------- entry point
_CACHE = {}


def _get_compiled(cfg, key, T, ops):
    if key not in _CACHE:
        nc = build_kernel(cfg, T, ops)
        nc.m = get_hw_module(nc.m)
        _CACHE[key] = nc
    return _CACHE[key]


def run(cfg, inputs):
    in_maps, meta = preprocess(cfg, **inputs)
    key = (cfg.N, cfg.F_IN, meta["TT"])
    nc = _get_compiled(cfg, key, meta["T"], meta["ops"])
    res = bass_utils.run_bass_kernel_spmd(
        nc, in_maps, core_ids=list(range(cfg.ncores)))
    out = assemble_output(cfg, meta, res.results)
    return out, res


class _TimedRunner:
    """PJRT runner mirroring bass2jax.run_bass_via_pjrt's multi-core branch,
    but with a cached jit and device-resident inputs for repeatable timing."""

    def __init__(self, nc, n_cores):
        import jax
        import concourse.mybir as mb
        from concourse import bass2jax
        from jax.sharding import Mesh, PartitionSpec, NamedSharding
        from jax.experimental.shard_map import shard_map

        bass2jax.install_neuronx_cc_hook()
        partition_name = (nc.partition_id_tensor.name
                          if nc.partition_id_tensor else None)
        in_names, out_names, out_avals, zero_shapes = [], [], [], []
        for alloc in nc.m.functions[0].allocations:
            if not isinstance(alloc, mb.MemoryLocationSet):
                continue
            name = alloc.memorylocations[0].name
            if alloc.kind == "ExternalInput":
                if name != partition_name:
                    in_names.append(name)
            elif alloc.kind == "ExternalOutput":
                out_names.append(name)
                shape = tuple(alloc.tensor_shape)
                dtype = mb.dt.np(alloc.dtype)
                out_avals.append(jax.core.ShapedArray(shape, dtype))
                zero_shapes.append((shape, dtype))
        n_params = len(in_names)
        all_in_names = list(in_names) + list(out_names)
        if partition_name is not None:
            all_in_names.append(partition_name)
        donate = tuple(range(n_params, n_params + len(out_names)))

        def _body(*args):
            operands = list(args)
            if partition_name is not None:
                operands.append(bass2jax.partition_id_tensor())
            outs = bass2jax._bass_exec_p.bind(
                *operands,
                out_avals=tuple(out_avals),
                in_names=tuple(all_in_names),
                out_names=tuple(out_names),
                lowering_input_output_aliases=(),
                sim_require_finite=True,
                sim_require_nnan=True,
                nc=nc,
            )
            return tuple(outs)

        devices = jax.devices()[:n_cores]
        mesh = Mesh(np.asarray(devices), ("core",))
        in_specs = (PartitionSpec("core"),) * (n_params + len(out_names))
        out_specs = (PartitionSpec("core"),) * len(out_names)
        self.fn = jax.jit(
            shard_map(_body, mesh=mesh, in_specs=in_specs,
                      out_specs=out_specs, check_rep=False),
            donate_argnums=donate, keep_unused=True)
        self.jax = jax
        self.mesh = mesh
        self.sharding = NamedSharding(mesh, PartitionSpec("core"))
        self.in_names = in_names
        self.out_names = out_names
        self.zero_shapes = zero_shapes
        self.n_cores = n_cores
        self.dev_inputs = None

    def stage_inputs(self, in_maps):
        concat_in = [
            np.concatenate([np.asarray(in_maps[c][n])
                            for c in range(self.n_cores)], axis=0)
            for n in self.in_names
        ]
        self.dev_inputs = [self.jax.device_put(a, self.sharding)
                           for a in concat_in]
        for a in self.dev_inputs:
            a.block_until_ready()

    def exec_once(self):
        import time
        zeros = [np.zeros((self.n_cores * s[0], *s[1:]), d)
                 for s, d in self.zero_shapes]
        dz = [self.jax.device_put(z, self.sharding) for z in zeros]
        for z in dz:
            z.block_until_ready()
        t0 = time.perf_counter()
        outs = self.fn(*self.dev_inputs, *dz)
        for o in outs:
            o.block_until_ready()
        t1 = time.perf_counter()
        return outs, t1 - t0

    def results(self, outs):
        res = []
        for c in range(self.n_cores):
            m = {}
            for i, n in enumerate(self.out_names):
                full = np.asarray(outs[i])
                per = full.reshape(self.n_cores, -1, *full.shape[1:])[c]
                m[n] = per
            res.append(m)
        return res


def run_timed(cfg, inputs, iters=3):
    in_maps, meta = preprocess(cfg, **inputs)
    key = (cfg.N, cfg.F_IN, meta["TT"])
    nc = _get_compiled(cfg, key, meta["T"], meta["ops"])
    rkey = ("runner",) + key
    if rkey not in _CACHE:
        _CACHE[rkey] = _TimedRunner(nc, cfg.ncores)
    runner = _CACHE[rkey]
    runner.stage_inputs(in_maps)
    times = []
    outs = None
    for _ in range(iters):
        outs, dt = runner.exec_once()
        times.append(dt)
    results = runner.results(outs)
    out = assemble_output(cfg, meta, results)
    return out, times


def kernel(x, edge_index, W1, b1, W2, b2, Wl, bl):
    out, _ = run(FULL, dict(x=x, edge_index=edge_index, W1=W1, b1=b1,
                            W2=W2, b2=b2, Wl=Wl, bl=bl))
    return out

